# revision 1
# baseline (speedup 1.0000x reference)
"""Decoder-layer Trainium2 kernel: 8-core SPMD, single launch, no collectives.

Sharding: core c -> (batch b = c // 2, sequence-half hf = c % 2). Each core
computes the full decoder layer for 512 query tokens of one sequence.
All cores run ONE identical program over a canonical virtual sequence of
1024 kv tokens with queries at virtual positions 512..1023; first-half cores
get their 512 real tokens placed at virtual 512..1023 with zero-padded kv
prefix and a `valid` vector that zeroes the pad contribution to the softmax
denominator.

v2 changes vs baseline:
- softmax denominators ride along in the ctx matmul via an augmented V
  (per head-pair V layout [Edims|Eden|Oden|zeros63|Odims], 193 wide): even
  heads matmul M=65 -> dims at psum rows 0..63 + den at row 64; odd heads
  M=128 with a zero block -> den at row 0 + dims at rows 64..127. Kills the
  65536 rows of separate [1,512] denominator matmuls.
- ctx matmuls are causally restricted to the live query range per kv tile
  (like scores), saving another 12288 rows.
- LN1 stats matmuls run inline as each head pair finishes its xres tile.
- fc2 + LN2 run in two token-half passes so the final normalize/store of
  half 0 overlaps the fc2 matmuls of half 1.
"""

import sys

sys.path.insert(0, "/opt/trn_rl_repo")

import math

import numpy as np
import ml_dtypes

import concourse.bass as bass
import concourse.mybir as mybir
from concourse.tile import TileContext, TilePool
from concourse.vector_clock import ScopedClock

BF16 = mybir.dt.bfloat16
F32 = mybir.dt.float32
AF = mybir.ActivationFunctionType
OP = mybir.AluOpType

B, L, D = 4, 1024, 1024
H, DH = 16, 64
DFF = 4 * D
P = 128
QTOK = 512  # query tokens per core
KV = 1024  # canonical kv length (virtual)
NKT = D // P  # 8 d-tiles
NOT1 = DFF // P  # 32 fc1 out tiles
MASK_NEG = -1.0e9
VW = 193  # augmented V width per head pair: [Ed 64|Eden|Oden|z 63|Od 64]

SELU_S = 1.0507009873554804934193349852946
SELU_A = 1.6732632423543772848170429916717
SELU_SA = SELU_S * SELU_A
LN_SA = math.log(SELU_SA)
LN_EPS = 1e-5


class PatchedTileContext(TileContext):
    """TileContext whose exit drain respects this walrus build's limit of
    ONE semaphore wait per instruction: the global-clock waits are spread
    across standalone NOPs and the butterfly barrier (whose sem-eq waits
    walrus rejects) is replaced by the NRT-expanded pseudo barrier."""

    def _drain_and_barrier(self, tick_clock, wait_clock):
        nc = self.nc
        carrier = nc.sync.nop()
        wait_clock.add_sem_waits(
            carrier.ins, ScopedClock({None: tick_clock.global_clock})
        )
        waits = list(carrier.ins.sync_info.on_wait)
        ups = list(carrier.ins.sync_info.on_update)
        if len(waits) > 1:
            carrier.ins.sync_info = mybir.SyncInfo(on_wait=[waits[0]], on_update=ups)
            for w in waits[1:]:
                extra = nc.sync.nop()
                extra.ins.sync_info = mybir.SyncInfo(on_wait=[w], on_update=[])
        for eng in nc.engines.values():
            eng.drain()
        nc._nrt_pseudo_barrier()
        popped = nc._tile_sem_poison_stack.pop()
        assert popped is self._sem_poison
        nc.clear_and_free_semaphores(list(self.sems.allocated().values()))
        nc._nrt_pseudo_barrier()


def _legalize_waits(nc):
    """This walrus build accepts at most ONE semaphore wait per instruction.
    Tile's sem-assignment can attach several; hoist the extras onto same-engine
    NOPs inserted immediately before the instruction (waits are a conjunction,
    so a sequence of single-wait stalls is equivalent)."""
    n = 0
    for fn in nc.m.functions:
        for blk in fn.blocks:
            out = []
            changed = False
            for inst in blk.instructions:
                si = getattr(inst, "sync_info", None)
                if si is not None and len(si.on_wait) > 1:
                    waits = list(si.on_wait)
                    for w in waits[:-1]:
                        nop = mybir.InstNoOp(name=f"waitnop_{n}", ins=[], outs=[])
                        n += 1
                        nop.engine = inst.engine
                        nop.sync_info = mybir.SyncInfo(on_wait=[w], on_update=[])
                        out.append(nop)
                    inst.sync_info = mybir.SyncInfo(
                        on_wait=[waits[-1]], on_update=list(si.on_update)
                    )
                    changed = True
                out.append(inst)
            if changed:
                blk.instructions = out
    return n


def _build_nc():
    nc = bass.Bass("TRN2", target_bir_lowering=False, debug=False, num_devices=8)

    def din(name, shape, dt):
        return nc.dram_tensor(name, shape, dt, kind="ExternalInput").ap()

    xt = din("xt", [P, NKT, KV], BF16)  # X[b].T tiled, virtual-padded
    xres = din("xres", [P, NKT, QTOK], F32)  # q tokens transposed, fp32
    valid16 = din("valid16", [P, NKT, NKT, 2], BF16)  # valid flag, [8hp x 2]
    wq = din("wq", [P, NKT, NKT, P], BF16)  # [dpart, ot, kt, o]
    wk = din("wk", [P, NKT, NKT, P], BF16)
    wv = din("wv", [P, NKT, D], BF16)  # rhs layout [dpart, kt, o]
    w1 = din("w1", [P, NOT1, NKT, P], BF16)
    w2 = din("w2", [P, NKT, NOT1, P], BF16)
    b1r = din("b1r", [P, NOT1], F32)  # SELU_S * b1
    b1e = din("b1e", [P, NOT1], F32)  # b1 + ln(SELU_S*SELU_A)
    b2t = din("b2t", [P, NKT], F32)
    g1t = din("g1t", [P, NKT], F32)
    be1t = din("be1t", [P, NKT], F32)
    g2t = din("g2t", [P, NKT], F32)
    be2t = din("be2t", [P, NKT], F32)
    out = nc.dram_tensor("out", [P, NKT, QTOK], F32, kind="ExternalOutput").ap()

    with PatchedTileContext(nc) as tc:
        import contextlib

        with contextlib.ExitStack() as ctx:
            persist = ctx.enter_context(tc.tile_pool(name="persist", bufs=1))
            bc = ctx.enter_context(tc.tile_pool(name="bc", bufs=1))
            wpool = ctx.enter_context(tc.tile_pool(name="wpool", bufs=3))
            tmp = ctx.enter_context(tc.tile_pool(name="tmp", bufs=2))
            tmp2 = ctx.enter_context(tc.tile_pool(name="tmp2", bufs=2))
            lnp = ctx.enter_context(tc.tile_pool(name="lnp", bufs=1))
            w1pool = ctx.enter_context(tc.tile_pool(name="w1pool", bufs=1))
            ps_mm = ctx.enter_context(tc.tile_pool(name="ps_mm", bufs=3, space="PSUM"))

            # ---- constants ----
            w1buf_a = w1pool.tile([P, 2, NKT, P], BF16, tag="w1A")
            w1buf_b = w1pool.tile([P, 2, NKT, P], BF16, tag="w1B")
            w1buf_c = w1pool.tile([P, 2, NKT, P], BF16, tag="w1C")
            w1bufs = [w1buf_a, w1buf_b, w1buf_c]
            w2buf_a = w1pool.tile([P, NOT1, P], BF16, tag="w2A")
            w2buf_b = w1pool.tile([P, NOT1, P], BF16, tag="w2B")
            w2buf_c = w1pool.tile([P, NOT1, P], BF16, tag="w2C")
            w2bufs = [w2buf_a, w2buf_b, w2buf_c]
            ones128 = persist.tile([P, P], BF16, tag="ones128")
            nc.gpsimd.memset(ones128[:], 1.0)
            ones_r0 = persist.tile([P, P], BF16, tag="ones_r0")
            nc.gpsimd.memset(ones_r0[:], 0.0)
            nc.gpsimd.memset(ones_r0[0:1, :], 1.0)
            ones_r64 = persist.tile([P, P], BF16, tag="ones_r64")
            nc.gpsimd.memset(ones_r64[:], 0.0)
            nc.gpsimd.memset(ones_r64[64:65, :], 1.0)
            srowE_bf = persist.tile([P, QTOK], BF16, tag="srowEbf")
            nc.vector.memset(srowE_bf[:], 0.0)
            srowO_bf = persist.tile([P, QTOK], BF16, tag="srowObf")
            nc.vector.memset(srowO_bf[:], 0.0)
            eps_ap = persist.tile([P, 1], F32, tag="eps")
            nc.gpsimd.memset(eps_ap[:], LN_EPS)

            def ln_stats_mm(ps0, ps1, cast_t, sq_t, kt, n=NKT, ncols=QTOK):
                nc.tensor.matmul(
                    ps0[:, 0:ncols],
                    ones128[:],
                    cast_t[:],
                    start=(kt == 0),
                    stop=(kt == n - 1),
                )
                nc.tensor.matmul(
                    ps1[:, 0:ncols],
                    ones128[:],
                    sq_t[:],
                    start=(kt == 0),
                    stop=(kt == n - 1),
                )

            def ln_meanvar(ps0, ps1, ncols=QTOK):
                """stats psums -> (mean, rstd) broadcast tiles."""
                mean_bc = bc.tile([P, ncols], F32, tag="mean")
                nc.vector.tensor_scalar_mul(mean_bc[:], ps0[:, 0:ncols], 1.0 / D)
                var_bc = bc.tile([P, ncols], F32, tag="var")
                nc.vector.tensor_scalar_mul(var_bc[:], ps1[:, 0:ncols], 1.0 / D)
                m2 = tmp2.tile([P, ncols], F32, tag="lnt")
                nc.vector.tensor_tensor(m2[:], mean_bc[:], mean_bc[:], OP.mult)
                nc.vector.tensor_tensor(var_bc[:], var_bc[:], m2[:], OP.subtract)
                nc.scalar.activation(var_bc[:], var_bc[:], AF.Sqrt, bias=eps_ap[:])
                nc.vector.reciprocal(var_bc[:], var_bc[:])  # now rstd
                return mean_bc, var_bc

            def ln_apply(src_kt, mean_bc, var_bc, g_ap, b_ap, dst_kt, kt, ncols=QTOK):
                t1 = tmp2.tile([P, ncols], F32, tag="lnt")
                nc.vector.tensor_tensor(t1[:], src_kt, mean_bc[:], OP.subtract)
                nc.vector.tensor_tensor(t1[:], t1[:], var_bc[:], OP.mult)
                if kt % 2 == 0:
                    nc.scalar.activation(
                        dst_kt,
                        t1[:],
                        AF.Identity,
                        scale=g_ap[:, kt : kt + 1],
                        bias=b_ap[:, kt : kt + 1],
                    )
                else:
                    nc.vector.tensor_scalar(
                        dst_kt,
                        t1[:],
                        g_ap[:, kt : kt + 1],
                        b_ap[:, kt : kt + 1],
                        OP.mult,
                        OP.add,
                    )

            # ---- phase 1+2 fused: QKV projections + attention ----
            with tc.tile_pool(name="pproj", bufs=1) as pproj:
                import contextlib as _ctl

                attn_stack = _ctl.ExitStack()
                pattn = attn_stack.enter_context(tc.tile_pool(name="pattn", bufs=3))
                ps_ctx = attn_stack.enter_context(
                    tc.tile_pool(name="ps_ctx", bufs=2, space="PSUM")
                )
                ps_b1 = attn_stack.enter_context(
                    tc.tile_pool(name="ps_b1", bufs=1, space="PSUM")
                )
                ps_fill = attn_stack.enter_context(
                    tc.tile_pool(name="ps_fill", bufs=2, space="PSUM")
                )
                pxstack = _ctl.ExitStack()
                px = pxstack.enter_context(tc.tile_pool(name="px", bufs=1))
                wq_t0 = wpool.tile([P, NKT, P], BF16, tag="wqkv")
                nc.sync.dma_start(out=wq_t0[:], in_=wq[:, 0])
                xt_s = px.tile([P, NKT, KV], BF16, tag="xt")
                for kt in range(NKT):
                    nc.sync.dma_start(out=xt_s[:, kt], in_=xt[:, kt])
                wk_t0 = wpool.tile([P, NKT, P], BF16, tag="wqkv")
                nc.sync.dma_start(out=wk_t0[:], in_=wk[:, 0])
                qt_s = pproj.tile([P, NKT, QTOK], BF16, tag="qt")
                kt_s = pproj.tile([P, NKT, KV], BF16, tag="kt")
                # augmented V: per (kv-tile j, head pair hp) 193 cols:
                # [Edims 64 | Eden 1 | Oden 1 | zeros 63 | Odims 64]
                vaug = pproj.tile([P, NKT, NKT, VW], BF16, tag="vaug")
                nc.gpsimd.memset(vaug[:, :, :, 66:129], 0.0)

                wv_s = px.tile([P, NKT, D], BF16, tag="wv")
                for c in range(2):
                    nc.sync.dma_start(
                        out=wv_s[:, 4 * c : 4 * (c + 1)], in_=wv[:, 4 * c : 4 * (c + 1)]
                    )
                val_s = pproj.tile([P, NKT, NKT, 2], BF16, tag="val")
                nc.sync.dma_start(out=val_s[:], in_=valid16[:])
                xres_s = pproj.tile([P, NKT, QTOK], F32, tag="xres")
                b1r_s = persist.tile([P, NOT1], F32, tag="b1r")
                b1e_s = persist.tile([P, NOT1], F32, tag="b1e")
                small = {}
                sum1_sb = pproj.tile([P, 512], F32, tag="sum1")
                sumsq1_sb = pproj.tile([P, 512], F32, tag="sumsq1")

                qk_done = set()
                v_done = {0: 0, 1: 0}

                def emit_qproj(ot):
                    if ot == 0:
                        wq_t = wq_t0
                    else:
                        wq_t = wpool.tile([P, NKT, P], BF16, tag="wqkv")
                        nc.sync.dma_start(out=wq_t[:], in_=wq[:, ot])
                    ps = ps_fill.tile([P, 512], F32, tag="fill")
                    for kt in range(NKT):
                        nc.tensor.matmul(
                            ps[:],
                            wq_t[:, kt],
                            xt_s[:, kt, 512:1024],
                            start=(kt == 0),
                            stop=(kt == NKT - 1),
                        )
                    nc.vector.tensor_copy(qt_s[:, ot], ps[:])

                wk_ts = {0: wk_t0}

                def emit_kproj(ot, tb):
                    if tb == 0 and ot not in wk_ts:
                        wk_t = wpool.tile([P, NKT, P], BF16, tag="wqkv")
                        nc.sync.dma_start(out=wk_t[:], in_=wk[:, ot])
                        wk_ts[ot] = wk_t
                    wk_t = wk_ts[ot]
                    ps = ps_fill.tile([P, 512], F32, tag="fill")
                    for kt in range(NKT):
                        nc.tensor.matmul(
                            ps[:],
                            wk_t[:, kt],
                            xt_s[:, kt, tb * 512 : (tb + 1) * 512],
                            start=(kt == 0),
                            stop=(kt == NKT - 1),
                        )
                    if tb == 0:
                        nc.scalar.copy(kt_s[:, ot, 0:512], ps[:])
                    else:
                        nc.vector.tensor_copy(kt_s[:, ot, 512:1024], ps[:])
                        qk_done.add(ot)

                def emit_vproj(db, tk):
                    if db == 0:
                        # den columns for all 8 pairs x 2 parities
                        nc.vector.tensor_copy(vaug[:, tk, :, 64:66], val_s[:, tk])
                    ps = ps_fill.tile([P, 4, P], F32, tag="fill")
                    for kt in range(NKT):
                        nc.tensor.matmul(
                            ps[:, :, :],
                            xt_s[:, kt, tk * P : (tk + 1) * P],
                            wv_s[:, kt, db * 512 : (db + 1) * 512],
                            start=(kt == 0),
                            stop=(kt == NKT - 1),
                        )
                    hp0 = db * 4
                    nc.vector.tensor_copy(
                        vaug[:, tk, hp0 : hp0 + 4, 0:64], ps[:, :, 0:64]
                    )
                    nc.vector.tensor_copy(
                        vaug[:, tk, hp0 : hp0 + 4, 129:193], ps[:, :, 64:128]
                    )
                    v_done[db] += 1

                stats_pending = []

                def flush_stats():
                    while stats_pending:
                        cast_t, sq_t, hp = stats_pending.pop(0)
                        pss = ps_mm.tile([P, 512], F32, tag="mm")
                        nc.tensor.matmul(
                            pss[:], ones128[:], cast_t[:], start=True, stop=True
                        )
                        psq = ps_mm.tile([P, 512], F32, tag="mm")
                        nc.tensor.matmul(
                            psq[:], ones128[:], sq_t[:], start=True, stop=True
                        )
                        if hp == 0:
                            nc.vector.tensor_copy(sum1_sb[:], pss[:])
                            nc.vector.tensor_copy(sumsq1_sb[:], psq[:])
                        else:
                            nc.vector.tensor_tensor(
                                sum1_sb[:], sum1_sb[:], pss[:], OP.add
                            )
                            nc.vector.tensor_tensor(
                                sumsq1_sb[:], sumsq1_sb[:], psq[:], OP.add
                            )

                def emit_post(p):
                    h, cps, ctxn = p
                    hp, par = h // 2, h % 2
                    po = 64 * par
                    flush_stats()
                    bcp = ps_b1.tile([P, 512], F32, tag="bc")
                    if par == 0:
                        nc.tensor.matmul(
                            bcp[0:64],
                            ones_r64[:, 0:64],
                            srowE_bf[:],
                            start=True,
                            stop=True,
                        )
                    else:
                        nc.tensor.matmul(
                            bcp[64:128],
                            ones_r0[:, 0:64],
                            srowO_bf[:],
                            start=True,
                            stop=True,
                        )
                    bc_sb = tmp2.tile([P, 512], BF16, tag="bcsb")
                    nc.vector.tensor_copy(bc_sb[po : po + 64], bcp[po : po + 64])
                    nc.vector.tensor_tensor(
                        ctxn[po : po + 64],
                        cps[po : po + 64],
                        bc_sb[po : po + 64],
                        OP.mult,
                    )
                    if par == 1:
                        # pair finished: residual add + LN1 stats (Pool
                        # takes the add + cast + square; PE the stats,
                        # deferred one slot so PE never waits on Pool).
                        # Last pair runs on DVE: Pool's 0.42 efficiency
                        # would sit on the attention->LN1 critical path.
                        eng = nc.vector if hp == H // 2 - 1 else nc.gpsimd
                        eng.tensor_add(
                            xres_s[:, hp], xres_s[:, hp], ctxn[:]
                        )
                        cast_t = tmp.tile([P, 512], BF16, tag="lncast")
                        sq_t = tmp.tile([P, 512], BF16, tag="lnsq")
                        eng.tensor_copy(cast_t[:], xres_s[:, hp])
                        eng.tensor_mul(
                            sq_t[:], xres_s[:, hp], xres_s[:, hp]
                        )
                        stats_pending.append((cast_t, sq_t, hp))

                ctxn = None

                def emit_ctx(pr):
                    nonlocal ctxn
                    h, expt = pr
                    hp, par = h // 2, h % 2
                    cps = ps_ctx.tile([P, 512], F32, tag="ctx")
                    lsl = (0, 65) if par == 0 else (65, VW)
                    m = lsl[1] - lsl[0]
                    for j in range(NKT):
                        off = max(0, j - 4) * P
                        nc.tensor.matmul(
                            cps[0:m, off:512],
                            vaug[:, j, hp, lsl[0] : lsl[1]],
                            expt[:, j, off:512],
                            start=(j == 0),
                            stop=(j == NKT - 1),
                        )
                    with nc.allow_low_precision(
                        reason="softmax denominator reciprocal to bf16"
                    ):
                        if par == 0:
                            nc.vector.reciprocal(srowE_bf[64:65], cps[64:65])
                        else:
                            nc.vector.reciprocal(srowO_bf[0:1], cps[0:1])
                    if par == 0:
                        ctxn = tmp2.tile([P, 512], F32, tag="ctxn")
                    return (h, cps, ctxn)

                def emit_scores(h):
                    hp, par = h // 2, h % 2
                    ot = hp
                    po = 64 * par
                    expt = pattn.tile([P, NKT, 512], BF16, tag="expt")
                    for j in range(NKT):
                        off = max(0, j - 4) * P
                        n = 512 - off
                        ps = ps_mm.tile([P, 512], F32, tag="mm")
                        nc.tensor.matmul(
                            ps[:, :n],
                            kt_s[po : po + 64, ot, j * P : (j + 1) * P],
                            qt_s[po : po + 64, ot, off:512],
                            start=True,
                            stop=True,
                        )
                        nc.scalar.activation(
                            expt[:, j, off:512],
                            ps[:, :n],
                            AF.Exp,
                            scale=0.125,
                        )
                        if j >= 4:
                            # zero the masked upper triangle of the diagonal
                            # query block post-exp (Pool, off the hot engines)
                            nc.gpsimd.affine_select(
                                out=expt[:, j, off : off + P],
                                in_=expt[:, j, off : off + P],
                                compare_op=OP.is_ge,
                                fill=0.0,
                                base=0,
                                pattern=[[1, P]],
                                channel_multiplier=-1,
                            )
                    return (h, expt)

                # filler units: Q/K projections + V-proj tiles, ordered by
                # consumption deadline, drained 2 per head slot
                fillers = []
                for ot in (1,):
                    fillers += [
                        lambda o=ot: emit_qproj(o),
                        lambda o=ot: emit_kproj(o, 0),
                        lambda o=ot: emit_kproj(o, 1),
                    ]
                fillers += [lambda t=tk: emit_vproj(0, t) for tk in range(NKT)]
                for ot in (2, 3, 4):
                    fillers += [
                        lambda o=ot: emit_qproj(o),
                        lambda o=ot: emit_kproj(o, 0),
                        lambda o=ot: emit_kproj(o, 1),
                    ]
                fillers += [lambda t=tk: emit_vproj(1, t) for tk in range(NKT)]
                for ot in (5, 6, 7):
                    fillers += [
                        lambda o=ot: emit_qproj(o),
                        lambda o=ot: emit_kproj(o, 0),
                        lambda o=ot: emit_kproj(o, 1),
                    ]
                fillers.reverse()  # consume via pop()

                def drain(n):
                    for _ in range(n):
                        if fillers:
                            fillers.pop()()

                def need_qk(hp):
                    while hp not in qk_done:
                        assert fillers, f"filler queue dry before qk {hp}"
                        fillers.pop()()

                def need_v(db):
                    while v_done[db] < NKT:
                        assert fillers, f"filler queue dry before v {db}"
                        fillers.pop()()

                # warmup: pair-0 inputs, then 3 heads of scores while the
                # filler queue builds V/QK state; first ctx after V db0 done
                emit_qproj(0)
                emit_kproj(0, 0)
                emit_kproj(0, 1)
                from collections import deque

                prevs = deque()
                prevs.append(emit_scores(0))
                drain(5)
                prevs.append(emit_scores(1))
                nc.sync.dma_start(out=xres_s[:], in_=xres[:])
                drain(5)
                need_qk(1)
                prevs.append(emit_scores(2))
                drain(4)
                need_v(0)
                nc.sync.dma_start(out=b1r_s[:], in_=b1r[:])
                nc.sync.dma_start(out=b1e_s[:], in_=b1e[:])
                for nm, _src in (
                    ("b2t", b2t),
                    ("g1t", g1t),
                    ("be1t", be1t),
                    ("g2t", g2t),
                    ("be2t", be2t),
                ):
                    t = persist.tile([P, NKT], F32, tag=nm)
                    nc.sync.dma_start(out=t[:], in_=_src[:])
                    small[nm] = t
                pending = emit_ctx(prevs.popleft())
                for h in range(3, H):
                    need_qk(h // 2)
                    prevs.append(emit_scores(h))
                    drain(1)
                    emit_post(pending)
                    nh = prevs[0][0]
                    need_v(nh // 16 if False else (nh // 2) // 4)
                    pending = emit_ctx(prevs.popleft())
                drain(len(fillers))
                while prevs:
                    emit_post(pending)
                    need_v(1)
                    pending = emit_ctx(prevs.popleft())
                emit_post(pending)
                flush_stats()

                # ---- phase 3: LN1 (stats already accumulated) ----
                ln1_bf = lnp.tile([P, NKT, QTOK], BF16, tag="ln1")
                mean1, rstd1 = ln_meanvar(sum1_sb, sumsq1_sb)
                for kt in range(NKT):
                    ln_apply(
                        xres_s[:, kt], mean1, rstd1,
                        small["g1t"], small["be1t"], ln1_bf[:, kt], kt,
                    )
                pxstack.close()
                attn_stack.close()

            # ---- phase 4: fc1 + selu (w1 in JIT 4-ot chunks, depth 2) ----
            pffn_stack = contextlib.ExitStack()
            pffn = pffn_stack.enter_context(tc.tile_pool(name="pffn", bufs=1))
            ps_x = pffn_stack.enter_context(
                tc.tile_pool(name="ps_x", bufs=1, space="PSUM")
            )
            h1_bf = pffn.tile([P, NOT1, QTOK], BF16, tag="h1")
            for ot in range(NOT1):
                if ot % 2 == 0:
                    w1c = w1bufs[(ot // 2) % 3]
                    nc.sync.dma_start(out=w1c[:], in_=w1[:, ot : ot + 2])
                if ot % 4 == 3:
                    ps = ps_x.tile([P, 512], F32, tag="x")
                else:
                    ps = ps_mm.tile([P, 512], F32, tag="mm")
                for kt in range(NKT):
                    nc.tensor.matmul(
                        ps[:],
                        w1c[:, ot % 2, kt],
                        ln1_bf[:, kt],
                        start=(kt == 0),
                        stop=(kt == NKT - 1),
                    )
                p_t = tmp.tile([P, QTOK], F32, tag="selup")
                nc.scalar.activation(
                    p_t[:],
                    ps[:],
                    AF.Relu,
                    scale=SELU_S,
                    bias=b1r_s[:, ot : ot + 1],
                )
                e_t = tmp.tile([P, QTOK], F32, tag="selue")
                nc.scalar.activation(
                    e_t[:], ps[:], AF.Exp, bias=b1e_s[:, ot : ot + 1]
                )
                nc.vector.tensor_scalar(
                    e_t[:], e_t[:], SELU_SA, 0.0, OP.subtract, OP.min
                )
                nc.vector.tensor_tensor(h1_bf[:, ot], p_t[:], e_t[:], OP.add)

            # ---- phase 5: fc2 + residual + LN2 + store, two token halves ----
            ps_stat2 = pffn_stack.enter_context(
                tc.tile_pool(name="ps_stat2", bufs=4, space="PSUM")
            )
            res2 = pffn.tile([P, NKT, QTOK], F32, tag="res2")
            HT = QTOK // 2  # 256-token half
            for tb in range(2):
                c0, c1 = tb * HT, (tb + 1) * HT
                ps0_2 = ps_stat2.tile([P, 512], F32, tag="stat2")
                ps1_2 = ps_stat2.tile([P, 512], F32, tag="stat2")
                for ot in range(NKT):
                    w2_t = w2bufs[(tb * NKT + ot) % 3]
                    nc.sync.dma_start(out=w2_t[:], in_=w2[:, ot])
                    if ot % 4 == 3:
                        ps = ps_x.tile([P, 512], F32, tag="x")
                    else:
                        ps = ps_mm.tile([P, 512], F32, tag="mm")
                    for kt in range(NOT1):
                        nc.tensor.matmul(
                            ps[:, 0:HT],
                            w2_t[:, kt],
                            h1_bf[:, kt, c0:c1],
                            start=(kt == 0),
                            stop=(kt == NOT1 - 1),
                        )
                    t1 = tmp2.tile([P, HT], F32, tag="r2t")
                    nc.vector.tensor_tensor(
                        t1[:], ps[:, 0:HT], ln1_bf[:, ot, c0:c1], OP.add
                    )
                    nc.scalar.activation(
                        res2[:, ot, c0:c1],
                        t1[:],
                        AF.Identity,
                        bias=small["b2t"][:, ot : ot + 1],
                    )
                    cast_t = tmp.tile([P, HT], BF16, tag="lncast2")
                    sq_t = tmp.tile([P, HT], BF16, tag="lnsq2")
                    nc.vector.tensor_copy(cast_t[:], res2[:, ot, c0:c1])
                    nc.scalar.activation(sq_t[:], res2[:, ot, c0:c1], AF.Square)
                    ln_stats_mm(ps0_2, ps1_2, cast_t, sq_t, ot, ncols=HT)
                mean2, rstd2 = ln_meanvar(ps0_2, ps1_2, ncols=HT)
                for kt in range(NKT):
                    ln_apply(
                        res2[:, kt, c0:c1], mean2, rstd2,
                        small["g2t"], small["be2t"], res2[:, kt, c0:c1], kt,
                        ncols=HT,
                    )
                    nc.sync.dma_start(out=out[:, kt, c0:c1], in_=res2[:, kt, c0:c1])
            pffn_stack.close()

    _legalize_waits(nc)
    return nc


_NC_CACHE = None
TRACE = False
LAST_EXEC_NS = None


def _get_nc():
    global _NC_CACHE
    if _NC_CACHE is None:
        _NC_CACHE = _build_nc()
    return _NC_CACHE


def _tile_w(a):
    """[Din, O] -> [P, O//P(ot), Din//P(kt), P] with ot-contiguous DMA slices."""
    Din, O = a.shape
    return np.ascontiguousarray(
        a.reshape(Din // P, P, O // P, P).transpose(1, 2, 0, 3)
    )


def _pp(v, n):
    """[n*P] -> [P, n] per-partition layout."""
    return np.ascontiguousarray(v.reshape(n, P).T)


def kernel(X, wq, wk, wv, ln1_g, ln1_b, w1, b1, w2, b2, ln2_g, ln2_b):
    from concourse.bass_utils import run_bass_kernel_spmd

    X = np.asarray(X, np.float32)
    bf = ml_dtypes.bfloat16
    wqT = _tile_w(np.asarray(wq, np.float32).T).astype(bf)
    wkT = _tile_w(np.asarray(wk, np.float32).T).astype(bf)
    wvT = np.ascontiguousarray(
        np.asarray(wv, np.float32).T.reshape(NKT, P, D).transpose(1, 0, 2)
    ).astype(bf)
    w1T = _tile_w(np.asarray(w1, np.float32).T).astype(bf)
    w2T = _tile_w(np.asarray(w2, np.float32).T).astype(bf)
    b1 = np.asarray(b1, np.float32)
    shared = dict(
        wq=wqT,
        wk=wkT,
        wv=wvT,
        w1=w1T,
        w2=w2T,
        b1r=_pp(SELU_S * b1, NOT1),
        b1e=_pp(b1 + LN_SA, NOT1),
        b2t=_pp(np.asarray(b2, np.float32), NKT),
        g1t=_pp(np.asarray(ln1_g, np.float32), NKT),
        be1t=_pp(np.asarray(ln1_b, np.float32), NKT),
        g2t=_pp(np.asarray(ln2_g, np.float32), NKT),
        be2t=_pp(np.asarray(ln2_b, np.float32), NKT),
    )

    in_maps = []
    for c in range(8):
        b, hf = c // 2, c % 2
        if hf == 1:
            xkv = X[b].T  # [D, L]
            valid = np.ones(KV, np.float32)
            xq = X[b, 512:]
        else:
            xkv = np.concatenate(
                [np.zeros((D, 512), np.float32), X[b, :512].T], axis=1
            )
            valid = np.concatenate(
                [np.zeros(512, np.float32), np.ones(512, np.float32)]
            )
            xq = X[b, :512]
        xt = (
            np.ascontiguousarray(xkv.reshape(NKT, P, KV).transpose(1, 0, 2))
        ).astype(bf)
        xres = np.ascontiguousarray(xq.T.reshape(NKT, P, QTOK).transpose(1, 0, 2))
        vt = valid.reshape(NKT, P).T  # [P, NKT]
        val16 = (
            np.repeat(vt[:, :, None], H, axis=2).reshape(P, NKT, NKT, 2).astype(bf)
        )
        m = dict(shared)
        m.update(xt=xt, xres=xres, valid16=np.ascontiguousarray(val16))
        in_maps.append(m)

    nc = _get_nc()
    global LAST_EXEC_NS
    if TRACE:
        res = run_bass_kernel_spmd(nc, in_maps, list(range(8)), trace=True)
        LAST_EXEC_NS = res.exec_time_ns
    else:
        res = run_bass_kernel_spmd(nc, in_maps, list(range(8)))

    out = np.empty((B, L, D), np.float32)
    for c in range(8):
        b, hf = c // 2, c % 2
        o = res.results[c]["out"]  # [P, NKT, QTOK]
        o = o.transpose(1, 0, 2).reshape(D, QTOK).T  # [QTOK, D]
        out[b, hf * 512 : hf * 512 + 512] = o
    return out



# revision 49
# speedup vs baseline: 1.0990x; 1.0990x over previous
"""Decoder-layer Trainium2 kernel: 8-core SPMD, single launch, no collectives.

Sharding: core c -> (batch b = c // 2, sequence-half hf = c % 2). Each core
computes the full decoder layer for 512 query tokens of one sequence.
All cores run ONE identical program over a canonical virtual sequence of
1024 kv tokens with queries at virtual positions 512..1023; first-half cores
get their 512 real tokens placed at virtual 512..1023 with zero-padded kv
prefix and a `valid` vector that zeroes the pad contribution to the softmax
denominator.

v2 changes vs baseline:
- softmax denominators ride along in the ctx matmul via an augmented V
  (per head-pair V layout [Edims|Eden|Oden|zeros63|Odims], 193 wide): even
  heads matmul M=65 -> dims at psum rows 0..63 + den at row 64; odd heads
  M=128 with a zero block -> den at row 0 + dims at rows 64..127. Kills the
  65536 rows of separate [1,512] denominator matmuls.
- ctx matmuls are causally restricted to the live query range per kv tile
  (like scores), saving another 12288 rows.
- LN1 stats matmuls run inline as each head pair finishes its xres tile.
- fc2 + LN2 run in two token-half passes so the final normalize/store of
  half 0 overlaps the fc2 matmuls of half 1.
"""

import sys

sys.path.insert(0, "/opt/trn_rl_repo")

import math

import numpy as np
import ml_dtypes

import concourse.bass as bass
import concourse.mybir as mybir
from concourse.tile import TileContext, TilePool
from concourse.vector_clock import ScopedClock

BF16 = mybir.dt.bfloat16
F8 = mybir.dt.float8e4
F32 = mybir.dt.float32
AF = mybir.ActivationFunctionType
OP = mybir.AluOpType
DR = mybir.MatmulPerfMode.DoubleRow
WS = 64.0  # fp8 weight pre-scale (wv/w1/w2)
# Q/K projections use a smaller pre-scale: bass float8e4 is IEEE e4m3
# (max finite 240, saturates to inf) and |K|*64 reaches ~290 on some
# batches; *32 keeps the fp8 Q/K copies comfortably finite.
WSQK = 32.0

B, L, D = 4, 1024, 1024
H, DH = 16, 64
DFF = 4 * D
P = 128
QTOK = 512  # query tokens per core
KV = 1024  # canonical kv length (virtual)
NKT = D // P  # 8 d-tiles
NOT1 = DFF // P  # 32 fc1 out tiles
MASK_NEG = -1.0e9
VW = 193  # augmented V width per head pair: [Ed 64|Eden|Oden|z 63|Od 64]

SELU_S = 1.0507009873554804934193349852946
SELU_A = 1.6732632423543772848170429916717
SELU_SA = SELU_S * SELU_A
LN_SA = math.log(SELU_SA)
LN_EPS = 1e-5


class PatchedTileContext(TileContext):
    """TileContext whose exit drain respects this walrus build's limit of
    ONE semaphore wait per instruction: the global-clock waits are spread
    across standalone NOPs and the butterfly barrier (whose sem-eq waits
    walrus rejects) is replaced by the NRT-expanded pseudo barrier."""

    def _drain_and_barrier(self, tick_clock, wait_clock):
        nc = self.nc
        carrier = nc.sync.nop()
        wait_clock.add_sem_waits(
            carrier.ins, ScopedClock({None: tick_clock.global_clock})
        )
        waits = list(carrier.ins.sync_info.on_wait)
        ups = list(carrier.ins.sync_info.on_update)
        if len(waits) > 1:
            carrier.ins.sync_info = mybir.SyncInfo(on_wait=[waits[0]], on_update=ups)
            for w in waits[1:]:
                extra = nc.sync.nop()
                extra.ins.sync_info = mybir.SyncInfo(on_wait=[w], on_update=[])
        for eng in nc.engines.values():
            eng.drain()
        nc._nrt_pseudo_barrier()
        popped = nc._tile_sem_poison_stack.pop()
        assert popped is self._sem_poison
        nc.clear_and_free_semaphores(list(self.sems.allocated().values()))
        nc._nrt_pseudo_barrier()


def _legalize_waits(nc):
    """This walrus build accepts at most ONE semaphore wait per instruction.
    Tile's sem-assignment can attach several; hoist the extras onto same-engine
    NOPs inserted immediately before the instruction (waits are a conjunction,
    so a sequence of single-wait stalls is equivalent)."""
    n = 0
    for fn in nc.m.functions:
        for blk in fn.blocks:
            out = []
            changed = False
            for inst in blk.instructions:
                si = getattr(inst, "sync_info", None)
                if si is not None and len(si.on_wait) > 1:
                    waits = list(si.on_wait)
                    for w in waits[:-1]:
                        nop = mybir.InstNoOp(name=f"waitnop_{n}", ins=[], outs=[])
                        n += 1
                        nop.engine = inst.engine
                        nop.sync_info = mybir.SyncInfo(on_wait=[w], on_update=[])
                        out.append(nop)
                    inst.sync_info = mybir.SyncInfo(
                        on_wait=[waits[-1]], on_update=list(si.on_update)
                    )
                    changed = True
                out.append(inst)
            if changed:
                blk.instructions = out
    return n


DEBUG_TAPS = False
DBG_HEAD = 0


def _build_nc():
    nc = bass.Bass("TRN2", target_bir_lowering=False, debug=False, num_devices=8)

    def din(name, shape, dt):
        return nc.dram_tensor(name, shape, dt, kind="ExternalInput").ap()

    xt = din("xt", [P, NKT, KV], F8)  # X[b].T tiled, virtual-padded
    xres = din("xres", [P, NKT, QTOK], F32)  # q tokens transposed, fp32
    valid16 = din("valid16", [P, NKT, NKT, 2], BF16)  # WS flag, [8hp x 2]
    wq = din("wq", [P, NKT, NKT, P], F8)  # [dpart, ot, kt, o], x WS
    wk = din("wk", [P, NKT, NKT, P], F8)
    wv = din("wv", [P, NKT, D], F8)  # rhs layout [dpart, kt, o], x WS
    w1 = din("w1", [P, NOT1, NKT, P], F8)  # fp8(WS*w1^T)
    w1e = din("w1e", [P, NOT1, NKT, P], F8)  # fp8 residual of the above
    w2 = din("w2", [P, NKT, NOT1, P], F8)
    w2e = din("w2e", [P, NKT, NOT1, P], F8)
    b1r = din("b1r", [P, NOT1], F32)  # SELU_S * b1
    b1e = din("b1e", [P, NOT1], F32)  # b1 + ln(SELU_S*SELU_A)
    b2t = din("b2t", [P, NKT], F32)
    g1t = din("g1t", [P, NKT], F32)
    be1t = din("be1t", [P, NKT], F32)
    g2t = din("g2t", [P, NKT], F32)
    be2t = din("be2t", [P, NKT], F32)
    out = nc.dram_tensor("out", [P, NKT, QTOK], F32, kind="ExternalOutput").ap()

    with PatchedTileContext(nc) as tc:
        import contextlib

        with contextlib.ExitStack() as ctx:
            persist = ctx.enter_context(tc.tile_pool(name="persist", bufs=1))
            bc = ctx.enter_context(tc.tile_pool(name="bc", bufs=1))
            wpool = ctx.enter_context(tc.tile_pool(name="wpool", bufs=3))
            tmp = ctx.enter_context(tc.tile_pool(name="tmp", bufs=2))
            tmp2 = ctx.enter_context(tc.tile_pool(name="tmp2", bufs=2))
            lnp = ctx.enter_context(tc.tile_pool(name="lnp", bufs=1))
            w1pool = ctx.enter_context(tc.tile_pool(name="w1pool", bufs=1))

            # ---- constants ----
            NW1B, NW2B = 5, 4
            w1bufs = [
                (
                    w1pool.tile(
                        [P, 2, NKT, P], F8, tag=f"w1{i}h", name=f"w1{i}h"
                    ),
                    w1pool.tile(
                        [P, 2, NKT, P], F8, tag=f"w1{i}e", name=f"w1{i}e"
                    ),
                )
                for i in range(NW1B)
            ]
            w2bufs = [
                (
                    w1pool.tile(
                        [P, NOT1, P], F8, tag=f"w2{i}h", name=f"w2{i}h"
                    ),
                    w1pool.tile(
                        [P, NOT1, P], F8, tag=f"w2{i}e", name=f"w2{i}e"
                    ),
                )
                for i in range(NW2B)
            ]

            def load_w1(chunk):
                if chunk < NOT1 // 2:
                    hb, lb = w1bufs[chunk % NW1B]
                    nc.sync.dma_start(out=hb[:], in_=w1[:, 2 * chunk : 2 * chunk + 2])
                    nc.sync.dma_start(out=lb[:], in_=w1e[:, 2 * chunk : 2 * chunk + 2])

            def load_w2(ot):
                if ot < NKT:
                    hb, lb = w2bufs[ot % NW2B]
                    nc.sync.dma_start(out=hb[:], in_=w2[:, ot])
                    nc.sync.dma_start(out=lb[:], in_=w2e[:, ot])
            ones128 = persist.tile([P, P], BF16, tag="ones128")
            nc.gpsimd.memset(ones128[:], 1.0)
            ones_r0 = persist.tile([P, P], BF16, tag="ones_r0")
            nc.gpsimd.memset(ones_r0[:], 0.0)
            nc.gpsimd.memset(ones_r0[0:1, :], 1.0)
            ones_r64 = persist.tile([P, P], BF16, tag="ones_r64")
            nc.gpsimd.memset(ones_r64[:], 0.0)
            nc.gpsimd.memset(ones_r64[64:65, :], 1.0)
            srowE_bf = persist.tile([P, QTOK], BF16, tag="srowEbf")
            nc.vector.memset(srowE_bf[:], 0.0)
            srowO_bf = persist.tile([P, QTOK], BF16, tag="srowObf")
            nc.vector.memset(srowO_bf[:], 0.0)
            eps_ap = persist.tile([P, 1], F32, tag="eps")
            nc.gpsimd.memset(eps_ap[:], LN_EPS)

            def ln_stats_mm(ps0, ps1, cast_t, sq_t, kt, n=NKT, ncols=QTOK):
                nc.tensor.matmul(
                    ps0[:, 0:ncols],
                    ones128[:],
                    cast_t[:],
                    start=(kt == 0),
                    stop=(kt == n - 1),
                )
                nc.tensor.matmul(
                    ps1[:, 0:ncols],
                    ones128[:],
                    sq_t[:],
                    start=(kt == 0),
                    stop=(kt == n - 1),
                )

            def ln_meanvar(ps0, ps1, ncols=QTOK):
                """stats psums -> (mean, rstd) broadcast tiles.

                rstd = exp(-0.5*ln(var+eps)): Ln and Exp share an Act table
                (natural_log_exp_and_others) with Relu/Identity/Square, so
                this never forces the 1.3us act-table reload that Sqrt would.
                """
                mean_bc = bc.tile([P, ncols], F32, tag="mean")
                nc.vector.tensor_scalar_mul(mean_bc[:], ps0[:, 0:ncols], 1.0 / D)
                var_bc = bc.tile([P, ncols], F32, tag="var")
                nc.vector.tensor_scalar_mul(var_bc[:], ps1[:, 0:ncols], 1.0 / D)
                m2 = tmp2.tile([P, ncols], F32, tag="lnt")
                nc.vector.tensor_tensor(m2[:], mean_bc[:], mean_bc[:], OP.mult)
                nc.vector.tensor_tensor(var_bc[:], var_bc[:], m2[:], OP.subtract)
                nc.scalar.activation(var_bc[:], var_bc[:], AF.Ln, bias=eps_ap[:])
                nc.scalar.activation(var_bc[:], var_bc[:], AF.Exp, scale=-0.5)
                return mean_bc, var_bc

            def ln_apply(
                src_kt, mean_bc, var_bc, g_ap, b_ap, dst_kt, kt, ncols=QTOK,
                eng=None,
            ):
                eng = eng or nc.vector
                t1 = tmp2.tile([P, ncols], F32, tag="lnt")
                eng.tensor_tensor(t1[:], src_kt, mean_bc[:], OP.subtract)
                eng.tensor_tensor(t1[:], t1[:], var_bc[:], OP.mult)
                if kt % 2 == 0:
                    nc.scalar.activation(
                        dst_kt,
                        t1[:],
                        AF.Identity,
                        scale=g_ap[:, kt : kt + 1],
                        bias=b_ap[:, kt : kt + 1],
                    )
                else:
                    nc.vector.tensor_scalar(
                        dst_kt,
                        t1[:],
                        g_ap[:, kt : kt + 1],
                        b_ap[:, kt : kt + 1],
                        OP.mult,
                        OP.add,
                    )

            # ---- phase 1+2 fused: QKV projections + attention ----
            with tc.tile_pool(name="pproj", bufs=1) as pproj:
                import contextlib as _ctl

                attn_stack = _ctl.ExitStack()
                pattn = attn_stack.enter_context(tc.tile_pool(name="pattn", bufs=3))
                ps_ctx = attn_stack.enter_context(
                    tc.tile_pool(name="ps_ctx", bufs=2, space="PSUM")
                )
                ps_sc = attn_stack.enter_context(
                    tc.tile_pool(name="ps_sc", bufs=2, space="PSUM")
                )
                ps_fill = attn_stack.enter_context(
                    tc.tile_pool(name="ps_fill", bufs=2, space="PSUM")
                )
                pxstack = _ctl.ExitStack()
                px = pxstack.enter_context(tc.tile_pool(name="px", bufs=1))
                wq_t0 = wpool.tile([P, NKT, P], F8, tag="wqkv")
                nc.sync.dma_start(out=wq_t0[:], in_=wq[:, 0])
                xt_s = px.tile([P, NKT, KV], F8, tag="xt")
                for kt in range(NKT):
                    nc.sync.dma_start(out=xt_s[:, kt], in_=xt[:, kt])
                wk_t0 = wpool.tile([P, NKT, P], F8, tag="wqkv")
                nc.sync.dma_start(out=wk_t0[:], in_=wk[:, 0])
                # Q/K in fp8, scores-DR grouped layout: tile ot = (u, c)
                # with u = ot//2 (head group 4u..4u+3), c = ot%2 (dh parity);
                # partition 32*g+ki holds head 4u+g, dh = 2*ki + c.
                qt_s = pproj.tile([P, NKT, QTOK], F8, tag="qt")
                kt_s = pproj.tile([P, NKT, KV], F8, tag="kt")
                # augmented V: per (kv-tile j, head pair hp) 193 cols:
                # [Edims 64 | Eden 1 | Oden 1 | zeros 63 | Odims 64]
                vaug = pproj.tile([P, NKT, NKT, VW], BF16, tag="vaug")
                nc.gpsimd.memset(vaug[:, :, :, 66:129], 0.0)

                wv_s = px.tile([P, NKT, D], F8, tag="wv")
                for c in range(2):
                    nc.sync.dma_start(
                        out=wv_s[:, 4 * c : 4 * (c + 1)], in_=wv[:, 4 * c : 4 * (c + 1)]
                    )
                val_s = pproj.tile([P, NKT, NKT, 2], BF16, tag="val")
                nc.sync.dma_start(out=val_s[:], in_=valid16[:])
                xres_s = pproj.tile([P, NKT, QTOK], F32, tag="xres")
                b1r_s = persist.tile([P, NOT1], F32, tag="b1r")
                b1e_s = persist.tile([P, NOT1], F32, tag="b1e")
                small = {}
                sum1_sb = pproj.tile([P, 512], F32, tag="sum1")
                sumsq1_sb = pproj.tile([P, 512], F32, tag="sumsq1")

                qk_done = set()
                v_done = {0: 0, 1: 0}

                def emit_qproj(ot):
                    if ot == 0:
                        wq_t = wq_t0
                    else:
                        wq_t = wpool.tile([P, NKT, P], F8, tag="wqkv")
                        nc.sync.dma_start(out=wq_t[:], in_=wq[:, ot])
                    ps = ps_fill.tile([P, 512], F32, tag="fill")
                    for kp in range(NKT // 2):
                        nc.tensor.matmul(
                            ps[:],
                            wq_t[:, 2 * kp : 2 * kp + 2],
                            xt_s[:, 2 * kp : 2 * kp + 2, 512:1024],
                            start=(kp == 0),
                            stop=(kp == NKT // 2 - 1),
                            perf_mode=DR,
                        )
                    nc.vector.tensor_copy(qt_s[:, ot], ps[:])

                wk_ts = {0: wk_t0}

                def emit_kproj(ot, tb):
                    if tb == 0 and ot not in wk_ts:
                        wk_t = wpool.tile([P, NKT, P], F8, tag="wqkv")
                        nc.sync.dma_start(out=wk_t[:], in_=wk[:, ot])
                        wk_ts[ot] = wk_t
                    wk_t = wk_ts[ot]
                    ps = ps_fill.tile([P, 512], F32, tag="fill")
                    for kp in range(NKT // 2):
                        nc.tensor.matmul(
                            ps[:],
                            wk_t[:, 2 * kp : 2 * kp + 2],
                            xt_s[:, 2 * kp : 2 * kp + 2, tb * 512 : (tb + 1) * 512],
                            start=(kp == 0),
                            stop=(kp == NKT // 2 - 1),
                            perf_mode=DR,
                        )
                    if tb == 0:
                        nc.scalar.copy(kt_s[:, ot, 0:512], ps[:])
                    else:
                        nc.vector.tensor_copy(kt_s[:, ot, 512:1024], ps[:])
                        qk_done.add(ot)

                def emit_vproj(db, tk):
                    if db == 0:
                        # den columns for all 8 pairs x 2 parities
                        nc.vector.tensor_copy(vaug[:, tk, :, 64:66], val_s[:, tk])
                    ps = ps_fill.tile([P, 4, P], F32, tag="fill")
                    for kp in range(NKT // 2):
                        nc.tensor.matmul(
                            ps[:, :, :],
                            xt_s[:, 2 * kp : 2 * kp + 2, tk * P : (tk + 1) * P],
                            wv_s[:, 2 * kp : 2 * kp + 2, db * 512 : (db + 1) * 512],
                            start=(kp == 0),
                            stop=(kp == NKT // 2 - 1),
                            perf_mode=DR,
                        )
                    hp0 = db * 4
                    nc.vector.tensor_copy(
                        vaug[:, tk, hp0 : hp0 + 4, 0:64], ps[:, :, 0:64]
                    )
                    nc.vector.tensor_copy(
                        vaug[:, tk, hp0 : hp0 + 4, 129:193], ps[:, :, 64:128]
                    )
                    v_done[db] += 1

                stats_pending = []

                def flush_stats():
                    while stats_pending:
                        cast_t, sq_t, hp = stats_pending.pop(0)
                        pss = ps_sc.tile([P, 1024], F32, tag="sc")
                        nc.tensor.matmul(
                            pss[:, 0:512], ones128[:], cast_t[:], start=True, stop=True
                        )
                        nc.tensor.matmul(
                            pss[:, 512:1024], ones128[:], sq_t[:], start=True, stop=True
                        )
                        if hp == 0:
                            nc.vector.tensor_copy(sum1_sb[:], pss[:, 0:512])
                            nc.vector.tensor_copy(sumsq1_sb[:], pss[:, 512:1024])
                        else:
                            nc.vector.tensor_tensor(
                                sum1_sb[:], sum1_sb[:], pss[:, 0:512], OP.add
                            )
                            nc.vector.tensor_tensor(
                                sumsq1_sb[:], sumsq1_sb[:], pss[:, 512:1024], OP.add
                            )

                def emit_post(p):
                    h, cps, ctxn = p
                    hp, par = h // 2, h % 2
                    po = 64 * par
                    flush_stats()
                    # broadcast the bf16 reciprocal row across the 64 ctx
                    # partitions via a ones-matmul (bcp shares the fill pool)
                    bcp = ps_fill.tile([P, 512], F32, tag="fill")
                    if par == 0:
                        nc.tensor.matmul(
                            bcp[0:64],
                            ones_r64[:, 0:64],
                            srowE_bf[:],
                            start=True,
                            stop=True,
                        )
                    else:
                        nc.tensor.matmul(
                            bcp[64:128],
                            ones_r0[:, 0:64],
                            srowO_bf[:],
                            start=True,
                            stop=True,
                        )
                    bc_sb = tmp2.tile([P, 512], BF16, tag="bcsb")
                    nc.vector.tensor_copy(bc_sb[po : po + 64], bcp[po : po + 64])
                    nc.vector.tensor_tensor(
                        ctxn[po : po + 64],
                        cps[po : po + 64],
                        bc_sb[po : po + 64],
                        OP.mult,
                    )
                    if par == 1:
                        # pair finished: residual add + LN1 stats (Pool
                        # takes the add + cast + square; PE the stats,
                        # deferred one slot so PE never waits on Pool).
                        # Last pair runs on DVE: Pool's 0.42 efficiency
                        # would sit on the attention->LN1 critical path.
                        eng = nc.vector if hp == H // 2 - 1 else nc.gpsimd
                        eng.tensor_add(
                            xres_s[:, hp], xres_s[:, hp], ctxn[:]
                        )
                        cast_t = tmp.tile([P, 512], BF16, tag="lncast")
                        sq_t = tmp.tile([P, 512], BF16, tag="lnsq")
                        eng.tensor_copy(cast_t[:], xres_s[:, hp])
                        eng.tensor_mul(
                            sq_t[:], xres_s[:, hp], xres_s[:, hp]
                        )
                        stats_pending.append((cast_t, sq_t, hp))

                ctxn = None
                # packed expt: per-j live query range [off_j, 512) stored
                # contiguously; POFF[j] is the packed start, NCOL[j] the width
                NCOL = [512 - max(0, j - 4) * P for j in range(NKT)]
                POFF = [0] * NKT
                for j in range(1, NKT):
                    POFF[j] = POFF[j - 1] + NCOL[j - 1]

                def emit_ctx(pr):
                    nonlocal ctxn
                    h, expt = pr
                    hp, par = h // 2, h % 2
                    cps = ps_ctx.tile([P, 512], F32, tag="ctx")
                    lsl = (0, 65) if par == 0 else (65, VW)
                    m = lsl[1] - lsl[0]
                    for j in range(NKT):
                        off = max(0, j - 4) * P
                        nc.tensor.matmul(
                            cps[0:m, off:512],
                            vaug[:, j, hp, lsl[0] : lsl[1]],
                            expt[:, POFF[j] : POFF[j] + NCOL[j]],
                            start=(j == 0),
                            stop=(j == NKT - 1),
                        )
                    with nc.allow_low_precision(
                        reason="softmax denominator reciprocal to bf16"
                    ):
                        if par == 0:
                            nc.vector.reciprocal(srowE_bf[64:65], cps[64:65])
                        else:
                            nc.vector.reciprocal(srowO_bf[0:1], cps[0:1])
                    if DEBUG_TAPS and h == DBG_HEAD:
                        dbg_cps = nc.dram_tensor(
                            "dbg_cps", [P, 512], F32, kind="ExternalOutput"
                        ).ap()
                        dbg_sb = persist.tile([P, 512], F32, tag="dbgsb")
                        nc.vector.memset(dbg_sb[:], 0.0)
                        _r0, _r1 = (0, 65) if par == 0 else (64, 128)
                        nc.vector.tensor_copy(dbg_sb[_r0:_r1], cps[_r0:_r1])
                        if par == 1:
                            nc.vector.tensor_copy(dbg_sb[0:1], cps[0:1])
                        nc.sync.dma_start(out=dbg_cps[:], in_=dbg_sb[:])
                        dbg_expt = nc.dram_tensor(
                            "dbg_expt", [P, 3328], BF16, kind="ExternalOutput"
                        ).ap()
                        nc.sync.dma_start(out=dbg_expt[:], in_=expt[:])
                        dbg_vaug = nc.dram_tensor(
                            "dbg_vaug", [P, NKT, VW], BF16, kind="ExternalOutput"
                        ).ap()
                        nc.sync.dma_start(out=dbg_vaug[:], in_=vaug[:, :, hp])
                    if par == 0:
                        ctxn = tmp2.tile([P, 512], F32, tag="ctxn")
                    return (h, cps, ctxn)

                def emit_scores(h):
                    # DoubleRow over dh: contraction (ki 32, parity 2); head
                    # h lives at partition group 32*(h%4) of ot pair
                    # (2*(h//4), 2*(h//4)+1). j-tiles are computed two per
                    # 2-bank psum so each Exp covers a pair in one shot.
                    u, sub = h // 4, h % 4
                    b0 = 32 * sub
                    expt = pattn.tile([P, 3328], BF16, tag="expt")
                    for pj in range(4):
                        j0 = 2 * pj
                        w0, w1 = NCOL[j0], NCOL[j0 + 1]
                        # two independent accumulation groups must not share
                        # a PSUM bank: place the second j at a 512 offset
                        po1 = max(w0, 512)
                        ps = ps_sc.tile([P, 1024], F32, tag="sc")
                        for j, w, po in ((j0, w0, 0), (j0 + 1, w1, po1)):
                            off = 512 - w
                            nc.tensor.matmul(
                                ps[:, po : po + w],
                                kt_s[b0 : b0 + 32, 2 * u : 2 * u + 2,
                                     j * P : (j + 1) * P],
                                qt_s[b0 : b0 + 32, 2 * u : 2 * u + 2, off:512],
                                start=True,
                                stop=True,
                                perf_mode=DR,
                                tile_position=(b0, 0),
                            )
                        if po1 == w0:
                            nc.scalar.activation(
                                expt[:, POFF[j0] : POFF[j0] + w0 + w1],
                                ps[:, 0 : w0 + w1],
                                AF.Exp,
                                scale=0.125 / (WSQK * WSQK),
                            )
                        else:
                            nc.scalar.activation(
                                expt[:, POFF[j0] : POFF[j0] + w0],
                                ps[:, 0:w0],
                                AF.Exp,
                                scale=0.125 / (WSQK * WSQK),
                            )
                            nc.scalar.activation(
                                expt[:, POFF[j0 + 1] : POFF[j0 + 1] + w1],
                                ps[:, po1 : po1 + w1],
                                AF.Exp,
                                scale=0.125 / (WSQK * WSQK),
                            )
                    for j in range(4, NKT):
                        # zero the masked upper triangle of the diagonal
                        # query block post-exp (Pool, off the hot engines)
                        nc.gpsimd.affine_select(
                            out=expt[:, POFF[j] : POFF[j] + P],
                            in_=expt[:, POFF[j] : POFF[j] + P],
                            compare_op=OP.is_ge,
                            fill=0.0,
                            base=0,
                            pattern=[[1, P]],
                            channel_multiplier=-1,
                        )
                    return (h, expt)

                # filler units: Q/K projections + V-proj tiles, ordered by
                # consumption deadline, drained during the head loop
                fillers = []
                for ot in (2, 3):
                    fillers += [
                        lambda o=ot: emit_qproj(o),
                        lambda o=ot: emit_kproj(o, 0),
                        lambda o=ot: emit_kproj(o, 1),
                    ]
                fillers += [lambda t=tk: emit_vproj(0, t) for tk in range(NKT)]
                for ot in (4, 5):
                    fillers += [
                        lambda o=ot: emit_qproj(o),
                        lambda o=ot: emit_kproj(o, 0),
                        lambda o=ot: emit_kproj(o, 1),
                    ]
                fillers += [lambda t=tk: emit_vproj(1, t) for tk in range(NKT)]
                for ot in (6, 7):
                    fillers += [
                        lambda o=ot: emit_qproj(o),
                        lambda o=ot: emit_kproj(o, 0),
                        lambda o=ot: emit_kproj(o, 1),
                    ]
                fillers.reverse()  # consume via pop()

                def drain(n):
                    for _ in range(n):
                        if fillers:
                            fillers.pop()()

                def need_qk(u):
                    # scores for head group u needs both parity tiles 2u, 2u+1
                    while not ({2 * u, 2 * u + 1} <= qk_done):
                        assert fillers, f"filler queue dry before qk pair {u}"
                        fillers.pop()()

                def need_v(db):
                    while v_done[db] < NKT:
                        assert fillers, f"filler queue dry before v {db}"
                        fillers.pop()()

                # warmup: head-group-0 Q/K, then 3 heads of scores while the
                # filler queue builds V/QK state; first ctx after V db0 done
                emit_qproj(0)
                emit_kproj(0, 0)
                emit_kproj(0, 1)
                emit_qproj(1)
                emit_kproj(1, 0)
                emit_kproj(1, 1)
                from collections import deque

                prevs = deque()
                prevs.append(emit_scores(0))
                drain(3)
                prevs.append(emit_scores(1))
                nc.sync.dma_start(out=xres_s[:], in_=xres[:])
                drain(3)
                prevs.append(emit_scores(2))
                drain(4)
                need_v(0)
                nc.sync.dma_start(out=b1r_s[:], in_=b1r[:])
                nc.sync.dma_start(out=b1e_s[:], in_=b1e[:])
                for nm, _src in (
                    ("b2t", b2t),
                    ("g1t", g1t),
                    ("be1t", be1t),
                    ("g2t", g2t),
                    ("be2t", be2t),
                ):
                    t = persist.tile([P, NKT], F32, tag=nm)
                    nc.sync.dma_start(out=t[:], in_=_src[:])
                    small[nm] = t
                # prefetch the first fc1/fc2 weight chunks during attention
                # so the FFN phases never wait on the serial SP DMA queue
                for _c in range(3):
                    load_w1(_c)
                load_w2(0)
                load_w2(1)
                pending = emit_ctx(prevs.popleft())
                for h in range(3, H):
                    need_qk(h // 4)
                    prevs.append(emit_scores(h))
                    drain(1)
                    emit_post(pending)
                    nh = prevs[0][0]
                    need_v(nh // 8)
                    pending = emit_ctx(prevs.popleft())
                drain(len(fillers))
                while prevs:
                    emit_post(pending)
                    need_v(1)
                    pending = emit_ctx(prevs.popleft())
                emit_post(pending)
                flush_stats()

                if DEBUG_TAPS:
                    dbg_xres = nc.dram_tensor(
                        "dbg_xres", [P, NKT, QTOK], F32, kind="ExternalOutput"
                    ).ap()
                    nc.sync.dma_start(out=dbg_xres[:], in_=xres_s[:])
                    dbg_xt = nc.dram_tensor(
                        "dbg_xt", [P, NKT, KV], F8, kind="ExternalOutput"
                    ).ap()
                    nc.sync.dma_start(out=dbg_xt[:], in_=xt_s[:])
                    dbg_kt = nc.dram_tensor(
                        "dbg_kt", [P, NKT, KV], F8, kind="ExternalOutput"
                    ).ap()
                    nc.sync.dma_start(out=dbg_kt[:], in_=kt_s[:])
                    dbg_stats = nc.dram_tensor(
                        "dbg_stats", [P, 2, 512], F32, kind="ExternalOutput"
                    ).ap()
                    nc.sync.dma_start(out=dbg_stats[:, 0], in_=sum1_sb[:])
                    nc.sync.dma_start(out=dbg_stats[:, 1], in_=sumsq1_sb[:])
                    dbg_srow = nc.dram_tensor(
                        "dbg_srow", [P, 2, QTOK], BF16, kind="ExternalOutput"
                    ).ap()
                    nc.sync.dma_start(out=dbg_srow[:, 0], in_=srowE_bf[:])
                    nc.sync.dma_start(out=dbg_srow[:, 1], in_=srowO_bf[:])

                # ---- phase 3: LN1 (stats already accumulated) ----
                # ln1_bf (bf16) is the fc2 residual; x8/x8e are the fp8
                # hi/lo pair feeding the compensated fc1 DoubleRow passes.
                ln1_bf = lnp.tile([P, NKT, QTOK], BF16, tag="ln1")
                x8 = lnp.tile([P, NKT, QTOK], F8, tag="x8")
                x8e = lnp.tile([P, NKT, QTOK], F8, tag="x8e")
                mean1, rstd1 = ln_meanvar(sum1_sb, sumsq1_sb)
                for kt in range(NKT):
                    ln_apply(
                        xres_s[:, kt], mean1, rstd1,
                        small["g1t"], small["be1t"], ln1_bf[:, kt], kt,
                    )
                    nc.gpsimd.tensor_copy(x8[:, kt], ln1_bf[:, kt])
                    nc.vector.tensor_tensor(
                        x8e[:, kt], ln1_bf[:, kt], x8[:, kt], OP.subtract
                    )
                pxstack.close()
                attn_stack.close()

            # ---- phase 4: fc1 + selu (w1 in JIT 4-ot chunks, depth 2) ----
            pffn_stack = contextlib.ExitStack()
            pffn = pffn_stack.enter_context(tc.tile_pool(name="pffn", bufs=1))
            ps_mm = pffn_stack.enter_context(
                tc.tile_pool(name="ps_mm", bufs=3, space="PSUM")
            )
            ps_x = pffn_stack.enter_context(
                tc.tile_pool(name="ps_x", bufs=1, space="PSUM")
            )
            h8 = pffn.tile([P, NOT1, QTOK], F8, tag="h8")
            h8e = pffn.tile([P, NOT1, QTOK], F8, tag="h8e")
            for ot in range(NOT1):
                w1h, w1l = w1bufs[(ot // 2) % NW1B]
                if ot % 2 == 0:
                    load_w1(ot // 2 + 3)  # chunks 0..2 preloaded in attention
                if ot % 4 == 3:
                    ps = ps_x.tile([P, 512], F32, tag="x")
                else:
                    ps = ps_mm.tile([P, 512], F32, tag="mm")
                passes = [(w1h, x8), (w1l, x8), (w1h, x8e)]
                for pi, (wt, xq) in enumerate(passes):
                    for kp in range(NKT // 2):
                        nc.tensor.matmul(
                            ps[:],
                            wt[:, ot % 2, 2 * kp : 2 * kp + 2],
                            xq[:, 2 * kp : 2 * kp + 2],
                            start=(pi == 0 and kp == 0),
                            stop=(pi == 2 and kp == NKT // 2 - 1),
                            perf_mode=DR,
                        )
                p_t = tmp.tile([P, QTOK], F32, tag="selup")
                nc.scalar.activation(
                    p_t[:],
                    ps[:],
                    AF.Relu,
                    scale=SELU_S / WS,
                    bias=b1r_s[:, ot : ot + 1],
                )
                e_t = tmp.tile([P, QTOK], F32, tag="selue")
                nc.scalar.activation(
                    e_t[:], ps[:], AF.Exp, scale=1.0 / WS, bias=b1e_s[:, ot : ot + 1]
                )
                # selu(z) = min(sa*e^z - sa, s*relu(z)): for z>0 the exp arm
                # exceeds s*z, for z<=0 relu is 0 and the exp arm is negative
                nc.vector.scalar_tensor_tensor(
                    p_t[:], e_t[:], SELU_SA, p_t[:], OP.subtract, OP.min
                )
                nc.gpsimd.tensor_copy(h8[:, ot], p_t[:])
                nc.vector.tensor_tensor(h8e[:, ot], p_t[:], h8[:, ot], OP.subtract)

            # ---- phase 5: fc2 + residual + LN2 + store (full 512 width) ----
            ps_stat2 = pffn_stack.enter_context(
                tc.tile_pool(name="ps_stat2", bufs=2, space="PSUM")
            )
            res2 = pffn.tile([P, NKT, QTOK], F32, tag="res2")
            ps0_2 = ps_stat2.tile([P, 512], F32, tag="stat2")
            ps1_2 = ps_stat2.tile([P, 512], F32, tag="stat2")
            for ot in range(NKT):
                w2h, w2l = w2bufs[ot % NW2B]
                load_w2(ot + 2)  # ots 0,1 preloaded in attention
                if ot % 4 == 3:
                    ps = ps_x.tile([P, 512], F32, tag="x")
                else:
                    ps = ps_mm.tile([P, 512], F32, tag="mm")
                passes = [(w2h, h8), (w2l, h8), (w2h, h8e)]
                for pi, (wt, hq) in enumerate(passes):
                    for kp in range(NOT1 // 2):
                        nc.tensor.matmul(
                            ps[:],
                            wt[:, 2 * kp : 2 * kp + 2],
                            hq[:, 2 * kp : 2 * kp + 2],
                            start=(pi == 0 and kp == 0),
                            stop=(pi == 2 and kp == NOT1 // 2 - 1),
                            perf_mode=DR,
                        )
                t1 = tmp2.tile([P, QTOK], F32, tag="r2t")
                nc.scalar.activation(
                    t1[:],
                    ps[:],
                    AF.Identity,
                    scale=1.0 / WS,
                    bias=small["b2t"][:, ot : ot + 1],
                )
                nc.vector.tensor_tensor(
                    res2[:, ot], t1[:], ln1_bf[:, ot], OP.add
                )
                cast_t = tmp.tile([P, QTOK], BF16, tag="lncast2")
                sq_t = tmp.tile([P, QTOK], BF16, tag="lnsq2")
                nc.vector.tensor_copy(cast_t[:], res2[:, ot])
                nc.scalar.activation(sq_t[:], res2[:, ot], AF.Square)
                ln_stats_mm(ps0_2, ps1_2, cast_t, sq_t, ot)
            mean2, rstd2 = ln_meanvar(ps0_2, ps1_2)
            for kt in range(NKT):
                # spread the tail normalize across DVE and Pool so the
                # final 8-tile chain isn't serialized on one engine
                eng = nc.gpsimd if kt in (2, 5) else nc.vector
                ln_apply(
                    res2[:, kt], mean2, rstd2,
                    small["g2t"], small["be2t"], res2[:, kt], kt,
                    eng=eng,
                )
                nc.sync.dma_start(out=out[:, kt], in_=res2[:, kt])
            pffn_stack.close()

    _legalize_waits(nc)
    return nc


_NC_CACHE = None
TRACE = False
LAST_EXEC_NS = None


def _get_nc():
    global _NC_CACHE
    if _NC_CACHE is None:
        _NC_CACHE = _build_nc()
    return _NC_CACHE


def _tile_w(a):
    """[Din, O] -> [P, O//P(ot), Din//P(kt), P] with ot-contiguous DMA slices."""
    Din, O = a.shape
    return np.ascontiguousarray(
        a.reshape(Din // P, P, O // P, P).transpose(1, 2, 0, 3)
    )


def _pp(v, n):
    """[n*P] -> [P, n] per-partition layout."""
    return np.ascontiguousarray(v.reshape(n, P).T)


def kernel(X, wq, wk, wv, ln1_g, ln1_b, w1, b1, w2, b2, ln2_g, ln2_b):
    from concourse.bass_utils import run_bass_kernel_spmd

    X = np.asarray(X, np.float32)
    bf = ml_dtypes.bfloat16
    f8 = ml_dtypes.float8_e4m3  # IEEE flavor — matches bass float8e4

    def hilo(wt):
        hi = wt.astype(f8)
        lo = (wt - hi.astype(np.float32)).astype(f8)
        return hi, lo

    # scores-DR out-dim permutation: slot (ot, i) holds projection row
    # head*64 + dh with head = 4*(ot//2) + i//32, dh = 2*(i%32) + ot%2,
    # so head h sits at partition group 32*(h%4) of tiles (2u, 2u+1)
    # with the dh parity split across the tile pair (DoubleRow Ko dim).
    qperm = np.empty(D, np.int64)
    for _ot in range(NKT):
        for _i in range(P):
            _h = 4 * (_ot // 2) + _i // 32
            _dh = 2 * (_i % 32) + (_ot % 2)
            qperm[_ot * P + _i] = _h * 64 + _dh
    wqT = _tile_w((WSQK * np.asarray(wq, np.float32).T)[:, qperm]).astype(f8)
    wkT = _tile_w((WSQK * np.asarray(wk, np.float32).T)[:, qperm]).astype(f8)
    wvT = np.ascontiguousarray(
        WS * np.asarray(wv, np.float32).T.reshape(NKT, P, D).transpose(1, 0, 2)
    ).astype(f8)
    w1hi, w1lo = hilo(_tile_w(WS * np.asarray(w1, np.float32).T))
    w2hi, w2lo = hilo(_tile_w(WS * np.asarray(w2, np.float32).T))
    b1 = np.asarray(b1, np.float32)
    shared = dict(
        wq=wqT,
        wk=wkT,
        wv=wvT,
        w1=w1hi,
        w1e=w1lo,
        w2=w2hi,
        w2e=w2lo,
        b1r=_pp(SELU_S * b1, NOT1),
        b1e=_pp(b1 + LN_SA, NOT1),
        b2t=_pp(np.asarray(b2, np.float32), NKT),
        g1t=_pp(np.asarray(ln1_g, np.float32), NKT),
        be1t=_pp(np.asarray(ln1_b, np.float32), NKT),
        g2t=_pp(np.asarray(ln2_g, np.float32), NKT),
        be2t=_pp(np.asarray(ln2_b, np.float32), NKT),
    )

    in_maps = []
    for c in range(8):
        b, hf = c // 2, c % 2
        if hf == 1:
            xkv = X[b].T  # [D, L]
            valid = np.full(KV, WS, np.float32)
            xq = X[b, 512:]
        else:
            xkv = np.concatenate(
                [np.zeros((D, 512), np.float32), X[b, :512].T], axis=1
            )
            valid = np.concatenate(
                [np.zeros(512, np.float32), np.full(512, WS, np.float32)]
            )
            xq = X[b, :512]
        xt = (
            np.ascontiguousarray(xkv.reshape(NKT, P, KV).transpose(1, 0, 2))
        ).astype(f8)
        xres = np.ascontiguousarray(xq.T.reshape(NKT, P, QTOK).transpose(1, 0, 2))
        vt = valid.reshape(NKT, P).T  # [P, NKT]
        val16 = (
            np.repeat(vt[:, :, None], H, axis=2).reshape(P, NKT, NKT, 2).astype(bf)
        )
        m = dict(shared)
        m.update(xt=xt, xres=xres, valid16=np.ascontiguousarray(val16))
        in_maps.append(m)

    nc = _get_nc()
    global LAST_EXEC_NS
    if TRACE:
        res = run_bass_kernel_spmd(nc, in_maps, list(range(8)), trace=True)
        LAST_EXEC_NS = res.exec_time_ns
    else:
        res = run_bass_kernel_spmd(nc, in_maps, list(range(8)))

    out = np.empty((B, L, D), np.float32)
    for c in range(8):
        b, hf = c // 2, c % 2
        o = res.results[c]["out"]  # [P, NKT, QTOK]
        o = o.transpose(1, 0, 2).reshape(D, QTOK).T  # [QTOK, D]
        out[b, hf * 512 : hf * 512 + 512] = o
    return out



# revision 73
# speedup vs baseline: 1.0994x; 1.0004x over previous
"""Decoder-layer Trainium2 kernel: 8-core SPMD, single launch, no collectives.

Sharding: core c -> (batch b = c // 2, sequence-half hf = c % 2). Each core
computes the full decoder layer for 512 query tokens of one sequence.
All cores run ONE identical program over a canonical virtual sequence of
1024 kv tokens with queries at virtual positions 512..1023; first-half cores
get their 512 real tokens placed at virtual 512..1023 with zero-padded kv
prefix and a `valid` vector that zeroes the pad contribution to the softmax
denominator.

v2 changes vs baseline:
- softmax denominators ride along in the ctx matmul via an augmented V
  (per head-pair V layout [Edims|Eden|Oden|zeros63|Odims], 193 wide): even
  heads matmul M=65 -> dims at psum rows 0..63 + den at row 64; odd heads
  M=128 with a zero block -> den at row 0 + dims at rows 64..127. Kills the
  65536 rows of separate [1,512] denominator matmuls.
- ctx matmuls are causally restricted to the live query range per kv tile
  (like scores), saving another 12288 rows.
- LN1 stats matmuls run inline as each head pair finishes its xres tile.
- fc2 + LN2 run in two token-half passes so the final normalize/store of
  half 0 overlaps the fc2 matmuls of half 1.
"""

import sys

sys.path.insert(0, "/opt/trn_rl_repo")

import math

import numpy as np
import ml_dtypes

import concourse.bass as bass
import concourse.mybir as mybir
from concourse.tile import TileContext, TilePool
from concourse.vector_clock import ScopedClock

BF16 = mybir.dt.bfloat16
F8 = mybir.dt.float8e4
F32 = mybir.dt.float32
AF = mybir.ActivationFunctionType
OP = mybir.AluOpType
DR = mybir.MatmulPerfMode.DoubleRow
WS = 64.0  # fp8 weight pre-scale (wv/w1/w2)
# Q/K projections use a smaller pre-scale: bass float8e4 is IEEE e4m3
# (max finite 240, saturates to inf) and |K|*64 reaches ~290 on some
# batches; *32 keeps the fp8 Q/K copies comfortably finite.
WSQK = 32.0

B, L, D = 4, 1024, 1024
H, DH = 16, 64
DFF = 4 * D
P = 128
QTOK = 512  # query tokens per core
KV = 1024  # canonical kv length (virtual)
NKT = D // P  # 8 d-tiles
NOT1 = DFF // P  # 32 fc1 out tiles
MASK_NEG = -1.0e9
VW = 193  # augmented V width per head pair: [Ed 64|Eden|Oden|z 63|Od 64]

SELU_S = 1.0507009873554804934193349852946
SELU_A = 1.6732632423543772848170429916717
SELU_SA = SELU_S * SELU_A
LN_SA = math.log(SELU_SA)
LN_EPS = 1e-5


class PatchedTileContext(TileContext):
    """TileContext whose exit drain respects this walrus build's limit of
    ONE semaphore wait per instruction: the global-clock waits are spread
    across standalone NOPs and the butterfly barrier (whose sem-eq waits
    walrus rejects) is replaced by the NRT-expanded pseudo barrier."""

    def _drain_and_barrier(self, tick_clock, wait_clock):
        nc = self.nc
        carrier = nc.sync.nop()
        wait_clock.add_sem_waits(
            carrier.ins, ScopedClock({None: tick_clock.global_clock})
        )
        waits = list(carrier.ins.sync_info.on_wait)
        ups = list(carrier.ins.sync_info.on_update)
        if len(waits) > 1:
            carrier.ins.sync_info = mybir.SyncInfo(on_wait=[waits[0]], on_update=ups)
            for w in waits[1:]:
                extra = nc.sync.nop()
                extra.ins.sync_info = mybir.SyncInfo(on_wait=[w], on_update=[])
        for eng in nc.engines.values():
            eng.drain()
        nc._nrt_pseudo_barrier()
        popped = nc._tile_sem_poison_stack.pop()
        assert popped is self._sem_poison
        nc.clear_and_free_semaphores(list(self.sems.allocated().values()))
        nc._nrt_pseudo_barrier()


def _legalize_waits(nc):
    """This walrus build accepts at most ONE semaphore wait per instruction.
    Tile's sem-assignment can attach several; hoist the extras onto same-engine
    NOPs inserted immediately before the instruction (waits are a conjunction,
    so a sequence of single-wait stalls is equivalent)."""
    n = 0
    for fn in nc.m.functions:
        for blk in fn.blocks:
            out = []
            changed = False
            for inst in blk.instructions:
                si = getattr(inst, "sync_info", None)
                if si is not None and len(si.on_wait) > 1:
                    waits = list(si.on_wait)
                    for w in waits[:-1]:
                        nop = mybir.InstNoOp(name=f"waitnop_{n}", ins=[], outs=[])
                        n += 1
                        nop.engine = inst.engine
                        nop.sync_info = mybir.SyncInfo(on_wait=[w], on_update=[])
                        out.append(nop)
                    inst.sync_info = mybir.SyncInfo(
                        on_wait=[waits[-1]], on_update=list(si.on_update)
                    )
                    changed = True
                out.append(inst)
            if changed:
                blk.instructions = out
    return n


DEBUG_TAPS = False
DBG_HEAD = 0


def _build_nc():
    nc = bass.Bass("TRN2", target_bir_lowering=False, debug=False, num_devices=8)

    def din(name, shape, dt):
        return nc.dram_tensor(name, shape, dt, kind="ExternalInput").ap()

    xt = din("xt", [P, NKT, KV], F8)  # X[b].T tiled, virtual-padded
    xres = din("xres", [P, NKT, QTOK], F32)  # q tokens transposed, fp32
    valid16 = din("valid16", [P, NKT, NKT, 2], BF16)  # WS flag, [8hp x 2]
    wq = din("wq", [P, NKT, NKT, P], F8)  # [dpart, ot, kt, o], x WS
    wk = din("wk", [P, NKT, NKT, P], F8)
    wv = din("wv", [P, NKT, D], F8)  # rhs layout [dpart, kt, o], x WS
    w1 = din("w1", [P, NOT1, NKT, P], F8)  # fp8(WS*w1^T)
    w1e = din("w1e", [P, NOT1, NKT, P], F8)  # fp8 residual of the above
    w2 = din("w2", [P, NKT, NOT1, P], F8)
    w2e = din("w2e", [P, NKT, NOT1, P], F8)
    b1r = din("b1r", [P, NOT1], F32)  # SELU_S * b1
    b1e = din("b1e", [P, NOT1], F32)  # b1 + ln(SELU_S*SELU_A)
    b2t = din("b2t", [P, NKT], F32)
    g1t = din("g1t", [P, NKT], F32)
    be1t = din("be1t", [P, NKT], F32)
    g2t = din("g2t", [P, NKT], F32)
    be2t = din("be2t", [P, NKT], F32)
    out = nc.dram_tensor("out", [P, NKT, QTOK], F32, kind="ExternalOutput").ap()

    with PatchedTileContext(nc) as tc:
        import contextlib

        with contextlib.ExitStack() as ctx:
            persist = ctx.enter_context(tc.tile_pool(name="persist", bufs=1))
            bc = ctx.enter_context(tc.tile_pool(name="bc", bufs=1))
            wpool = ctx.enter_context(tc.tile_pool(name="wpool", bufs=4))
            tmp = ctx.enter_context(tc.tile_pool(name="tmp", bufs=2))
            tmp2 = ctx.enter_context(tc.tile_pool(name="tmp2", bufs=2))
            lnp = ctx.enter_context(tc.tile_pool(name="lnp", bufs=1))
            w1pool = ctx.enter_context(tc.tile_pool(name="w1pool", bufs=1))

            # ---- constants ----
            NW1B, NW2B = 5, 4
            w1bufs = [
                (
                    w1pool.tile(
                        [P, 2, NKT, P], F8, tag=f"w1{i}h", name=f"w1{i}h"
                    ),
                    w1pool.tile(
                        [P, 2, NKT, P], F8, tag=f"w1{i}e", name=f"w1{i}e"
                    ),
                )
                for i in range(NW1B)
            ]
            w2bufs = [
                (
                    w1pool.tile(
                        [P, NOT1, P], F8, tag=f"w2{i}h", name=f"w2{i}h"
                    ),
                    w1pool.tile(
                        [P, NOT1, P], F8, tag=f"w2{i}e", name=f"w2{i}e"
                    ),
                )
                for i in range(NW2B)
            ]

            def load_w1(chunk):
                if chunk < NOT1 // 2:
                    hb, lb = w1bufs[chunk % NW1B]
                    nc.sync.dma_start(out=hb[:], in_=w1[:, 2 * chunk : 2 * chunk + 2])
                    nc.sync.dma_start(out=lb[:], in_=w1e[:, 2 * chunk : 2 * chunk + 2])

            def load_w2(ot):
                if ot < NKT:
                    hb, lb = w2bufs[ot % NW2B]
                    nc.sync.dma_start(out=hb[:], in_=w2[:, ot])
                    nc.sync.dma_start(out=lb[:], in_=w2e[:, ot])
            ones128 = persist.tile([P, P], BF16, tag="ones128")
            nc.gpsimd.memset(ones128[:], 1.0)
            ones_r0 = persist.tile([P, P], BF16, tag="ones_r0")
            nc.gpsimd.memset(ones_r0[:], 0.0)
            nc.gpsimd.memset(ones_r0[0:1, :], 1.0)
            ones_r64 = persist.tile([P, P], BF16, tag="ones_r64")
            nc.gpsimd.memset(ones_r64[:], 0.0)
            nc.gpsimd.memset(ones_r64[64:65, :], 1.0)
            srowE_bf = persist.tile([P, QTOK], BF16, tag="srowEbf")
            nc.vector.memset(srowE_bf[:], 0.0)
            srowO_bf = persist.tile([P, QTOK], BF16, tag="srowObf")
            nc.vector.memset(srowO_bf[:], 0.0)
            eps_ap = persist.tile([P, 1], F32, tag="eps")
            nc.gpsimd.memset(eps_ap[:], LN_EPS)

            def ln_stats_mm(ps0, ps1, cast_t, sq_t, kt, n=NKT, ncols=QTOK):
                nc.tensor.matmul(
                    ps0[:, 0:ncols],
                    ones128[:],
                    cast_t[:],
                    start=(kt == 0),
                    stop=(kt == n - 1),
                )
                nc.tensor.matmul(
                    ps1[:, 0:ncols],
                    ones128[:],
                    sq_t[:],
                    start=(kt == 0),
                    stop=(kt == n - 1),
                )

            def ln_meanvar(ps0, ps1, ncols=QTOK):
                """stats psums -> (mean, rstd) broadcast tiles.

                rstd = exp(-0.5*ln(var+eps)): Ln and Exp share an Act table
                (natural_log_exp_and_others) with Relu/Identity/Square, so
                this never forces the 1.3us act-table reload that Sqrt would.
                """
                if ps1 is None:
                    # ps0 is a [P, 2*ncols] psum holding [sum | sumsq]:
                    # scale both with one DVE op into an adjacent pair
                    mv = bc.tile([P, 2 * ncols], F32, tag="meanvar")
                    nc.vector.tensor_scalar_mul(mv[:], ps0[:, 0 : 2 * ncols], 1.0 / D)
                    mean_bc, var_bc = mv[:, 0:ncols], mv[:, ncols : 2 * ncols]
                else:
                    mean_t = bc.tile([P, ncols], F32, tag="mean", name="mean_t")
                    mean_bc = mean_t[:]
                    nc.vector.tensor_scalar_mul(mean_bc, ps0[:, 0:ncols], 1.0 / D)
                    var_t = bc.tile([P, ncols], F32, tag="var", name="var_t")
                    var_bc = var_t[:]
                    nc.vector.tensor_scalar_mul(var_bc, ps1[:, 0:ncols], 1.0 / D)
                m2 = tmp2.tile([P, ncols], F32, tag="lnt")
                nc.vector.tensor_tensor(m2[:], mean_bc, mean_bc, OP.mult)
                nc.vector.tensor_tensor(var_bc, var_bc, m2[:], OP.subtract)
                nc.scalar.activation(var_bc, var_bc, AF.Ln, bias=eps_ap[:])
                nc.scalar.activation(var_bc, var_bc, AF.Exp, scale=-0.5)
                return mean_bc, var_bc

            def ln_apply(
                src_kt, mean_bc, var_bc, g_ap, b_ap, dst_kt, kt, ncols=QTOK,
                eng=None, fin_act=None,
            ):
                eng = eng or nc.vector
                t1 = tmp2.tile([P, ncols], F32, tag="lnt")
                eng.tensor_tensor(t1[:], src_kt, mean_bc[:], OP.subtract)
                eng.tensor_tensor(t1[:], t1[:], var_bc[:], OP.mult)
                if fin_act if fin_act is not None else (kt % 2 == 0):
                    nc.scalar.activation(
                        dst_kt,
                        t1[:],
                        AF.Identity,
                        scale=g_ap[:, kt : kt + 1],
                        bias=b_ap[:, kt : kt + 1],
                    )
                else:
                    nc.vector.tensor_scalar(
                        dst_kt,
                        t1[:],
                        g_ap[:, kt : kt + 1],
                        b_ap[:, kt : kt + 1],
                        OP.mult,
                        OP.add,
                    )

            # ---- phase 1+2 fused: QKV projections + attention ----
            with tc.tile_pool(name="pproj", bufs=1) as pproj:
                import contextlib as _ctl

                attn_stack = _ctl.ExitStack()
                pattn = attn_stack.enter_context(tc.tile_pool(name="pattn", bufs=3))
                ps_ctx = attn_stack.enter_context(
                    tc.tile_pool(name="ps_ctx", bufs=2, space="PSUM")
                )
                ps_sc = attn_stack.enter_context(
                    tc.tile_pool(name="ps_sc", bufs=2, space="PSUM")
                )
                ps_fill = attn_stack.enter_context(
                    tc.tile_pool(name="ps_fill", bufs=2, space="PSUM")
                )
                pxstack = _ctl.ExitStack()
                px = pxstack.enter_context(tc.tile_pool(name="px", bufs=1))
                wq_t0 = wpool.tile([P, NKT, P], F8, tag="wqkv")
                nc.sync.dma_start(out=wq_t0[:], in_=wq[:, 0])
                xt_s = px.tile([P, NKT, KV], F8, tag="xt")
                for kt in range(NKT):
                    nc.sync.dma_start(out=xt_s[:, kt], in_=xt[:, kt])
                wk_t0 = wpool.tile([P, NKT, P], F8, tag="wqkv")
                nc.sync.dma_start(out=wk_t0[:], in_=wk[:, 0])
                # Q/K in fp8, scores-DR grouped layout: tile ot = (u, c)
                # with u = ot//2 (head group 4u..4u+3), c = ot%2 (dh parity);
                # partition 32*g+ki holds head 4u+g, dh = 2*ki + c.
                qt_s = pproj.tile([P, NKT, QTOK], F8, tag="qt")
                kt_s = pproj.tile([P, NKT, KV], F8, tag="kt")
                # augmented V: per (kv-tile j, head pair hp) 193 cols:
                # [Edims 64 | Eden 1 | Oden 1 | zeros 63 | Odims 64]
                vaug = pproj.tile([P, NKT, NKT, VW], BF16, tag="vaug")
                nc.gpsimd.memset(vaug[:, :, :, 66:129], 0.0)

                wv_s = px.tile([P, NKT, D], F8, tag="wv")
                for c in range(2):
                    nc.sync.dma_start(
                        out=wv_s[:, 4 * c : 4 * (c + 1)], in_=wv[:, 4 * c : 4 * (c + 1)]
                    )
                val_s = pproj.tile([P, NKT, NKT, 2], BF16, tag="val")
                nc.sync.dma_start(out=val_s[:], in_=valid16[:])
                xres_s = pproj.tile([P, NKT, QTOK], F32, tag="xres")
                b1r_s = persist.tile([P, NOT1], F32, tag="b1r")
                b1e_s = persist.tile([P, NOT1], F32, tag="b1e")
                small = {}
                sum1_sb = pproj.tile([P, 512], F32, tag="sum1")
                sumsq1_sb = pproj.tile([P, 512], F32, tag="sumsq1")

                qk_done = set()
                v_done = {0: 0, 1: 0}

                def emit_qproj(ot):
                    if ot == 0:
                        wq_t = wq_t0
                    else:
                        wq_t = wpool.tile([P, NKT, P], F8, tag="wqkv")
                        nc.sync.dma_start(out=wq_t[:], in_=wq[:, ot])
                    ps = ps_fill.tile([P, 512], F32, tag="fill")
                    for kp in range(NKT // 2):
                        nc.tensor.matmul(
                            ps[:],
                            wq_t[:, 2 * kp : 2 * kp + 2],
                            xt_s[:, 2 * kp : 2 * kp + 2, 512:1024],
                            start=(kp == 0),
                            stop=(kp == NKT // 2 - 1),
                            perf_mode=DR,
                        )
                    nc.vector.tensor_copy(qt_s[:, ot], ps[:])

                wk_ts = {0: wk_t0}

                def emit_kproj(ot, tb):
                    if tb == 0 and ot not in wk_ts:
                        wk_t = wpool.tile([P, NKT, P], F8, tag="wqkv")
                        nc.sync.dma_start(out=wk_t[:], in_=wk[:, ot])
                        wk_ts[ot] = wk_t
                    wk_t = wk_ts[ot]
                    ps = ps_fill.tile([P, 512], F32, tag="fill")
                    for kp in range(NKT // 2):
                        nc.tensor.matmul(
                            ps[:],
                            wk_t[:, 2 * kp : 2 * kp + 2],
                            xt_s[:, 2 * kp : 2 * kp + 2, tb * 512 : (tb + 1) * 512],
                            start=(kp == 0),
                            stop=(kp == NKT // 2 - 1),
                            perf_mode=DR,
                        )
                    if tb == 0:
                        nc.scalar.copy(kt_s[:, ot, 0:512], ps[:])
                    else:
                        nc.vector.tensor_copy(kt_s[:, ot, 512:1024], ps[:])
                        qk_done.add(ot)

                def emit_vproj(db, tk):
                    if db == 0:
                        # den columns for all 8 pairs x 2 parities
                        nc.vector.tensor_copy(vaug[:, tk, :, 64:66], val_s[:, tk])
                    ps = ps_fill.tile([P, 4, P], F32, tag="fill")
                    for kp in range(NKT // 2):
                        nc.tensor.matmul(
                            ps[:, :, :],
                            xt_s[:, 2 * kp : 2 * kp + 2, tk * P : (tk + 1) * P],
                            wv_s[:, 2 * kp : 2 * kp + 2, db * 512 : (db + 1) * 512],
                            start=(kp == 0),
                            stop=(kp == NKT // 2 - 1),
                            perf_mode=DR,
                        )
                    hp0 = db * 4
                    nc.vector.tensor_copy(
                        vaug[:, tk, hp0 : hp0 + 4, 0:64], ps[:, :, 0:64]
                    )
                    nc.vector.tensor_copy(
                        vaug[:, tk, hp0 : hp0 + 4, 129:193], ps[:, :, 64:128]
                    )
                    v_done[db] += 1

                stats_pending = []

                def flush_stats():
                    while stats_pending:
                        cast_t, sq_t, hp = stats_pending.pop(0)
                        pss = ps_sc.tile([P, 1024], F32, tag="sc")
                        nc.tensor.matmul(
                            pss[:, 0:512], ones128[:], cast_t[:], start=True, stop=True
                        )
                        nc.tensor.matmul(
                            pss[:, 512:1024], ones128[:], sq_t[:], start=True, stop=True
                        )
                        if hp == 0:
                            nc.vector.tensor_copy(sum1_sb[:], pss[:, 0:512])
                            nc.vector.tensor_copy(sumsq1_sb[:], pss[:, 512:1024])
                        else:
                            nc.vector.tensor_tensor(
                                sum1_sb[:], sum1_sb[:], pss[:, 0:512], OP.add
                            )
                            nc.vector.tensor_tensor(
                                sumsq1_sb[:], sumsq1_sb[:], pss[:, 512:1024], OP.add
                            )

                def emit_post(p):
                    h, cps, ctxn = p
                    hp, par = h // 2, h % 2
                    po = 64 * par
                    flush_stats()
                    # broadcast the bf16 reciprocal row across the 64 ctx
                    # partitions via a ones-matmul (bcp shares the fill pool)
                    bcp = ps_fill.tile([P, 512], F32, tag="fill")
                    if par == 0:
                        nc.tensor.matmul(
                            bcp[0:64],
                            ones_r64[:, 0:64],
                            srowE_bf[:],
                            start=True,
                            stop=True,
                        )
                    else:
                        nc.tensor.matmul(
                            bcp[64:128],
                            ones_r0[:, 0:64],
                            srowO_bf[:],
                            start=True,
                            stop=True,
                        )
                    bc_sb = tmp2.tile([P, 512], BF16, tag="bcsb")
                    nc.vector.tensor_copy(bc_sb[po : po + 64], bcp[po : po + 64])
                    nc.vector.tensor_tensor(
                        ctxn[po : po + 64],
                        cps[po : po + 64],
                        bc_sb[po : po + 64],
                        OP.mult,
                    )
                    if par == 1:
                        # pair finished: residual add + LN1 stats (Pool
                        # takes the add + cast + square; PE the stats,
                        # deferred one slot so PE never waits on Pool).
                        # Last pair runs on DVE: Pool's 0.42 efficiency
                        # would sit on the attention->LN1 critical path.
                        eng = nc.vector if hp == H // 2 - 1 else nc.gpsimd
                        eng.tensor_add(
                            xres_s[:, hp], xres_s[:, hp], ctxn[:]
                        )
                        cast_t = tmp.tile([P, 512], BF16, tag="lncast")
                        sq_t = tmp.tile([P, 512], BF16, tag="lnsq")
                        eng.tensor_copy(cast_t[:], xres_s[:, hp])
                        eng.tensor_mul(
                            sq_t[:], xres_s[:, hp], xres_s[:, hp]
                        )
                        stats_pending.append((cast_t, sq_t, hp))

                ctxn = None
                # packed expt: per-j live query range [off_j, 512) stored
                # contiguously; POFF[j] is the packed start, NCOL[j] the width
                NCOL = [512 - max(0, j - 4) * P for j in range(NKT)]
                POFF = [0] * NKT
                for j in range(1, NKT):
                    POFF[j] = POFF[j - 1] + NCOL[j - 1]

                def emit_ctx(pr):
                    nonlocal ctxn
                    h, expt = pr
                    hp, par = h // 2, h % 2
                    cps = ps_ctx.tile([P, 512], F32, tag="ctx")
                    lsl = (0, 65) if par == 0 else (65, VW)
                    m = lsl[1] - lsl[0]
                    for j in range(NKT):
                        off = max(0, j - 4) * P
                        nc.tensor.matmul(
                            cps[0:m, off:512],
                            vaug[:, j, hp, lsl[0] : lsl[1]],
                            expt[:, POFF[j] : POFF[j] + NCOL[j]],
                            start=(j == 0),
                            stop=(j == NKT - 1),
                        )
                    with nc.allow_low_precision(
                        reason="softmax denominator reciprocal to bf16"
                    ):
                        if par == 0:
                            nc.vector.reciprocal(srowE_bf[64:65], cps[64:65])
                        else:
                            nc.vector.reciprocal(srowO_bf[0:1], cps[0:1])
                    if DEBUG_TAPS and h == DBG_HEAD:
                        dbg_cps = nc.dram_tensor(
                            "dbg_cps", [P, 512], F32, kind="ExternalOutput"
                        ).ap()
                        dbg_sb = persist.tile([P, 512], F32, tag="dbgsb")
                        nc.vector.memset(dbg_sb[:], 0.0)
                        _r0, _r1 = (0, 65) if par == 0 else (64, 128)
                        nc.vector.tensor_copy(dbg_sb[_r0:_r1], cps[_r0:_r1])
                        if par == 1:
                            nc.vector.tensor_copy(dbg_sb[0:1], cps[0:1])
                        nc.sync.dma_start(out=dbg_cps[:], in_=dbg_sb[:])
                        dbg_expt = nc.dram_tensor(
                            "dbg_expt", [P, 3328], BF16, kind="ExternalOutput"
                        ).ap()
                        nc.sync.dma_start(out=dbg_expt[:], in_=expt[:])
                        dbg_vaug = nc.dram_tensor(
                            "dbg_vaug", [P, NKT, VW], BF16, kind="ExternalOutput"
                        ).ap()
                        nc.sync.dma_start(out=dbg_vaug[:], in_=vaug[:, :, hp])
                    if par == 0:
                        ctxn = tmp2.tile([P, 512], F32, tag="ctxn")
                    return (h, cps, ctxn)

                def emit_scores(h):
                    # DoubleRow over dh: contraction (ki 32, parity 2); head
                    # h lives at partition group 32*(h%4) of ot pair
                    # (2*(h//4), 2*(h//4)+1). j-tiles are computed two per
                    # 2-bank psum so each Exp covers a pair in one shot.
                    u, sub = h // 4, h % 4
                    b0 = 32 * sub
                    expt = pattn.tile([P, 3328], BF16, tag="expt")
                    for pj in range(4):
                        j0 = 2 * pj
                        w0, w1 = NCOL[j0], NCOL[j0 + 1]
                        # two independent accumulation groups must not share
                        # a PSUM bank: place the second j at a 512 offset
                        po1 = max(w0, 512)
                        ps = ps_sc.tile([P, 1024], F32, tag="sc")
                        for j, w, po in ((j0, w0, 0), (j0 + 1, w1, po1)):
                            off = 512 - w
                            nc.tensor.matmul(
                                ps[:, po : po + w],
                                kt_s[b0 : b0 + 32, 2 * u : 2 * u + 2,
                                     j * P : (j + 1) * P],
                                qt_s[b0 : b0 + 32, 2 * u : 2 * u + 2, off:512],
                                start=True,
                                stop=True,
                                perf_mode=DR,
                                tile_position=(b0, 0),
                            )
                        if po1 == w0:
                            nc.scalar.activation(
                                expt[:, POFF[j0] : POFF[j0] + w0 + w1],
                                ps[:, 0 : w0 + w1],
                                AF.Exp,
                                scale=0.125 / (WSQK * WSQK),
                            )
                        else:
                            nc.scalar.activation(
                                expt[:, POFF[j0] : POFF[j0] + w0],
                                ps[:, 0:w0],
                                AF.Exp,
                                scale=0.125 / (WSQK * WSQK),
                            )
                            nc.scalar.activation(
                                expt[:, POFF[j0 + 1] : POFF[j0 + 1] + w1],
                                ps[:, po1 : po1 + w1],
                                AF.Exp,
                                scale=0.125 / (WSQK * WSQK),
                            )
                    for j in range(4, NKT):
                        # zero the masked upper triangle of the diagonal
                        # query block post-exp (Pool, off the hot engines)
                        nc.gpsimd.affine_select(
                            out=expt[:, POFF[j] : POFF[j] + P],
                            in_=expt[:, POFF[j] : POFF[j] + P],
                            compare_op=OP.is_ge,
                            fill=0.0,
                            base=0,
                            pattern=[[1, P]],
                            channel_multiplier=-1,
                        )
                    return (h, expt)

                # filler units: Q/K projections + V-proj tiles, ordered by
                # consumption deadline, drained during the head loop
                fillers = []
                for ot in (2, 3):
                    fillers += [
                        lambda o=ot: emit_qproj(o),
                        lambda o=ot: emit_kproj(o, 0),
                        lambda o=ot: emit_kproj(o, 1),
                    ]
                fillers += [lambda t=tk: emit_vproj(0, t) for tk in range(NKT)]
                for ot in (4, 5):
                    fillers += [
                        lambda o=ot: emit_qproj(o),
                        lambda o=ot: emit_kproj(o, 0),
                        lambda o=ot: emit_kproj(o, 1),
                    ]
                fillers += [lambda t=tk: emit_vproj(1, t) for tk in range(NKT)]
                for ot in (6, 7):
                    fillers += [
                        lambda o=ot: emit_qproj(o),
                        lambda o=ot: emit_kproj(o, 0),
                        lambda o=ot: emit_kproj(o, 1),
                    ]
                fillers.reverse()  # consume via pop()

                def drain(n):
                    for _ in range(n):
                        if fillers:
                            fillers.pop()()

                def need_qk(u):
                    # scores for head group u needs both parity tiles 2u, 2u+1
                    while not ({2 * u, 2 * u + 1} <= qk_done):
                        assert fillers, f"filler queue dry before qk pair {u}"
                        fillers.pop()()

                def need_v(db):
                    while v_done[db] < NKT:
                        assert fillers, f"filler queue dry before v {db}"
                        fillers.pop()()

                # warmup: head-group-0 Q/K, then 3 heads of scores while the
                # filler queue builds V/QK state; first ctx after V db0 done
                emit_qproj(0)
                emit_kproj(0, 0)
                emit_kproj(0, 1)
                emit_qproj(1)
                emit_kproj(1, 0)
                emit_kproj(1, 1)
                from collections import deque

                prevs = deque()
                prevs.append(emit_scores(0))
                drain(3)
                prevs.append(emit_scores(1))
                nc.sync.dma_start(out=xres_s[:], in_=xres[:])
                drain(3)
                prevs.append(emit_scores(2))
                drain(4)
                need_v(0)
                nc.sync.dma_start(out=b1r_s[:], in_=b1r[:])
                nc.sync.dma_start(out=b1e_s[:], in_=b1e[:])
                for nm, _src in (
                    ("b2t", b2t),
                    ("g1t", g1t),
                    ("be1t", be1t),
                    ("g2t", g2t),
                    ("be2t", be2t),
                ):
                    t = persist.tile([P, NKT], F32, tag=nm)
                    nc.sync.dma_start(out=t[:], in_=_src[:])
                    small[nm] = t
                # prefetch the first fc1/fc2 weight chunks during attention
                # so the FFN phases never wait on the serial SP DMA queue
                for _c in range(3):
                    load_w1(_c)
                load_w2(0)
                load_w2(1)
                pending = emit_ctx(prevs.popleft())
                for h in range(3, H):
                    need_qk(h // 4)
                    prevs.append(emit_scores(h))
                    drain(1)
                    emit_post(pending)
                    nh = prevs[0][0]
                    need_v(nh // 8)
                    pending = emit_ctx(prevs.popleft())
                drain(len(fillers))
                while prevs:
                    emit_post(pending)
                    need_v(1)
                    pending = emit_ctx(prevs.popleft())
                emit_post(pending)
                flush_stats()

                if DEBUG_TAPS:
                    dbg_xres = nc.dram_tensor(
                        "dbg_xres", [P, NKT, QTOK], F32, kind="ExternalOutput"
                    ).ap()
                    nc.sync.dma_start(out=dbg_xres[:], in_=xres_s[:])
                    dbg_xt = nc.dram_tensor(
                        "dbg_xt", [P, NKT, KV], F8, kind="ExternalOutput"
                    ).ap()
                    nc.sync.dma_start(out=dbg_xt[:], in_=xt_s[:])
                    dbg_kt = nc.dram_tensor(
                        "dbg_kt", [P, NKT, KV], F8, kind="ExternalOutput"
                    ).ap()
                    nc.sync.dma_start(out=dbg_kt[:], in_=kt_s[:])
                    dbg_stats = nc.dram_tensor(
                        "dbg_stats", [P, 2, 512], F32, kind="ExternalOutput"
                    ).ap()
                    nc.sync.dma_start(out=dbg_stats[:, 0], in_=sum1_sb[:])
                    nc.sync.dma_start(out=dbg_stats[:, 1], in_=sumsq1_sb[:])
                    dbg_srow = nc.dram_tensor(
                        "dbg_srow", [P, 2, QTOK], BF16, kind="ExternalOutput"
                    ).ap()
                    nc.sync.dma_start(out=dbg_srow[:, 0], in_=srowE_bf[:])
                    nc.sync.dma_start(out=dbg_srow[:, 1], in_=srowO_bf[:])

                # ---- phase 3: LN1 (stats already accumulated) ----
                # ln1_bf (bf16) is the fc2 residual; x8/x8e are the fp8
                # hi/lo pair feeding the compensated fc1 DoubleRow passes.
                ln1_bf = lnp.tile([P, NKT, QTOK], BF16, tag="ln1")
                x8 = lnp.tile([P, NKT, QTOK], F8, tag="x8")
                x8e = lnp.tile([P, NKT, QTOK], F8, tag="x8e")
                mean1, rstd1 = ln_meanvar(sum1_sb, sumsq1_sb)
                for kt in range(NKT):
                    ln_apply(
                        xres_s[:, kt], mean1, rstd1,
                        small["g1t"], small["be1t"], ln1_bf[:, kt], kt,
                    )
                    nc.gpsimd.tensor_copy(x8[:, kt], ln1_bf[:, kt])
                    nc.vector.tensor_tensor(
                        x8e[:, kt], ln1_bf[:, kt], x8[:, kt], OP.subtract
                    )
                pxstack.close()
                attn_stack.close()

            # ---- phase 4: fc1 + selu (w1 in JIT 4-ot chunks, depth 2) ----
            pffn_stack = contextlib.ExitStack()
            pffn = pffn_stack.enter_context(tc.tile_pool(name="pffn", bufs=1))
            ps_mm = pffn_stack.enter_context(
                tc.tile_pool(name="ps_mm", bufs=3, space="PSUM")
            )
            ps_x = pffn_stack.enter_context(
                tc.tile_pool(name="ps_x", bufs=1, space="PSUM")
            )
            h8 = pffn.tile([P, NOT1, QTOK], F8, tag="h8")
            h8e = pffn.tile([P, NOT1, QTOK], F8, tag="h8e")
            for ot in range(NOT1):
                w1h, w1l = w1bufs[(ot // 2) % NW1B]
                if ot % 2 == 0:
                    load_w1(ot // 2 + 3)  # chunks 0..2 preloaded in attention
                if ot % 4 == 3:
                    ps = ps_x.tile([P, 512], F32, tag="x")
                else:
                    ps = ps_mm.tile([P, 512], F32, tag="mm")
                passes = [(w1h, x8), (w1l, x8), (w1h, x8e)]
                for pi, (wt, xq) in enumerate(passes):
                    for kp in range(NKT // 2):
                        nc.tensor.matmul(
                            ps[:],
                            wt[:, ot % 2, 2 * kp : 2 * kp + 2],
                            xq[:, 2 * kp : 2 * kp + 2],
                            start=(pi == 0 and kp == 0),
                            stop=(pi == 2 and kp == NKT // 2 - 1),
                            perf_mode=DR,
                        )
                p_t = tmp.tile([P, QTOK], F32, tag="selup")
                nc.scalar.activation(
                    p_t[:],
                    ps[:],
                    AF.Relu,
                    scale=SELU_S / WS,
                    bias=b1r_s[:, ot : ot + 1],
                )
                e_t = tmp.tile([P, QTOK], F32, tag="selue")
                nc.scalar.activation(
                    e_t[:], ps[:], AF.Exp, scale=1.0 / WS, bias=b1e_s[:, ot : ot + 1]
                )
                # selu(z) = min(sa*e^z - sa, s*relu(z))
                nc.vector.scalar_tensor_tensor(
                    p_t[:], e_t[:], SELU_SA, p_t[:], OP.subtract, OP.min
                )
                nc.gpsimd.tensor_copy(h8[:, ot], p_t[:])
                nc.vector.tensor_tensor(h8e[:, ot], p_t[:], h8[:, ot], OP.subtract)

            # ---- phase 5: fc2 + residual + LN2 + store (full 512 width) ----
            ps_stat2 = pffn_stack.enter_context(
                tc.tile_pool(name="ps_stat2", bufs=2, space="PSUM")
            )
            res2 = pffn.tile([P, NKT, QTOK], F32, tag="res2")
            ps0_2 = ps_stat2.tile([P, 512], F32, tag="stat2")
            ps1_2 = ps_stat2.tile([P, 512], F32, tag="stat2")
            for ot in range(NKT):
                w2h, w2l = w2bufs[ot % NW2B]
                load_w2(ot + 2)  # ots 0,1 preloaded in attention
                if ot % 4 == 3:
                    ps = ps_x.tile([P, 512], F32, tag="x")
                else:
                    ps = ps_mm.tile([P, 512], F32, tag="mm")
                passes = [(w2h, h8), (w2l, h8), (w2h, h8e)]
                for pi, (wt, hq) in enumerate(passes):
                    for kp in range(NOT1 // 2):
                        nc.tensor.matmul(
                            ps[:],
                            wt[:, 2 * kp : 2 * kp + 2],
                            hq[:, 2 * kp : 2 * kp + 2],
                            start=(pi == 0 and kp == 0),
                            stop=(pi == 2 and kp == NOT1 // 2 - 1),
                            perf_mode=DR,
                        )
                t1 = tmp2.tile([P, QTOK], F32, tag="r2t")
                nc.scalar.activation(
                    t1[:],
                    ps[:],
                    AF.Identity,
                    scale=1.0 / WS,
                    bias=small["b2t"][:, ot : ot + 1],
                )
                nc.vector.tensor_tensor(
                    res2[:, ot], t1[:], ln1_bf[:, ot], OP.add
                )
                cast_t = tmp.tile([P, QTOK], BF16, tag="lncast2")
                sq_t = tmp.tile([P, QTOK], BF16, tag="lnsq2")
                nc.vector.tensor_copy(cast_t[:], res2[:, ot])
                nc.scalar.activation(sq_t[:], res2[:, ot], AF.Square)
                ln_stats_mm(ps0_2, ps1_2, cast_t, sq_t, ot)
            mean2, rstd2 = ln_meanvar(ps0_2, ps1_2)
            for kt in range(NKT):
                # spread the tail normalize across DVE and Pool so the
                # final 8-tile chain isn't serialized on one engine; the
                # scale-bias always runs on the otherwise-idle Act engine
                eng = nc.gpsimd if kt in (2, 5) else nc.vector
                ln_apply(
                    res2[:, kt], mean2, rstd2,
                    small["g2t"], small["be2t"], res2[:, kt], kt,
                    eng=eng,
                )
                nc.sync.dma_start(out=out[:, kt], in_=res2[:, kt])
            pffn_stack.close()

    _legalize_waits(nc)
    return nc


_NC_CACHE = None
TRACE = False
LAST_EXEC_NS = None


def _get_nc():
    global _NC_CACHE
    if _NC_CACHE is None:
        _NC_CACHE = _build_nc()
    return _NC_CACHE


def _tile_w(a):
    """[Din, O] -> [P, O//P(ot), Din//P(kt), P] with ot-contiguous DMA slices."""
    Din, O = a.shape
    return np.ascontiguousarray(
        a.reshape(Din // P, P, O // P, P).transpose(1, 2, 0, 3)
    )


def _pp(v, n):
    """[n*P] -> [P, n] per-partition layout."""
    return np.ascontiguousarray(v.reshape(n, P).T)


def kernel(X, wq, wk, wv, ln1_g, ln1_b, w1, b1, w2, b2, ln2_g, ln2_b):
    from concourse.bass_utils import run_bass_kernel_spmd

    X = np.asarray(X, np.float32)
    bf = ml_dtypes.bfloat16
    f8 = ml_dtypes.float8_e4m3  # IEEE flavor — matches bass float8e4

    def hilo(wt):
        hi = wt.astype(f8)
        lo = (wt - hi.astype(np.float32)).astype(f8)
        return hi, lo

    # scores-DR out-dim permutation: slot (ot, i) holds projection row
    # head*64 + dh with head = 4*(ot//2) + i//32, dh = 2*(i%32) + ot%2,
    # so head h sits at partition group 32*(h%4) of tiles (2u, 2u+1)
    # with the dh parity split across the tile pair (DoubleRow Ko dim).
    qperm = np.empty(D, np.int64)
    for _ot in range(NKT):
        for _i in range(P):
            _h = 4 * (_ot // 2) + _i // 32
            _dh = 2 * (_i % 32) + (_ot % 2)
            qperm[_ot * P + _i] = _h * 64 + _dh
    wqT = _tile_w((WSQK * np.asarray(wq, np.float32).T)[:, qperm]).astype(f8)
    wkT = _tile_w((WSQK * np.asarray(wk, np.float32).T)[:, qperm]).astype(f8)
    wvT = np.ascontiguousarray(
        WS * np.asarray(wv, np.float32).T.reshape(NKT, P, D).transpose(1, 0, 2)
    ).astype(f8)
    w1hi, w1lo = hilo(_tile_w(WS * np.asarray(w1, np.float32).T))
    w2hi, w2lo = hilo(_tile_w(WS * np.asarray(w2, np.float32).T))
    b1 = np.asarray(b1, np.float32)
    shared = dict(
        wq=wqT,
        wk=wkT,
        wv=wvT,
        w1=w1hi,
        w1e=w1lo,
        w2=w2hi,
        w2e=w2lo,
        b1r=_pp(SELU_S * b1, NOT1),
        b1e=_pp(b1 + LN_SA, NOT1),
        b2t=_pp(np.asarray(b2, np.float32), NKT),
        g1t=_pp(np.asarray(ln1_g, np.float32), NKT),
        be1t=_pp(np.asarray(ln1_b, np.float32), NKT),
        g2t=_pp(np.asarray(ln2_g, np.float32), NKT),
        be2t=_pp(np.asarray(ln2_b, np.float32), NKT),
    )

    in_maps = []
    for c in range(8):
        b, hf = c // 2, c % 2
        if hf == 1:
            xkv = X[b].T  # [D, L]
            valid = np.full(KV, WS, np.float32)
            xq = X[b, 512:]
        else:
            xkv = np.concatenate(
                [np.zeros((D, 512), np.float32), X[b, :512].T], axis=1
            )
            valid = np.concatenate(
                [np.zeros(512, np.float32), np.full(512, WS, np.float32)]
            )
            xq = X[b, :512]
        xt = (
            np.ascontiguousarray(xkv.reshape(NKT, P, KV).transpose(1, 0, 2))
        ).astype(f8)
        xres = np.ascontiguousarray(xq.T.reshape(NKT, P, QTOK).transpose(1, 0, 2))
        vt = valid.reshape(NKT, P).T  # [P, NKT]
        val16 = (
            np.repeat(vt[:, :, None], H, axis=2).reshape(P, NKT, NKT, 2).astype(bf)
        )
        m = dict(shared)
        m.update(xt=xt, xres=xres, valid16=np.ascontiguousarray(val16))
        in_maps.append(m)

    nc = _get_nc()
    global LAST_EXEC_NS
    if TRACE:
        res = run_bass_kernel_spmd(nc, in_maps, list(range(8)), trace=True)
        LAST_EXEC_NS = res.exec_time_ns
    else:
        res = run_bass_kernel_spmd(nc, in_maps, list(range(8)))

    out = np.empty((B, L, D), np.float32)
    for c in range(8):
        b, hf = c // 2, c % 2
        o = res.results[c]["out"]  # [P, NKT, QTOK]
        o = o.transpose(1, 0, 2).reshape(D, QTOK).T  # [QTOK, D]
        out[b, hf * 512 : hf * 512 + 512] = o
    return out



# revision 79
# speedup vs baseline: 1.2002x; 1.0917x over previous
"""Decoder-layer Trainium2 kernel: 8-core SPMD, single launch, no collectives.

Sharding: core c -> (batch b = c // 2, sequence-half hf = c % 2). Each core
computes the full decoder layer for 512 query tokens of one sequence.
All cores run ONE identical program over a canonical virtual sequence of
1024 kv tokens with queries at virtual positions 512..1023; first-half cores
get their 512 real tokens placed at virtual 512..1023 with zero-padded kv
prefix and a `valid` vector that zeroes the pad contribution to the softmax
denominator.

v2 changes vs baseline:
- softmax denominators ride along in the ctx matmul via an augmented V
  (per head-pair V layout [Edims|Eden|Oden|zeros63|Odims], 193 wide): even
  heads matmul M=65 -> dims at psum rows 0..63 + den at row 64; odd heads
  M=128 with a zero block -> den at row 0 + dims at rows 64..127. Kills the
  65536 rows of separate [1,512] denominator matmuls.
- ctx matmuls are causally restricted to the live query range per kv tile
  (like scores), saving another 12288 rows.
- LN1 stats matmuls run inline as each head pair finishes its xres tile.
- fc2 + LN2 run in two token-half passes so the final normalize/store of
  half 0 overlaps the fc2 matmuls of half 1.
"""

import sys

sys.path.insert(0, "/opt/trn_rl_repo")

import math

import numpy as np
import ml_dtypes

import concourse.bass as bass
import concourse.mybir as mybir
from concourse.tile import TileContext, TilePool
from concourse.vector_clock import ScopedClock

BF16 = mybir.dt.bfloat16
F8 = mybir.dt.float8e4
F32 = mybir.dt.float32
AF = mybir.ActivationFunctionType
OP = mybir.AluOpType
DR = mybir.MatmulPerfMode.DoubleRow
WS = 64.0  # fp8 weight pre-scale (wv/w1/w2)
# Q/K projections use a smaller pre-scale: bass float8e4 is IEEE e4m3
# (max finite 240, saturates to inf) and |K|*64 reaches ~290 on some
# batches; *32 keeps the fp8 Q/K copies comfortably finite.
WSQK = 32.0

B, L, D = 4, 1024, 1024
H, DH = 16, 64
DFF = 4 * D
P = 128
QTOK = 512  # query tokens per core
KV = 1024  # canonical kv length (virtual)
NKT = D // P  # 8 d-tiles
NOT1 = DFF // P  # 32 fc1 out tiles
MASK_NEG = -1.0e9
VW = 193  # augmented V width per head pair: [Ed 64|Eden|Oden|z 63|Od 64]

SELU_S = 1.0507009873554804934193349852946
SELU_A = 1.6732632423543772848170429916717
SELU_SA = SELU_S * SELU_A
LN_SA = math.log(SELU_SA)
LN_EPS = 1e-5


class PatchedTileContext(TileContext):
    """TileContext whose exit drain respects this walrus build's limit of
    ONE semaphore wait per instruction: the global-clock waits are spread
    across standalone NOPs and the butterfly barrier (whose sem-eq waits
    walrus rejects) is replaced by the NRT-expanded pseudo barrier."""

    def _drain_and_barrier(self, tick_clock, wait_clock):
        nc = self.nc
        carrier = nc.sync.nop()
        wait_clock.add_sem_waits(
            carrier.ins, ScopedClock({None: tick_clock.global_clock})
        )
        waits = list(carrier.ins.sync_info.on_wait)
        ups = list(carrier.ins.sync_info.on_update)
        if len(waits) > 1:
            carrier.ins.sync_info = mybir.SyncInfo(on_wait=[waits[0]], on_update=ups)
            for w in waits[1:]:
                extra = nc.sync.nop()
                extra.ins.sync_info = mybir.SyncInfo(on_wait=[w], on_update=[])
        for eng in nc.engines.values():
            eng.drain()
        nc._nrt_pseudo_barrier()
        popped = nc._tile_sem_poison_stack.pop()
        assert popped is self._sem_poison
        nc.clear_and_free_semaphores(list(self.sems.allocated().values()))
        nc._nrt_pseudo_barrier()


def _legalize_waits(nc):
    """This walrus build accepts at most ONE semaphore wait per instruction.
    Tile's sem-assignment can attach several; hoist the extras onto same-engine
    NOPs inserted immediately before the instruction (waits are a conjunction,
    so a sequence of single-wait stalls is equivalent)."""
    n = 0
    for fn in nc.m.functions:
        for blk in fn.blocks:
            out = []
            changed = False
            for inst in blk.instructions:
                si = getattr(inst, "sync_info", None)
                if si is not None and len(si.on_wait) > 1:
                    waits = list(si.on_wait)
                    for w in waits[:-1]:
                        nop = mybir.InstNoOp(name=f"waitnop_{n}", ins=[], outs=[])
                        n += 1
                        nop.engine = inst.engine
                        nop.sync_info = mybir.SyncInfo(on_wait=[w], on_update=[])
                        out.append(nop)
                    inst.sync_info = mybir.SyncInfo(
                        on_wait=[waits[-1]], on_update=list(si.on_update)
                    )
                    changed = True
                out.append(inst)
            if changed:
                blk.instructions = out
    return n


DEBUG_TAPS = False
DBG_HEAD = 0


def _build_nc():
    nc = bass.Bass("TRN2", target_bir_lowering=False, debug=False, num_devices=8)

    def din(name, shape, dt):
        return nc.dram_tensor(name, shape, dt, kind="ExternalInput").ap()

    xt = din("xt", [P, NKT, KV], F8)  # X[b].T tiled, virtual-padded
    xres = din("xres", [P, NKT, QTOK], F32)  # q tokens transposed, fp32
    valid16 = din("valid16", [P, NKT, NKT, 2], BF16)  # WS flag, [8hp x 2]
    wq = din("wq", [P, NKT, NKT, P], F8)  # [dpart, ot, kt, o], x WS
    wk = din("wk", [P, NKT, NKT, P], F8)
    wv = din("wv", [P, NKT, D], F8)  # rhs layout [dpart, kt, o], x WS
    w1 = din("w1", [P, NOT1, NKT, P], F8)  # fp8(WS*w1^T)
    w1e = din("w1e", [P, NOT1, NKT, P], F8)  # fp8 residual of the above
    w2 = din("w2", [P, NKT, NOT1, P], F8)
    w2e = din("w2e", [P, NKT, NOT1, P], F8)
    b1r = din("b1r", [P, NOT1], F32)  # SELU_S * b1
    b1e = din("b1e", [P, NOT1], F32)  # b1 + ln(SELU_S*SELU_A)
    b2t = din("b2t", [P, NKT], F32)
    g1t = din("g1t", [P, NKT], F32)
    be1t = din("be1t", [P, NKT], F32)
    g2t = din("g2t", [P, NKT], F32)
    be2t = din("be2t", [P, NKT], F32)
    out = nc.dram_tensor("out", [P, NKT, QTOK], F32, kind="ExternalOutput").ap()

    with PatchedTileContext(nc) as tc:
        import contextlib

        with contextlib.ExitStack() as ctx:
            persist = ctx.enter_context(tc.tile_pool(name="persist", bufs=1))
            bc = ctx.enter_context(tc.tile_pool(name="bc", bufs=1))
            wpool = ctx.enter_context(tc.tile_pool(name="wpool", bufs=4))
            tmp = ctx.enter_context(tc.tile_pool(name="tmp", bufs=2))
            tmp2 = ctx.enter_context(tc.tile_pool(name="tmp2", bufs=2))
            lnp = ctx.enter_context(tc.tile_pool(name="lnp", bufs=1))
            w1pool = ctx.enter_context(tc.tile_pool(name="w1pool", bufs=1))

            # ---- constants ----
            NW1B, NW2B = 5, 4
            w1bufs = [
                (
                    w1pool.tile(
                        [P, 2, NKT, P], F8, tag=f"w1{i}h", name=f"w1{i}h"
                    ),
                    w1pool.tile(
                        [P, 2, NKT, P], F8, tag=f"w1{i}e", name=f"w1{i}e"
                    ),
                )
                for i in range(NW1B)
            ]
            w2bufs = [
                (
                    w1pool.tile(
                        [P, NOT1, P], F8, tag=f"w2{i}h", name=f"w2{i}h"
                    ),
                    w1pool.tile(
                        [P, NOT1, P], F8, tag=f"w2{i}e", name=f"w2{i}e"
                    ),
                )
                for i in range(NW2B)
            ]

            def load_w1(chunk):
                if chunk < NOT1 // 2:
                    hb, lb = w1bufs[chunk % NW1B]
                    nc.sync.dma_start(out=hb[:], in_=w1[:, 2 * chunk : 2 * chunk + 2])
                    nc.sync.dma_start(out=lb[:], in_=w1e[:, 2 * chunk : 2 * chunk + 2])

            def load_w2(ot):
                if ot < NKT:
                    hb, lb = w2bufs[ot % NW2B]
                    nc.sync.dma_start(out=hb[:], in_=w2[:, ot])
                    nc.sync.dma_start(out=lb[:], in_=w2e[:, ot])
            ones128 = persist.tile([P, P], BF16, tag="ones128")
            nc.gpsimd.memset(ones128[:], 1.0)
            ones_r0 = persist.tile([P, P], BF16, tag="ones_r0")
            nc.gpsimd.memset(ones_r0[:], 0.0)
            nc.gpsimd.memset(ones_r0[0:1, :], 1.0)
            ones_r64 = persist.tile([P, P], BF16, tag="ones_r64")
            nc.gpsimd.memset(ones_r64[:], 0.0)
            nc.gpsimd.memset(ones_r64[64:65, :], 1.0)
            srowE_bf = persist.tile([P, QTOK], BF16, tag="srowEbf")
            nc.vector.memset(srowE_bf[:], 0.0)
            srowO_bf = persist.tile([P, QTOK], BF16, tag="srowObf")
            nc.vector.memset(srowO_bf[:], 0.0)
            eps_ap = persist.tile([P, 1], F32, tag="eps")
            nc.gpsimd.memset(eps_ap[:], LN_EPS)

            def ln_stats_mm(ps0, ps1, cast_t, sq_t, kt, n=NKT, ncols=QTOK):
                nc.tensor.matmul(
                    ps0[:, 0:ncols],
                    ones128[:],
                    cast_t[:],
                    start=(kt == 0),
                    stop=(kt == n - 1),
                )
                nc.tensor.matmul(
                    ps1[:, 0:ncols],
                    ones128[:],
                    sq_t[:],
                    start=(kt == 0),
                    stop=(kt == n - 1),
                )

            def ln_meanvar(ps0, ps1, ncols=QTOK):
                """stats psums -> (mean, rstd) broadcast tiles.

                rstd = exp(-0.5*ln(var+eps)): Ln and Exp share an Act table
                (natural_log_exp_and_others) with Relu/Identity/Square, so
                this never forces the 1.3us act-table reload that Sqrt would.
                """
                if ps1 is None:
                    # ps0 is a [P, 2*ncols] psum holding [sum | sumsq]:
                    # scale both with one DVE op into an adjacent pair
                    mv = bc.tile([P, 2 * ncols], F32, tag="meanvar")
                    nc.vector.tensor_scalar_mul(mv[:], ps0[:, 0 : 2 * ncols], 1.0 / D)
                    mean_bc, var_bc = mv[:, 0:ncols], mv[:, ncols : 2 * ncols]
                else:
                    mean_t = bc.tile([P, ncols], F32, tag="mean", name="mean_t")
                    mean_bc = mean_t[:]
                    nc.vector.tensor_scalar_mul(mean_bc, ps0[:, 0:ncols], 1.0 / D)
                    var_t = bc.tile([P, ncols], F32, tag="var", name="var_t")
                    var_bc = var_t[:]
                    nc.vector.tensor_scalar_mul(var_bc, ps1[:, 0:ncols], 1.0 / D)
                m2 = tmp2.tile([P, ncols], F32, tag="lnt")
                nc.vector.tensor_tensor(m2[:], mean_bc, mean_bc, OP.mult)
                nc.vector.tensor_tensor(var_bc, var_bc, m2[:], OP.subtract)
                nc.scalar.activation(var_bc, var_bc, AF.Ln, bias=eps_ap[:])
                nc.scalar.activation(var_bc, var_bc, AF.Exp, scale=-0.5)
                return mean_bc, var_bc

            def ln_apply(
                src_kt, mean_bc, var_bc, g_ap, b_ap, dst_kt, kt, ncols=QTOK,
                eng=None, fin_act=None,
            ):
                eng = eng or nc.vector
                t1 = tmp2.tile([P, ncols], F32, tag="lnt")
                eng.tensor_tensor(t1[:], src_kt, mean_bc[:], OP.subtract)
                eng.tensor_tensor(t1[:], t1[:], var_bc[:], OP.mult)
                if fin_act if fin_act is not None else (kt % 2 == 0):
                    nc.scalar.activation(
                        dst_kt,
                        t1[:],
                        AF.Identity,
                        scale=g_ap[:, kt : kt + 1],
                        bias=b_ap[:, kt : kt + 1],
                    )
                else:
                    nc.vector.tensor_scalar(
                        dst_kt,
                        t1[:],
                        g_ap[:, kt : kt + 1],
                        b_ap[:, kt : kt + 1],
                        OP.mult,
                        OP.add,
                    )

            # ---- phase 1+2 fused: QKV projections + attention ----
            with tc.tile_pool(name="pproj", bufs=1) as pproj:
                import contextlib as _ctl

                attn_stack = _ctl.ExitStack()
                pattn = attn_stack.enter_context(tc.tile_pool(name="pattn", bufs=3))
                ps_ctx = attn_stack.enter_context(
                    tc.tile_pool(name="ps_ctx", bufs=2, space="PSUM")
                )
                ps_sc = attn_stack.enter_context(
                    tc.tile_pool(name="ps_sc", bufs=2, space="PSUM")
                )
                ps_fill = attn_stack.enter_context(
                    tc.tile_pool(name="ps_fill", bufs=2, space="PSUM")
                )
                pxstack = _ctl.ExitStack()
                px = pxstack.enter_context(tc.tile_pool(name="px", bufs=1))
                wq_t0 = wpool.tile([P, NKT, P], F8, tag="wqkv")
                nc.sync.dma_start(out=wq_t0[:], in_=wq[:, 0])
                xt_s = px.tile([P, NKT, KV], F8, tag="xt")
                nc.sync.dma_start(out=xt_s[:, 0:4], in_=xt[:, 0:4])
                nc.sync.dma_start(out=xt_s[:, 4:8], in_=xt[:, 4:8])
                wk_t0 = wpool.tile([P, NKT, P], F8, tag="wqkv")
                nc.sync.dma_start(out=wk_t0[:], in_=wk[:, 0])
                # Q/K in fp8, scores-DR grouped layout: tile ot = (u, c)
                # with u = ot//2 (head group 4u..4u+3), c = ot%2 (dh parity);
                # partition 32*g+ki holds head 4u+g, dh = 2*ki + c.
                qt_s = pproj.tile([P, NKT, QTOK], F8, tag="qt")
                kt_s = pproj.tile([P, NKT, KV], F8, tag="kt")
                # augmented V: per (kv-tile j, head pair hp) 193 cols:
                # [Edims 64 | Eden 1 | Oden 1 | zeros 63 | Odims 64]
                vaug = pproj.tile([P, NKT, NKT, VW], BF16, tag="vaug")
                nc.gpsimd.memset(vaug[:, 0:4, :, 66:129], 0.0)
                nc.vector.memset(vaug[:, 4:8, :, 66:129], 0.0)

                wv_s = px.tile([P, NKT, D], F8, tag="wv")
                nc.sync.dma_start(out=wv_s[:], in_=wv[:])
                val_s = pproj.tile([P, NKT, NKT, 2], BF16, tag="val")
                nc.sync.dma_start(out=val_s[:], in_=valid16[:])
                xres_s = pproj.tile([P, NKT, QTOK], F32, tag="xres")
                b1r_s = persist.tile([P, NOT1], F32, tag="b1r")
                b1e_s = persist.tile([P, NOT1], F32, tag="b1e")
                small = {}
                sum1_sb = pproj.tile([P, 512], F32, tag="sum1")
                sumsq1_sb = pproj.tile([P, 512], F32, tag="sumsq1")

                qk_done = set()
                v_done = {0: 0, 1: 0}

                def emit_qproj(ot):
                    if ot == 0:
                        wq_t = wq_t0
                    else:
                        wq_t = wpool.tile([P, NKT, P], F8, tag="wqkv")
                        nc.sync.dma_start(out=wq_t[:], in_=wq[:, ot])
                    ps = ps_fill.tile([P, 512], F32, tag="fill")
                    for kp in range(NKT // 2):
                        nc.tensor.matmul(
                            ps[:],
                            wq_t[:, 2 * kp : 2 * kp + 2],
                            xt_s[:, 2 * kp : 2 * kp + 2, 512:1024],
                            start=(kp == 0),
                            stop=(kp == NKT // 2 - 1),
                            perf_mode=DR,
                        )
                    nc.vector.tensor_copy(qt_s[:, ot], ps[:])

                wk_ts = {0: wk_t0}

                def emit_kproj(ot, tb):
                    if tb == 0 and ot not in wk_ts:
                        wk_t = wpool.tile([P, NKT, P], F8, tag="wqkv")
                        nc.sync.dma_start(out=wk_t[:], in_=wk[:, ot])
                        wk_ts[ot] = wk_t
                    wk_t = wk_ts[ot]
                    ps = ps_fill.tile([P, 512], F32, tag="fill")
                    for kp in range(NKT // 2):
                        nc.tensor.matmul(
                            ps[:],
                            wk_t[:, 2 * kp : 2 * kp + 2],
                            xt_s[:, 2 * kp : 2 * kp + 2, tb * 512 : (tb + 1) * 512],
                            start=(kp == 0),
                            stop=(kp == NKT // 2 - 1),
                            perf_mode=DR,
                        )
                    if tb == 0:
                        nc.scalar.copy(kt_s[:, ot, 0:512], ps[:])
                    else:
                        nc.vector.tensor_copy(kt_s[:, ot, 512:1024], ps[:])
                        qk_done.add(ot)

                def emit_vproj(db, tk):
                    if db == 0:
                        # den columns for all 8 pairs x 2 parities
                        nc.vector.tensor_copy(vaug[:, tk, :, 64:66], val_s[:, tk])
                    ps = ps_fill.tile([P, 4, P], F32, tag="fill")
                    for kp in range(NKT // 2):
                        nc.tensor.matmul(
                            ps[:, :, :],
                            xt_s[:, 2 * kp : 2 * kp + 2, tk * P : (tk + 1) * P],
                            wv_s[:, 2 * kp : 2 * kp + 2, db * 512 : (db + 1) * 512],
                            start=(kp == 0),
                            stop=(kp == NKT // 2 - 1),
                            perf_mode=DR,
                        )
                    hp0 = db * 4
                    nc.vector.tensor_copy(
                        vaug[:, tk, hp0 : hp0 + 4, 0:64], ps[:, :, 0:64]
                    )
                    nc.vector.tensor_copy(
                        vaug[:, tk, hp0 : hp0 + 4, 129:193], ps[:, :, 64:128]
                    )
                    v_done[db] += 1

                stats_pending = []

                def flush_stats():
                    while stats_pending:
                        cast_t, sq_t, hp = stats_pending.pop(0)
                        pss = ps_sc.tile([P, 1024], F32, tag="sc")
                        nc.tensor.matmul(
                            pss[:, 0:512], ones128[:], cast_t[:], start=True, stop=True
                        )
                        nc.tensor.matmul(
                            pss[:, 512:1024], ones128[:], sq_t[:], start=True, stop=True
                        )
                        if hp == 0:
                            nc.vector.tensor_copy(sum1_sb[:], pss[:, 0:512])
                            nc.vector.tensor_copy(sumsq1_sb[:], pss[:, 512:1024])
                        else:
                            nc.vector.tensor_tensor(
                                sum1_sb[:], sum1_sb[:], pss[:, 0:512], OP.add
                            )
                            nc.vector.tensor_tensor(
                                sumsq1_sb[:], sumsq1_sb[:], pss[:, 512:1024], OP.add
                            )

                def emit_post(p):
                    h, cps, ctxn = p
                    hp, par = h // 2, h % 2
                    po = 64 * par
                    flush_stats()
                    # broadcast the bf16 reciprocal row across the 64 ctx
                    # partitions via a ones-matmul (bcp shares the fill pool)
                    bcp = ps_fill.tile([P, 512], F32, tag="fill")
                    if par == 0:
                        nc.tensor.matmul(
                            bcp[0:64],
                            ones_r64[:, 0:64],
                            srowE_bf[:],
                            start=True,
                            stop=True,
                        )
                    else:
                        nc.tensor.matmul(
                            bcp[64:128],
                            ones_r0[:, 0:64],
                            srowO_bf[:],
                            start=True,
                            stop=True,
                        )
                    bc_sb = tmp2.tile([P, 512], BF16, tag="bcsb")
                    nc.vector.tensor_copy(bc_sb[po : po + 64], bcp[po : po + 64])
                    nc.vector.tensor_tensor(
                        ctxn[po : po + 64],
                        cps[po : po + 64],
                        bc_sb[po : po + 64],
                        OP.mult,
                    )
                    if par == 1:
                        # pair finished: residual add + LN1 stats (Pool
                        # takes the add + cast + square; PE the stats,
                        # deferred one slot so PE never waits on Pool).
                        # Last pair runs on DVE: Pool's 0.42 efficiency
                        # would sit on the attention->LN1 critical path.
                        eng = nc.vector if hp == H // 2 - 1 else nc.gpsimd
                        eng.tensor_add(
                            xres_s[:, hp], xres_s[:, hp], ctxn[:]
                        )
                        cast_t = tmp.tile([P, 512], BF16, tag="lncast")
                        sq_t = tmp.tile([P, 512], BF16, tag="lnsq")
                        eng.tensor_copy(cast_t[:], xres_s[:, hp])
                        eng.tensor_mul(
                            sq_t[:], xres_s[:, hp], xres_s[:, hp]
                        )
                        stats_pending.append((cast_t, sq_t, hp))

                ctxn = None
                # packed expt: per-j live query range [off_j, 512) stored
                # contiguously; POFF[j] is the packed start, NCOL[j] the width
                NCOL = [512 - max(0, j - 4) * P for j in range(NKT)]
                POFF = [0] * NKT
                for j in range(1, NKT):
                    POFF[j] = POFF[j - 1] + NCOL[j - 1]

                def emit_ctx(pr):
                    nonlocal ctxn
                    h, expt = pr
                    hp, par = h // 2, h % 2
                    cps = ps_ctx.tile([P, 512], F32, tag="ctx")
                    lsl = (0, 65) if par == 0 else (65, VW)
                    m = lsl[1] - lsl[0]
                    for j in range(NKT):
                        off = max(0, j - 4) * P
                        nc.tensor.matmul(
                            cps[0:m, off:512],
                            vaug[:, j, hp, lsl[0] : lsl[1]],
                            expt[:, POFF[j] : POFF[j] + NCOL[j]],
                            start=(j == 0),
                            stop=(j == NKT - 1),
                        )
                    with nc.allow_low_precision(
                        reason="softmax denominator reciprocal to bf16"
                    ):
                        if par == 0:
                            nc.vector.reciprocal(srowE_bf[64:65], cps[64:65])
                        else:
                            nc.vector.reciprocal(srowO_bf[0:1], cps[0:1])
                    if DEBUG_TAPS and h == DBG_HEAD:
                        dbg_cps = nc.dram_tensor(
                            "dbg_cps", [P, 512], F32, kind="ExternalOutput"
                        ).ap()
                        dbg_sb = persist.tile([P, 512], F32, tag="dbgsb")
                        nc.vector.memset(dbg_sb[:], 0.0)
                        _r0, _r1 = (0, 65) if par == 0 else (64, 128)
                        nc.vector.tensor_copy(dbg_sb[_r0:_r1], cps[_r0:_r1])
                        if par == 1:
                            nc.vector.tensor_copy(dbg_sb[0:1], cps[0:1])
                        nc.sync.dma_start(out=dbg_cps[:], in_=dbg_sb[:])
                        dbg_expt = nc.dram_tensor(
                            "dbg_expt", [P, 3328], BF16, kind="ExternalOutput"
                        ).ap()
                        nc.sync.dma_start(out=dbg_expt[:], in_=expt[:])
                        dbg_vaug = nc.dram_tensor(
                            "dbg_vaug", [P, NKT, VW], BF16, kind="ExternalOutput"
                        ).ap()
                        nc.sync.dma_start(out=dbg_vaug[:], in_=vaug[:, :, hp])
                    if par == 0:
                        ctxn = tmp2.tile([P, 512], F32, tag="ctxn")
                    return (h, cps, ctxn)

                def emit_scores(h):
                    # DoubleRow over dh: contraction (ki 32, parity 2); head
                    # h lives at partition group 32*(h%4) of ot pair
                    # (2*(h//4), 2*(h//4)+1). j-tiles are computed two per
                    # 2-bank psum so each Exp covers a pair in one shot.
                    u, sub = h // 4, h % 4
                    b0 = 32 * sub
                    expt = pattn.tile([P, 3328], BF16, tag="expt")
                    for pj in range(4):
                        j0 = 2 * pj
                        w0, w1 = NCOL[j0], NCOL[j0 + 1]
                        # two independent accumulation groups must not share
                        # a PSUM bank: place the second j at a 512 offset
                        po1 = max(w0, 512)
                        ps = ps_sc.tile([P, 1024], F32, tag="sc")
                        for j, w, po in ((j0, w0, 0), (j0 + 1, w1, po1)):
                            off = 512 - w
                            nc.tensor.matmul(
                                ps[:, po : po + w],
                                kt_s[b0 : b0 + 32, 2 * u : 2 * u + 2,
                                     j * P : (j + 1) * P],
                                qt_s[b0 : b0 + 32, 2 * u : 2 * u + 2, off:512],
                                start=True,
                                stop=True,
                                perf_mode=DR,
                                tile_position=(b0, 0),
                            )
                        if po1 == w0:
                            nc.scalar.activation(
                                expt[:, POFF[j0] : POFF[j0] + w0 + w1],
                                ps[:, 0 : w0 + w1],
                                AF.Exp,
                                scale=0.125 / (WSQK * WSQK),
                            )
                        else:
                            nc.scalar.activation(
                                expt[:, POFF[j0] : POFF[j0] + w0],
                                ps[:, 0:w0],
                                AF.Exp,
                                scale=0.125 / (WSQK * WSQK),
                            )
                            nc.scalar.activation(
                                expt[:, POFF[j0 + 1] : POFF[j0 + 1] + w1],
                                ps[:, po1 : po1 + w1],
                                AF.Exp,
                                scale=0.125 / (WSQK * WSQK),
                            )
                    for j in range(4, NKT):
                        # zero the masked upper triangle of the diagonal
                        # query block post-exp (Pool, off the hot engines)
                        nc.gpsimd.affine_select(
                            out=expt[:, POFF[j] : POFF[j] + P],
                            in_=expt[:, POFF[j] : POFF[j] + P],
                            compare_op=OP.is_ge,
                            fill=0.0,
                            base=0,
                            pattern=[[1, P]],
                            channel_multiplier=-1,
                        )
                    return (h, expt)

                # filler units: Q/K projections + V-proj tiles, ordered by
                # consumption deadline, drained during the head loop
                fillers = []
                for ot in (2, 3):
                    fillers += [
                        lambda o=ot: emit_qproj(o),
                        lambda o=ot: emit_kproj(o, 0),
                        lambda o=ot: emit_kproj(o, 1),
                    ]
                fillers += [lambda t=tk: emit_vproj(0, t) for tk in range(NKT)]
                for ot in (4, 5):
                    fillers += [
                        lambda o=ot: emit_qproj(o),
                        lambda o=ot: emit_kproj(o, 0),
                        lambda o=ot: emit_kproj(o, 1),
                    ]
                fillers += [lambda t=tk: emit_vproj(1, t) for tk in range(NKT)]
                for ot in (6, 7):
                    fillers += [
                        lambda o=ot: emit_qproj(o),
                        lambda o=ot: emit_kproj(o, 0),
                        lambda o=ot: emit_kproj(o, 1),
                    ]
                fillers.reverse()  # consume via pop()

                def drain(n):
                    for _ in range(n):
                        if fillers:
                            fillers.pop()()

                def need_qk(u):
                    # scores for head group u needs both parity tiles 2u, 2u+1
                    while not ({2 * u, 2 * u + 1} <= qk_done):
                        assert fillers, f"filler queue dry before qk pair {u}"
                        fillers.pop()()

                def need_v(db):
                    while v_done[db] < NKT:
                        assert fillers, f"filler queue dry before v {db}"
                        fillers.pop()()

                # warmup: head-group-0 Q/K, then 3 heads of scores while the
                # filler queue builds V/QK state; first ctx after V db0 done
                emit_qproj(0)
                emit_kproj(0, 0)
                emit_kproj(0, 1)
                emit_qproj(1)
                emit_kproj(1, 0)
                emit_kproj(1, 1)
                from collections import deque

                prevs = deque()
                prevs.append(emit_scores(0))
                drain(3)
                prevs.append(emit_scores(1))
                nc.sync.dma_start(out=xres_s[:], in_=xres[:])
                drain(3)
                prevs.append(emit_scores(2))
                drain(4)
                need_v(0)
                nc.sync.dma_start(out=b1r_s[:], in_=b1r[:])
                nc.sync.dma_start(out=b1e_s[:], in_=b1e[:])
                for nm, _src in (
                    ("b2t", b2t),
                    ("g1t", g1t),
                    ("be1t", be1t),
                    ("g2t", g2t),
                    ("be2t", be2t),
                ):
                    t = persist.tile([P, NKT], F32, tag=nm)
                    nc.sync.dma_start(out=t[:], in_=_src[:])
                    small[nm] = t
                # prefetch the first fc1/fc2 weight chunks during attention
                # so the FFN phases never wait on the serial SP DMA queue
                for _c in range(3):
                    load_w1(_c)
                load_w2(0)
                load_w2(1)
                pending = emit_ctx(prevs.popleft())
                for h in range(3, H):
                    need_qk(h // 4)
                    prevs.append(emit_scores(h))
                    drain(1)
                    emit_post(pending)
                    nh = prevs[0][0]
                    need_v(nh // 8)
                    pending = emit_ctx(prevs.popleft())
                drain(len(fillers))
                while prevs:
                    emit_post(pending)
                    need_v(1)
                    pending = emit_ctx(prevs.popleft())
                emit_post(pending)
                flush_stats()

                if DEBUG_TAPS:
                    dbg_xres = nc.dram_tensor(
                        "dbg_xres", [P, NKT, QTOK], F32, kind="ExternalOutput"
                    ).ap()
                    nc.sync.dma_start(out=dbg_xres[:], in_=xres_s[:])
                    dbg_xt = nc.dram_tensor(
                        "dbg_xt", [P, NKT, KV], F8, kind="ExternalOutput"
                    ).ap()
                    nc.sync.dma_start(out=dbg_xt[:], in_=xt_s[:])
                    dbg_kt = nc.dram_tensor(
                        "dbg_kt", [P, NKT, KV], F8, kind="ExternalOutput"
                    ).ap()
                    nc.sync.dma_start(out=dbg_kt[:], in_=kt_s[:])
                    dbg_stats = nc.dram_tensor(
                        "dbg_stats", [P, 2, 512], F32, kind="ExternalOutput"
                    ).ap()
                    nc.sync.dma_start(out=dbg_stats[:, 0], in_=sum1_sb[:])
                    nc.sync.dma_start(out=dbg_stats[:, 1], in_=sumsq1_sb[:])
                    dbg_srow = nc.dram_tensor(
                        "dbg_srow", [P, 2, QTOK], BF16, kind="ExternalOutput"
                    ).ap()
                    nc.sync.dma_start(out=dbg_srow[:, 0], in_=srowE_bf[:])
                    nc.sync.dma_start(out=dbg_srow[:, 1], in_=srowO_bf[:])

                # ---- phase 3: LN1 (stats already accumulated) ----
                # ln1_bf (bf16) is the fc2 residual; x8/x8e are the fp8
                # hi/lo pair feeding the compensated fc1 DoubleRow passes.
                ln1_bf = lnp.tile([P, NKT, QTOK], BF16, tag="ln1")
                x8 = lnp.tile([P, NKT, QTOK], F8, tag="x8")
                x8e = lnp.tile([P, NKT, QTOK], F8, tag="x8e")
                mean1, rstd1 = ln_meanvar(sum1_sb, sumsq1_sb)
                for kt in range(NKT):
                    ln_apply(
                        xres_s[:, kt], mean1, rstd1,
                        small["g1t"], small["be1t"], ln1_bf[:, kt], kt,
                    )
                    nc.gpsimd.tensor_copy(x8[:, kt], ln1_bf[:, kt])
                    nc.vector.tensor_tensor(
                        x8e[:, kt], ln1_bf[:, kt], x8[:, kt], OP.subtract
                    )
                pxstack.close()
                attn_stack.close()

            # ---- phase 4: fc1 + selu (w1 in JIT 4-ot chunks, depth 2) ----
            pffn_stack = contextlib.ExitStack()
            pffn = pffn_stack.enter_context(tc.tile_pool(name="pffn", bufs=1))
            ps_mm = pffn_stack.enter_context(
                tc.tile_pool(name="ps_mm", bufs=3, space="PSUM")
            )
            ps_x = pffn_stack.enter_context(
                tc.tile_pool(name="ps_x", bufs=1, space="PSUM")
            )
            # deep ring for the selu temporaries: with only 2 bufs the
            # Relu of ot must wait for Pool's h8 copy of ot-2 (slot reuse),
            # putting ~1us of Pool/Act latency on the PE critical path
            pselu = pffn_stack.enter_context(tc.tile_pool(name="pselu", bufs=4))
            h8 = pffn.tile([P, NOT1, QTOK], F8, tag="h8")
            h8e = pffn.tile([P, NOT1, QTOK], F8, tag="h8e")
            # ots 0..3 run kp-major across four live psums so each matmul
            # group consumes x8/x8e kt-pairs as LN1 streams them out --
            # otherwise the first psum group alone needs the full x8 tile
            # and the PE idles through the whole LN1 quant trench
            ps03 = []
            for ot in range(4):
                w1h, w1l = w1bufs[(ot // 2) % NW1B]
                if ot == 3:
                    psi = ps_x.tile([P, 512], F32, tag="x", name=f"ps03_{ot}")
                else:
                    psi = ps_mm.tile([P, 512], F32, tag="mm", name=f"ps03_{ot}")
                ps03.append((psi, w1h, w1l))
            load_w1(3)
            load_w1(4)
            for kp in range(NKT // 2):
                for ot in range(4):
                    psi, w1h, w1l = ps03[ot]
                    for pi, xq_w in enumerate(((w1h, x8), (w1l, x8), (w1h, x8e))):
                        wt, xq = xq_w
                        nc.tensor.matmul(
                            psi[:],
                            wt[:, ot % 2, 2 * kp : 2 * kp + 2],
                            xq[:, 2 * kp : 2 * kp + 2],
                            start=(kp == 0 and pi == 0),
                            stop=(kp == NKT // 2 - 1 and pi == 2),
                            perf_mode=DR,
                        )
            for ot in range(NOT1):
                if ot < 4:
                    ps = ps03[ot][0]
                else:
                    w1h, w1l = w1bufs[(ot // 2) % NW1B]
                    if ot % 2 == 0:
                        load_w1(ot // 2 + 3)
                    if ot % 4 == 3:
                        ps = ps_x.tile([P, 512], F32, tag="x")
                    else:
                        ps = ps_mm.tile([P, 512], F32, tag="mm")
                    passes = [(w1h, x8), (w1l, x8), (w1h, x8e)]
                    for pi, (wt, xq) in enumerate(passes):
                        for kp in range(NKT // 2):
                            nc.tensor.matmul(
                                ps[:],
                                wt[:, ot % 2, 2 * kp : 2 * kp + 2],
                                xq[:, 2 * kp : 2 * kp + 2],
                                start=(pi == 0 and kp == 0),
                                stop=(pi == 2 and kp == NKT // 2 - 1),
                                perf_mode=DR,
                            )
                p_t = pselu.tile([P, QTOK], F32, tag="selup")
                nc.scalar.activation(
                    p_t[:],
                    ps[:],
                    AF.Relu,
                    scale=SELU_S / WS,
                    bias=b1r_s[:, ot : ot + 1],
                )
                e_t = pselu.tile([P, QTOK], F32, tag="selue")
                nc.scalar.activation(
                    e_t[:], ps[:], AF.Exp, scale=1.0 / WS, bias=b1e_s[:, ot : ot + 1]
                )
                # selu(z) = min(sa*e^z - sa, s*relu(z))
                nc.vector.scalar_tensor_tensor(
                    p_t[:], e_t[:], SELU_SA, p_t[:], OP.subtract, OP.min
                )
                nc.gpsimd.tensor_copy(h8[:, ot], p_t[:])
                nc.vector.tensor_tensor(h8e[:, ot], p_t[:], h8[:, ot], OP.subtract)

            # ---- phase 5: fc2 + residual + LN2 + store (full 512 width) ----
            ps_stat2 = pffn_stack.enter_context(
                tc.tile_pool(name="ps_stat2", bufs=2, space="PSUM")
            )
            res2 = pffn.tile([P, NKT, QTOK], F32, tag="res2")
            ps0_2 = ps_stat2.tile([P, 512], F32, tag="stat2")
            ps1_2 = ps_stat2.tile([P, 512], F32, tag="stat2")
            for ot in range(NKT):
                w2h, w2l = w2bufs[ot % NW2B]
                load_w2(ot + 2)  # ots 0,1 preloaded in attention
                if ot % 4 == 3:
                    ps = ps_x.tile([P, 512], F32, tag="x")
                else:
                    ps = ps_mm.tile([P, 512], F32, tag="mm")
                passes = [(w2h, h8), (w2l, h8), (w2h, h8e)]
                for pi, (wt, hq) in enumerate(passes):
                    for kp in range(NOT1 // 2):
                        nc.tensor.matmul(
                            ps[:],
                            wt[:, 2 * kp : 2 * kp + 2],
                            hq[:, 2 * kp : 2 * kp + 2],
                            start=(pi == 0 and kp == 0),
                            stop=(pi == 2 and kp == NOT1 // 2 - 1),
                            perf_mode=DR,
                        )
                t1 = tmp2.tile([P, QTOK], F32, tag="r2t")
                nc.scalar.activation(
                    t1[:],
                    ps[:],
                    AF.Identity,
                    scale=1.0 / WS,
                    bias=small["b2t"][:, ot : ot + 1],
                )
                nc.vector.tensor_tensor(
                    res2[:, ot], t1[:], ln1_bf[:, ot], OP.add
                )
                cast_t = tmp.tile([P, QTOK], BF16, tag="lncast2")
                sq_t = tmp.tile([P, QTOK], BF16, tag="lnsq2")
                nc.vector.tensor_copy(cast_t[:], res2[:, ot])
                nc.scalar.activation(sq_t[:], res2[:, ot], AF.Square)
                ln_stats_mm(ps0_2, ps1_2, cast_t, sq_t, ot)
            mean2, rstd2 = ln_meanvar(ps0_2, ps1_2)
            for kt in range(NKT):
                # spread the tail normalize across DVE and Pool so the
                # final 8-tile chain isn't serialized on one engine; the
                # scale-bias always runs on the otherwise-idle Act engine
                eng = nc.gpsimd if kt in (2, 5) else nc.vector
                ln_apply(
                    res2[:, kt], mean2, rstd2,
                    small["g2t"], small["be2t"], res2[:, kt], kt,
                    eng=eng,
                )
                nc.sync.dma_start(out=out[:, kt], in_=res2[:, kt])
            pffn_stack.close()

    _legalize_waits(nc)
    return nc


_NC_CACHE = None
TRACE = False
LAST_EXEC_NS = None


def _get_nc():
    global _NC_CACHE
    if _NC_CACHE is None:
        _NC_CACHE = _build_nc()
    return _NC_CACHE


def _tile_w(a):
    """[Din, O] -> [P, O//P(ot), Din//P(kt), P] with ot-contiguous DMA slices."""
    Din, O = a.shape
    return np.ascontiguousarray(
        a.reshape(Din // P, P, O // P, P).transpose(1, 2, 0, 3)
    )


def _pp(v, n):
    """[n*P] -> [P, n] per-partition layout."""
    return np.ascontiguousarray(v.reshape(n, P).T)


def kernel(X, wq, wk, wv, ln1_g, ln1_b, w1, b1, w2, b2, ln2_g, ln2_b):
    from concourse.bass_utils import run_bass_kernel_spmd

    X = np.asarray(X, np.float32)
    bf = ml_dtypes.bfloat16
    f8 = ml_dtypes.float8_e4m3  # IEEE flavor — matches bass float8e4

    def hilo(wt):
        hi = wt.astype(f8)
        lo = (wt - hi.astype(np.float32)).astype(f8)
        return hi, lo

    # scores-DR out-dim permutation: slot (ot, i) holds projection row
    # head*64 + dh with head = 4*(ot//2) + i//32, dh = 2*(i%32) + ot%2,
    # so head h sits at partition group 32*(h%4) of tiles (2u, 2u+1)
    # with the dh parity split across the tile pair (DoubleRow Ko dim).
    qperm = np.empty(D, np.int64)
    for _ot in range(NKT):
        for _i in range(P):
            _h = 4 * (_ot // 2) + _i // 32
            _dh = 2 * (_i % 32) + (_ot % 2)
            qperm[_ot * P + _i] = _h * 64 + _dh
    wqT = _tile_w((WSQK * np.asarray(wq, np.float32).T)[:, qperm]).astype(f8)
    wkT = _tile_w((WSQK * np.asarray(wk, np.float32).T)[:, qperm]).astype(f8)
    wvT = np.ascontiguousarray(
        WS * np.asarray(wv, np.float32).T.reshape(NKT, P, D).transpose(1, 0, 2)
    ).astype(f8)
    w1hi, w1lo = hilo(_tile_w(WS * np.asarray(w1, np.float32).T))
    w2hi, w2lo = hilo(_tile_w(WS * np.asarray(w2, np.float32).T))
    b1 = np.asarray(b1, np.float32)
    shared = dict(
        wq=wqT,
        wk=wkT,
        wv=wvT,
        w1=w1hi,
        w1e=w1lo,
        w2=w2hi,
        w2e=w2lo,
        b1r=_pp(SELU_S * b1, NOT1),
        b1e=_pp(b1 + LN_SA, NOT1),
        b2t=_pp(np.asarray(b2, np.float32), NKT),
        g1t=_pp(np.asarray(ln1_g, np.float32), NKT),
        be1t=_pp(np.asarray(ln1_b, np.float32), NKT),
        g2t=_pp(np.asarray(ln2_g, np.float32), NKT),
        be2t=_pp(np.asarray(ln2_b, np.float32), NKT),
    )

    in_maps = []
    for c in range(8):
        b, hf = c // 2, c % 2
        if hf == 1:
            xkv = X[b].T  # [D, L]
            valid = np.full(KV, WS, np.float32)
            xq = X[b, 512:]
        else:
            xkv = np.concatenate(
                [np.zeros((D, 512), np.float32), X[b, :512].T], axis=1
            )
            valid = np.concatenate(
                [np.zeros(512, np.float32), np.full(512, WS, np.float32)]
            )
            xq = X[b, :512]
        xt = (
            np.ascontiguousarray(xkv.reshape(NKT, P, KV).transpose(1, 0, 2))
        ).astype(f8)
        xres = np.ascontiguousarray(xq.T.reshape(NKT, P, QTOK).transpose(1, 0, 2))
        vt = valid.reshape(NKT, P).T  # [P, NKT]
        val16 = (
            np.repeat(vt[:, :, None], H, axis=2).reshape(P, NKT, NKT, 2).astype(bf)
        )
        m = dict(shared)
        m.update(xt=xt, xres=xres, valid16=np.ascontiguousarray(val16))
        in_maps.append(m)

    nc = _get_nc()
    global LAST_EXEC_NS
    if TRACE:
        res = run_bass_kernel_spmd(nc, in_maps, list(range(8)), trace=True)
        LAST_EXEC_NS = res.exec_time_ns
    else:
        res = run_bass_kernel_spmd(nc, in_maps, list(range(8)))

    out = np.empty((B, L, D), np.float32)
    for c in range(8):
        b, hf = c // 2, c % 2
        o = res.results[c]["out"]  # [P, NKT, QTOK]
        o = o.transpose(1, 0, 2).reshape(D, QTOK).T  # [QTOK, D]
        out[b, hf * 512 : hf * 512 + 512] = o
    return out



# revision 82
# speedup vs baseline: 1.2121x; 1.0099x over previous
"""Decoder-layer Trainium2 kernel: 8-core SPMD, single launch, no collectives.

Sharding: core c -> (batch b = c // 2, sequence-half hf = c % 2). Each core
computes the full decoder layer for 512 query tokens of one sequence.
All cores run ONE identical program over a canonical virtual sequence of
1024 kv tokens with queries at virtual positions 512..1023; first-half cores
get their 512 real tokens placed at virtual 512..1023 with zero-padded kv
prefix and a `valid` vector that zeroes the pad contribution to the softmax
denominator.

v2 changes vs baseline:
- softmax denominators ride along in the ctx matmul via an augmented V
  (per head-pair V layout [Edims|Eden|Oden|zeros63|Odims], 193 wide): even
  heads matmul M=65 -> dims at psum rows 0..63 + den at row 64; odd heads
  M=128 with a zero block -> den at row 0 + dims at rows 64..127. Kills the
  65536 rows of separate [1,512] denominator matmuls.
- ctx matmuls are causally restricted to the live query range per kv tile
  (like scores), saving another 12288 rows.
- LN1 stats matmuls run inline as each head pair finishes its xres tile.
- fc2 + LN2 run in two token-half passes so the final normalize/store of
  half 0 overlaps the fc2 matmuls of half 1.
"""

import sys

sys.path.insert(0, "/opt/trn_rl_repo")

import math

import numpy as np
import ml_dtypes

import concourse.bass as bass
import concourse.mybir as mybir
from concourse.tile import TileContext, TilePool
from concourse.vector_clock import ScopedClock

BF16 = mybir.dt.bfloat16
F8 = mybir.dt.float8e4
F32 = mybir.dt.float32
AF = mybir.ActivationFunctionType
OP = mybir.AluOpType
DR = mybir.MatmulPerfMode.DoubleRow
WS = 64.0  # fp8 weight pre-scale (wv/w1/w2)
# Q/K projections use a smaller pre-scale: bass float8e4 is IEEE e4m3
# (max finite 240, saturates to inf) and |K|*64 reaches ~290 on some
# batches; *32 keeps the fp8 Q/K copies comfortably finite.
WSQK = 32.0

B, L, D = 4, 1024, 1024
H, DH = 16, 64
DFF = 4 * D
P = 128
QTOK = 512  # query tokens per core
KV = 1024  # canonical kv length (virtual)
NKT = D // P  # 8 d-tiles
NOT1 = DFF // P  # 32 fc1 out tiles
MASK_NEG = -1.0e9
VW = 193  # augmented V width per head pair: [Ed 64|Eden|Oden|z 63|Od 64]

SELU_S = 1.0507009873554804934193349852946
SELU_A = 1.6732632423543772848170429916717
SELU_SA = SELU_S * SELU_A
LN_SA = math.log(SELU_SA)
LN_EPS = 1e-5


class PatchedTileContext(TileContext):
    """TileContext whose exit drain respects this walrus build's limit of
    ONE semaphore wait per instruction: the global-clock waits are spread
    across standalone NOPs and the butterfly barrier (whose sem-eq waits
    walrus rejects) is replaced by the NRT-expanded pseudo barrier."""

    def _drain_and_barrier(self, tick_clock, wait_clock):
        nc = self.nc
        carrier = nc.sync.nop()
        wait_clock.add_sem_waits(
            carrier.ins, ScopedClock({None: tick_clock.global_clock})
        )
        waits = list(carrier.ins.sync_info.on_wait)
        ups = list(carrier.ins.sync_info.on_update)
        if len(waits) > 1:
            carrier.ins.sync_info = mybir.SyncInfo(on_wait=[waits[0]], on_update=ups)
            for w in waits[1:]:
                extra = nc.sync.nop()
                extra.ins.sync_info = mybir.SyncInfo(on_wait=[w], on_update=[])
        for eng in nc.engines.values():
            eng.drain()
        nc._nrt_pseudo_barrier()
        popped = nc._tile_sem_poison_stack.pop()
        assert popped is self._sem_poison
        nc.clear_and_free_semaphores(list(self.sems.allocated().values()))
        nc._nrt_pseudo_barrier()


def _legalize_waits(nc):
    """This walrus build accepts at most ONE semaphore wait per instruction.
    Tile's sem-assignment can attach several; hoist the extras onto same-engine
    NOPs inserted immediately before the instruction (waits are a conjunction,
    so a sequence of single-wait stalls is equivalent)."""
    n = 0
    for fn in nc.m.functions:
        for blk in fn.blocks:
            out = []
            changed = False
            for inst in blk.instructions:
                si = getattr(inst, "sync_info", None)
                if si is not None and len(si.on_wait) > 1:
                    waits = list(si.on_wait)
                    for w in waits[:-1]:
                        nop = mybir.InstNoOp(name=f"waitnop_{n}", ins=[], outs=[])
                        n += 1
                        nop.engine = inst.engine
                        nop.sync_info = mybir.SyncInfo(on_wait=[w], on_update=[])
                        out.append(nop)
                    inst.sync_info = mybir.SyncInfo(
                        on_wait=[waits[-1]], on_update=list(si.on_update)
                    )
                    changed = True
                out.append(inst)
            if changed:
                blk.instructions = out
    return n


DEBUG_TAPS = False
DBG_HEAD = 0


def _build_nc():
    nc = bass.Bass("TRN2", target_bir_lowering=False, debug=False, num_devices=8)

    def din(name, shape, dt):
        return nc.dram_tensor(name, shape, dt, kind="ExternalInput").ap()

    xt = din("xt", [P, NKT, KV], F8)  # X[b].T tiled, virtual-padded
    xres = din("xres", [P, NKT, QTOK], F32)  # q tokens transposed, fp32
    valid16 = din("valid16", [P, NKT, NKT, 2], BF16)  # WS flag, [8hp x 2]
    wq = din("wq", [P, NKT, NKT, P], F8)  # [dpart, ot, kt, o], x WS
    wk = din("wk", [P, NKT, NKT, P], F8)
    wv = din("wv", [P, NKT, D], F8)  # rhs layout [dpart, kt, o], x WS
    w1 = din("w1", [P, NOT1, NKT, P], F8)  # fp8(WS*w1^T)
    w1e = din("w1e", [P, NOT1, NKT, P], F8)  # fp8 residual of the above
    w2 = din("w2", [P, NKT, NOT1, P], F8)
    w2e = din("w2e", [P, NKT, NOT1, P], F8)
    b1r = din("b1r", [P, NOT1], F32)  # SELU_S * b1
    b1e = din("b1e", [P, NOT1], F32)  # b1 + ln(SELU_S*SELU_A)
    b2t = din("b2t", [P, NKT], F32)
    g1t = din("g1t", [P, NKT], F32)
    be1t = din("be1t", [P, NKT], F32)
    g2t = din("g2t", [P, NKT], F32)
    be2t = din("be2t", [P, NKT], F32)
    out = nc.dram_tensor("out", [P, NKT, QTOK], F32, kind="ExternalOutput").ap()

    with PatchedTileContext(nc) as tc:
        import contextlib

        with contextlib.ExitStack() as ctx:
            persist = ctx.enter_context(tc.tile_pool(name="persist", bufs=1))
            bc = ctx.enter_context(tc.tile_pool(name="bc", bufs=1))
            wpool = ctx.enter_context(tc.tile_pool(name="wpool", bufs=4))
            tmp = ctx.enter_context(tc.tile_pool(name="tmp", bufs=2))
            tmp2 = ctx.enter_context(tc.tile_pool(name="tmp2", bufs=2))
            lnp = ctx.enter_context(tc.tile_pool(name="lnp", bufs=1))
            w1pool = ctx.enter_context(tc.tile_pool(name="w1pool", bufs=1))

            # ---- constants ----
            NW1B, NW2B = 5, 4
            w1bufs = [
                (
                    w1pool.tile(
                        [P, 2, NKT, P], F8, tag=f"w1{i}h", name=f"w1{i}h"
                    ),
                    w1pool.tile(
                        [P, 2, NKT, P], F8, tag=f"w1{i}e", name=f"w1{i}e"
                    ),
                )
                for i in range(NW1B)
            ]
            w2bufs = [
                (
                    w1pool.tile(
                        [P, NOT1, P], F8, tag=f"w2{i}h", name=f"w2{i}h"
                    ),
                    w1pool.tile(
                        [P, NOT1, P], F8, tag=f"w2{i}e", name=f"w2{i}e"
                    ),
                )
                for i in range(NW2B)
            ]

            def load_w1(chunk):
                if chunk < NOT1 // 2:
                    hb, lb = w1bufs[chunk % NW1B]
                    nc.sync.dma_start(out=hb[:], in_=w1[:, 2 * chunk : 2 * chunk + 2])
                    nc.sync.dma_start(out=lb[:], in_=w1e[:, 2 * chunk : 2 * chunk + 2])

            def load_w2(ot):
                if ot < NKT:
                    hb, lb = w2bufs[ot % NW2B]
                    nc.sync.dma_start(out=hb[:], in_=w2[:, ot])
                    nc.sync.dma_start(out=lb[:], in_=w2e[:, ot])
            ones128 = persist.tile([P, P], BF16, tag="ones128")
            nc.gpsimd.memset(ones128[:], 1.0)
            ones_r0 = persist.tile([P, P], BF16, tag="ones_r0")
            nc.gpsimd.memset(ones_r0[:], 0.0)
            nc.gpsimd.memset(ones_r0[0:1, :], 1.0)
            ones_r64 = persist.tile([P, P], BF16, tag="ones_r64")
            nc.gpsimd.memset(ones_r64[:], 0.0)
            nc.gpsimd.memset(ones_r64[64:65, :], 1.0)
            srowE_bf = persist.tile([P, QTOK], BF16, tag="srowEbf")
            nc.vector.memset(srowE_bf[:], 0.0)
            srowO_bf = persist.tile([P, QTOK], BF16, tag="srowObf")
            nc.vector.memset(srowO_bf[:], 0.0)
            eps_ap = persist.tile([P, 1], F32, tag="eps")
            nc.gpsimd.memset(eps_ap[:], LN_EPS)

            def ln_stats_mm(ps0, ps1, cast_t, sq_t, kt, n=NKT, ncols=QTOK):
                nc.tensor.matmul(
                    ps0[:, 0:ncols],
                    ones128[:],
                    cast_t[:],
                    start=(kt == 0),
                    stop=(kt == n - 1),
                )
                nc.tensor.matmul(
                    ps1[:, 0:ncols],
                    ones128[:],
                    sq_t[:],
                    start=(kt == 0),
                    stop=(kt == n - 1),
                )

            def ln_meanvar(ps0, ps1, ncols=QTOK):
                """stats psums -> (mean, rstd) broadcast tiles.

                rstd = exp(-0.5*ln(var+eps)): Ln and Exp share an Act table
                (natural_log_exp_and_others) with Relu/Identity/Square, so
                this never forces the 1.3us act-table reload that Sqrt would.
                """
                if ps1 is None:
                    # ps0 is a [P, 2*ncols] psum holding [sum | sumsq]:
                    # scale both with one DVE op into an adjacent pair
                    mv = bc.tile([P, 2 * ncols], F32, tag="meanvar")
                    nc.vector.tensor_scalar_mul(mv[:], ps0[:, 0 : 2 * ncols], 1.0 / D)
                    mean_bc, var_bc = mv[:, 0:ncols], mv[:, ncols : 2 * ncols]
                else:
                    mean_t = bc.tile([P, ncols], F32, tag="mean", name="mean_t")
                    mean_bc = mean_t[:]
                    nc.vector.tensor_scalar_mul(mean_bc, ps0[:, 0:ncols], 1.0 / D)
                    var_t = bc.tile([P, ncols], F32, tag="var", name="var_t")
                    var_bc = var_t[:]
                    nc.vector.tensor_scalar_mul(var_bc, ps1[:, 0:ncols], 1.0 / D)
                m2 = tmp2.tile([P, ncols], F32, tag="lnt")
                nc.vector.tensor_tensor(m2[:], mean_bc, mean_bc, OP.mult)
                nc.vector.tensor_tensor(var_bc, var_bc, m2[:], OP.subtract)
                nc.scalar.activation(var_bc, var_bc, AF.Ln, bias=eps_ap[:])
                nc.scalar.activation(var_bc, var_bc, AF.Exp, scale=-0.5)
                return mean_bc, var_bc

            def ln_apply(
                src_kt, mean_bc, var_bc, g_ap, b_ap, dst_kt, kt, ncols=QTOK,
                eng=None, fin_act=None,
            ):
                eng = eng or nc.vector
                t1 = tmp2.tile([P, ncols], F32, tag="lnt")
                eng.tensor_tensor(t1[:], src_kt, mean_bc[:], OP.subtract)
                eng.tensor_tensor(t1[:], t1[:], var_bc[:], OP.mult)
                if fin_act if fin_act is not None else (kt % 2 == 0):
                    nc.scalar.activation(
                        dst_kt,
                        t1[:],
                        AF.Identity,
                        scale=g_ap[:, kt : kt + 1],
                        bias=b_ap[:, kt : kt + 1],
                    )
                else:
                    nc.vector.tensor_scalar(
                        dst_kt,
                        t1[:],
                        g_ap[:, kt : kt + 1],
                        b_ap[:, kt : kt + 1],
                        OP.mult,
                        OP.add,
                    )

            # ---- phase 1+2 fused: QKV projections + attention ----
            with tc.tile_pool(name="pproj", bufs=1) as pproj:
                import contextlib as _ctl

                attn_stack = _ctl.ExitStack()
                pattn = attn_stack.enter_context(tc.tile_pool(name="pattn", bufs=3))
                ps_ctx = attn_stack.enter_context(
                    tc.tile_pool(name="ps_ctx", bufs=2, space="PSUM")
                )
                ps_sc = attn_stack.enter_context(
                    tc.tile_pool(name="ps_sc", bufs=2, space="PSUM")
                )
                ps_fill = attn_stack.enter_context(
                    tc.tile_pool(name="ps_fill", bufs=2, space="PSUM")
                )
                pxstack = _ctl.ExitStack()
                px = pxstack.enter_context(tc.tile_pool(name="px", bufs=1))
                wq_t0 = wpool.tile([P, NKT, P], F8, tag="wqkv")
                nc.sync.dma_start(out=wq_t0[:], in_=wq[:, 0])
                xt_s = px.tile([P, NKT, KV], F8, tag="xt")
                nc.sync.dma_start(out=xt_s[:, 0:4], in_=xt[:, 0:4])
                nc.sync.dma_start(out=xt_s[:, 4:8], in_=xt[:, 4:8])
                wk_t0 = wpool.tile([P, NKT, P], F8, tag="wqkv")
                nc.sync.dma_start(out=wk_t0[:], in_=wk[:, 0])
                # Q/K in fp8, scores-DR grouped layout: tile ot = (u, c)
                # with u = ot//2 (head group 4u..4u+3), c = ot%2 (dh parity);
                # partition 32*g+ki holds head 4u+g, dh = 2*ki + c.
                qt_s = pproj.tile([P, NKT, QTOK], F8, tag="qt")
                kt_s = pproj.tile([P, NKT, KV], F8, tag="kt")
                # augmented V: per (kv-tile j, head pair hp) 193 cols:
                # [Edims 64 | Eden 1 | Oden 1 | zeros 63 | Odims 64]
                vaug = pproj.tile([P, NKT, NKT, VW], BF16, tag="vaug")
                nc.gpsimd.memset(vaug[:, :, :, 66:129], 0.0)

                wv_s = px.tile([P, NKT, D], F8, tag="wv")
                nc.sync.dma_start(out=wv_s[:], in_=wv[:])
                val_s = pproj.tile([P, NKT, NKT, 2], BF16, tag="val")
                nc.sync.dma_start(out=val_s[:], in_=valid16[:])
                xres_s = pproj.tile([P, NKT, QTOK], F32, tag="xres")
                b1r_s = persist.tile([P, NOT1], F32, tag="b1r")
                b1e_s = persist.tile([P, NOT1], F32, tag="b1e")
                small = {}
                sum1_sb = pproj.tile([P, 512], F32, tag="sum1")
                sumsq1_sb = pproj.tile([P, 512], F32, tag="sumsq1")

                qk_done = set()
                v_done = {0: 0, 1: 0}

                def emit_qproj(ot):
                    if ot == 0:
                        wq_t = wq_t0
                    else:
                        wq_t = wpool.tile([P, NKT, P], F8, tag="wqkv")
                        nc.sync.dma_start(out=wq_t[:], in_=wq[:, ot])
                    ps = ps_fill.tile([P, 512], F32, tag="fill")
                    for kp in range(NKT // 2):
                        nc.tensor.matmul(
                            ps[:],
                            wq_t[:, 2 * kp : 2 * kp + 2],
                            xt_s[:, 2 * kp : 2 * kp + 2, 512:1024],
                            start=(kp == 0),
                            stop=(kp == NKT // 2 - 1),
                            perf_mode=DR,
                        )
                    nc.vector.tensor_copy(qt_s[:, ot], ps[:])

                wk_ts = {0: wk_t0}

                def emit_kproj(ot, tb):
                    if tb == 0 and ot not in wk_ts:
                        wk_t = wpool.tile([P, NKT, P], F8, tag="wqkv")
                        nc.sync.dma_start(out=wk_t[:], in_=wk[:, ot])
                        wk_ts[ot] = wk_t
                    wk_t = wk_ts[ot]
                    ps = ps_fill.tile([P, 512], F32, tag="fill")
                    for kp in range(NKT // 2):
                        nc.tensor.matmul(
                            ps[:],
                            wk_t[:, 2 * kp : 2 * kp + 2],
                            xt_s[:, 2 * kp : 2 * kp + 2, tb * 512 : (tb + 1) * 512],
                            start=(kp == 0),
                            stop=(kp == NKT // 2 - 1),
                            perf_mode=DR,
                        )
                    if tb == 0:
                        nc.scalar.copy(kt_s[:, ot, 0:512], ps[:])
                    else:
                        nc.vector.tensor_copy(kt_s[:, ot, 512:1024], ps[:])
                        qk_done.add(ot)

                def emit_vproj(db, tk):
                    if db == 0:
                        # den columns for all 8 pairs x 2 parities
                        nc.vector.tensor_copy(vaug[:, tk, :, 64:66], val_s[:, tk])
                    ps = ps_fill.tile([P, 4, P], F32, tag="fill")
                    for kp in range(NKT // 2):
                        nc.tensor.matmul(
                            ps[:, :, :],
                            xt_s[:, 2 * kp : 2 * kp + 2, tk * P : (tk + 1) * P],
                            wv_s[:, 2 * kp : 2 * kp + 2, db * 512 : (db + 1) * 512],
                            start=(kp == 0),
                            stop=(kp == NKT // 2 - 1),
                            perf_mode=DR,
                        )
                    hp0 = db * 4
                    nc.vector.tensor_copy(
                        vaug[:, tk, hp0 : hp0 + 4, 0:64], ps[:, :, 0:64]
                    )
                    nc.vector.tensor_copy(
                        vaug[:, tk, hp0 : hp0 + 4, 129:193], ps[:, :, 64:128]
                    )
                    v_done[db] += 1

                stats_pending = []

                def flush_stats():
                    while stats_pending:
                        cast_t, sq_t, hp = stats_pending.pop(0)
                        pss = ps_sc.tile([P, 1024], F32, tag="sc")
                        nc.tensor.matmul(
                            pss[:, 0:512], ones128[:], cast_t[:], start=True, stop=True
                        )
                        nc.tensor.matmul(
                            pss[:, 512:1024], ones128[:], sq_t[:], start=True, stop=True
                        )
                        if hp == 0:
                            nc.vector.tensor_copy(sum1_sb[:], pss[:, 0:512])
                            nc.vector.tensor_copy(sumsq1_sb[:], pss[:, 512:1024])
                        else:
                            nc.vector.tensor_tensor(
                                sum1_sb[:], sum1_sb[:], pss[:, 0:512], OP.add
                            )
                            nc.vector.tensor_tensor(
                                sumsq1_sb[:], sumsq1_sb[:], pss[:, 512:1024], OP.add
                            )

                def emit_post(p):
                    h, cps, ctxn = p
                    hp, par = h // 2, h % 2
                    po = 64 * par
                    flush_stats()
                    # broadcast the bf16 reciprocal row across the 64 ctx
                    # partitions via a ones-matmul (bcp shares the fill pool)
                    bcp = ps_fill.tile([P, 512], F32, tag="fill")
                    if par == 0:
                        nc.tensor.matmul(
                            bcp[0:64],
                            ones_r64[:, 0:64],
                            srowE_bf[:],
                            start=True,
                            stop=True,
                        )
                    else:
                        nc.tensor.matmul(
                            bcp[64:128],
                            ones_r0[:, 0:64],
                            srowO_bf[:],
                            start=True,
                            stop=True,
                        )
                    bc_sb = tmp2.tile([P, 512], BF16, tag="bcsb")
                    nc.vector.tensor_copy(bc_sb[po : po + 64], bcp[po : po + 64])
                    nc.vector.tensor_tensor(
                        ctxn[po : po + 64],
                        cps[po : po + 64],
                        bc_sb[po : po + 64],
                        OP.mult,
                    )
                    if par == 1:
                        # pair finished: residual add + LN1 stats (Pool
                        # takes the add + cast + square; PE the stats,
                        # deferred one slot so PE never waits on Pool).
                        # Last pair runs on DVE: Pool's 0.42 efficiency
                        # would sit on the attention->LN1 critical path.
                        eng = nc.vector if hp == H // 2 - 1 else nc.gpsimd
                        eng.tensor_add(
                            xres_s[:, hp], xres_s[:, hp], ctxn[:]
                        )
                        cast_t = tmp.tile([P, 512], BF16, tag="lncast")
                        sq_t = tmp.tile([P, 512], BF16, tag="lnsq")
                        eng.tensor_copy(cast_t[:], xres_s[:, hp])
                        eng.tensor_mul(
                            sq_t[:], xres_s[:, hp], xres_s[:, hp]
                        )
                        stats_pending.append((cast_t, sq_t, hp))

                ctxn = None
                # packed expt: per-j live query range [off_j, 512) stored
                # contiguously; POFF[j] is the packed start, NCOL[j] the width
                NCOL = [512 - max(0, j - 4) * P for j in range(NKT)]
                POFF = [0] * NKT
                for j in range(1, NKT):
                    POFF[j] = POFF[j - 1] + NCOL[j - 1]

                def emit_ctx(pr):
                    nonlocal ctxn
                    h, expt = pr
                    hp, par = h // 2, h % 2
                    cps = ps_ctx.tile([P, 512], F32, tag="ctx")
                    lsl = (0, 65) if par == 0 else (65, VW)
                    m = lsl[1] - lsl[0]
                    for j in range(NKT):
                        off = max(0, j - 4) * P
                        nc.tensor.matmul(
                            cps[0:m, off:512],
                            vaug[:, j, hp, lsl[0] : lsl[1]],
                            expt[:, POFF[j] : POFF[j] + NCOL[j]],
                            start=(j == 0),
                            stop=(j == NKT - 1),
                        )
                    with nc.allow_low_precision(
                        reason="softmax denominator reciprocal to bf16"
                    ):
                        if par == 0:
                            nc.vector.reciprocal(srowE_bf[64:65], cps[64:65])
                        else:
                            nc.vector.reciprocal(srowO_bf[0:1], cps[0:1])
                    if DEBUG_TAPS and h == DBG_HEAD:
                        dbg_cps = nc.dram_tensor(
                            "dbg_cps", [P, 512], F32, kind="ExternalOutput"
                        ).ap()
                        dbg_sb = persist.tile([P, 512], F32, tag="dbgsb")
                        nc.vector.memset(dbg_sb[:], 0.0)
                        _r0, _r1 = (0, 65) if par == 0 else (64, 128)
                        nc.vector.tensor_copy(dbg_sb[_r0:_r1], cps[_r0:_r1])
                        if par == 1:
                            nc.vector.tensor_copy(dbg_sb[0:1], cps[0:1])
                        nc.sync.dma_start(out=dbg_cps[:], in_=dbg_sb[:])
                        dbg_expt = nc.dram_tensor(
                            "dbg_expt", [P, 3328], BF16, kind="ExternalOutput"
                        ).ap()
                        nc.sync.dma_start(out=dbg_expt[:], in_=expt[:])
                        dbg_vaug = nc.dram_tensor(
                            "dbg_vaug", [P, NKT, VW], BF16, kind="ExternalOutput"
                        ).ap()
                        nc.sync.dma_start(out=dbg_vaug[:], in_=vaug[:, :, hp])
                    if par == 0:
                        ctxn = tmp2.tile([P, 512], F32, tag="ctxn")
                    return (h, cps, ctxn)

                def emit_scores(h):
                    # DoubleRow over dh: contraction (ki 32, parity 2); head
                    # h lives at partition group 32*(h%4) of ot pair
                    # (2*(h//4), 2*(h//4)+1). j-tiles are computed two per
                    # 2-bank psum so each Exp covers a pair in one shot.
                    u, sub = h // 4, h % 4
                    b0 = 32 * sub
                    expt = pattn.tile([P, 3328], BF16, tag="expt")
                    for pj in range(4):
                        j0 = 2 * pj
                        w0, w1 = NCOL[j0], NCOL[j0 + 1]
                        # two independent accumulation groups must not share
                        # a PSUM bank: place the second j at a 512 offset
                        po1 = max(w0, 512)
                        ps = ps_sc.tile([P, 1024], F32, tag="sc")
                        for j, w, po in ((j0, w0, 0), (j0 + 1, w1, po1)):
                            off = 512 - w
                            nc.tensor.matmul(
                                ps[:, po : po + w],
                                kt_s[b0 : b0 + 32, 2 * u : 2 * u + 2,
                                     j * P : (j + 1) * P],
                                qt_s[b0 : b0 + 32, 2 * u : 2 * u + 2, off:512],
                                start=True,
                                stop=True,
                                perf_mode=DR,
                                tile_position=(b0, 0),
                            )
                        if po1 == w0:
                            nc.scalar.activation(
                                expt[:, POFF[j0] : POFF[j0] + w0 + w1],
                                ps[:, 0 : w0 + w1],
                                AF.Exp,
                                scale=0.125 / (WSQK * WSQK),
                            )
                        else:
                            nc.scalar.activation(
                                expt[:, POFF[j0] : POFF[j0] + w0],
                                ps[:, 0:w0],
                                AF.Exp,
                                scale=0.125 / (WSQK * WSQK),
                            )
                            nc.scalar.activation(
                                expt[:, POFF[j0 + 1] : POFF[j0 + 1] + w1],
                                ps[:, po1 : po1 + w1],
                                AF.Exp,
                                scale=0.125 / (WSQK * WSQK),
                            )
                    for j in range(4, NKT):
                        # zero the masked upper triangle of the diagonal
                        # query block post-exp (Pool, off the hot engines)
                        nc.gpsimd.affine_select(
                            out=expt[:, POFF[j] : POFF[j] + P],
                            in_=expt[:, POFF[j] : POFF[j] + P],
                            compare_op=OP.is_ge,
                            fill=0.0,
                            base=0,
                            pattern=[[1, P]],
                            channel_multiplier=-1,
                        )
                    return (h, expt)

                # filler units: Q/K projections + V-proj tiles, ordered by
                # consumption deadline, drained during the head loop
                fillers = []
                for ot in (2, 3):
                    fillers += [
                        lambda o=ot: emit_qproj(o),
                        lambda o=ot: emit_kproj(o, 0),
                        lambda o=ot: emit_kproj(o, 1),
                    ]
                fillers += [lambda t=tk: emit_vproj(0, t) for tk in range(NKT)]
                for ot in (4, 5):
                    fillers += [
                        lambda o=ot: emit_qproj(o),
                        lambda o=ot: emit_kproj(o, 0),
                        lambda o=ot: emit_kproj(o, 1),
                    ]
                fillers += [lambda t=tk: emit_vproj(1, t) for tk in range(NKT)]
                for ot in (6, 7):
                    fillers += [
                        lambda o=ot: emit_qproj(o),
                        lambda o=ot: emit_kproj(o, 0),
                        lambda o=ot: emit_kproj(o, 1),
                    ]
                fillers.reverse()  # consume via pop()

                def drain(n):
                    for _ in range(n):
                        if fillers:
                            fillers.pop()()

                def need_qk(u):
                    # scores for head group u needs both parity tiles 2u, 2u+1
                    while not ({2 * u, 2 * u + 1} <= qk_done):
                        assert fillers, f"filler queue dry before qk pair {u}"
                        fillers.pop()()

                def need_v(db):
                    while v_done[db] < NKT:
                        assert fillers, f"filler queue dry before v {db}"
                        fillers.pop()()

                # warmup: head-group-0 Q/K, then 3 heads of scores while the
                # filler queue builds V/QK state; first ctx after V db0 done
                emit_qproj(0)
                emit_kproj(0, 0)
                emit_kproj(0, 1)
                emit_qproj(1)
                emit_kproj(1, 0)
                emit_kproj(1, 1)
                from collections import deque

                prevs = deque()
                prevs.append(emit_scores(0))
                drain(3)
                prevs.append(emit_scores(1))
                nc.sync.dma_start(out=xres_s[:], in_=xres[:])
                drain(3)
                prevs.append(emit_scores(2))
                drain(4)
                need_v(0)
                nc.sync.dma_start(out=b1r_s[:], in_=b1r[:])
                nc.sync.dma_start(out=b1e_s[:], in_=b1e[:])
                for nm, _src in (
                    ("b2t", b2t),
                    ("g1t", g1t),
                    ("be1t", be1t),
                    ("g2t", g2t),
                    ("be2t", be2t),
                ):
                    t = persist.tile([P, NKT], F32, tag=nm)
                    nc.sync.dma_start(out=t[:], in_=_src[:])
                    small[nm] = t
                # prefetch the first fc1/fc2 weight chunks during attention
                # so the FFN phases never wait on the serial SP DMA queue
                for _c in range(3):
                    load_w1(_c)
                load_w2(0)
                load_w2(1)
                pending = emit_ctx(prevs.popleft())
                for h in range(3, H):
                    need_qk(h // 4)
                    prevs.append(emit_scores(h))
                    drain(1)
                    emit_post(pending)
                    nh = prevs[0][0]
                    need_v(nh // 8)
                    pending = emit_ctx(prevs.popleft())
                drain(len(fillers))
                while prevs:
                    emit_post(pending)
                    need_v(1)
                    pending = emit_ctx(prevs.popleft())
                emit_post(pending)
                flush_stats()

                if DEBUG_TAPS:
                    dbg_xres = nc.dram_tensor(
                        "dbg_xres", [P, NKT, QTOK], F32, kind="ExternalOutput"
                    ).ap()
                    nc.sync.dma_start(out=dbg_xres[:], in_=xres_s[:])
                    dbg_xt = nc.dram_tensor(
                        "dbg_xt", [P, NKT, KV], F8, kind="ExternalOutput"
                    ).ap()
                    nc.sync.dma_start(out=dbg_xt[:], in_=xt_s[:])
                    dbg_kt = nc.dram_tensor(
                        "dbg_kt", [P, NKT, KV], F8, kind="ExternalOutput"
                    ).ap()
                    nc.sync.dma_start(out=dbg_kt[:], in_=kt_s[:])
                    dbg_stats = nc.dram_tensor(
                        "dbg_stats", [P, 2, 512], F32, kind="ExternalOutput"
                    ).ap()
                    nc.sync.dma_start(out=dbg_stats[:, 0], in_=sum1_sb[:])
                    nc.sync.dma_start(out=dbg_stats[:, 1], in_=sumsq1_sb[:])
                    dbg_srow = nc.dram_tensor(
                        "dbg_srow", [P, 2, QTOK], BF16, kind="ExternalOutput"
                    ).ap()
                    nc.sync.dma_start(out=dbg_srow[:, 0], in_=srowE_bf[:])
                    nc.sync.dma_start(out=dbg_srow[:, 1], in_=srowO_bf[:])

                # ---- phase 3: LN1 (stats already accumulated) ----
                # ln1_bf (bf16) is the fc2 residual; x8/x8e are the fp8
                # hi/lo pair feeding the compensated fc1 DoubleRow passes.
                ln1_bf = lnp.tile([P, NKT, QTOK], BF16, tag="ln1")
                x8 = lnp.tile([P, NKT, QTOK], F8, tag="x8")
                x8e = lnp.tile([P, NKT, QTOK], F8, tag="x8e")
                mean1, rstd1 = ln_meanvar(sum1_sb, sumsq1_sb)
                for kt in range(NKT):
                    ln_apply(
                        xres_s[:, kt], mean1, rstd1,
                        small["g1t"], small["be1t"], ln1_bf[:, kt], kt,
                    )
                    nc.gpsimd.tensor_copy(x8[:, kt], ln1_bf[:, kt])
                    xeeng = nc.gpsimd if kt % 2 else nc.vector
                    xeeng.tensor_tensor(
                        x8e[:, kt], ln1_bf[:, kt], x8[:, kt], OP.subtract
                    )
                pxstack.close()
                attn_stack.close()

            # ---- phase 4: fc1 + selu (w1 in JIT 4-ot chunks, depth 2) ----
            pffn_stack = contextlib.ExitStack()
            pffn = pffn_stack.enter_context(tc.tile_pool(name="pffn", bufs=1))
            ps_mm = pffn_stack.enter_context(
                tc.tile_pool(name="ps_mm", bufs=3, space="PSUM")
            )
            ps_x = pffn_stack.enter_context(
                tc.tile_pool(name="ps_x", bufs=1, space="PSUM")
            )
            # deep ring for the selu temporaries: with only 2 bufs the
            # Relu of ot must wait for Pool's h8 copy of ot-2 (slot reuse),
            # putting ~1us of Pool/Act latency on the PE critical path
            pselu = pffn_stack.enter_context(tc.tile_pool(name="pselu", bufs=4))
            h8 = pffn.tile([P, NOT1, QTOK], F8, tag="h8")
            h8e = pffn.tile([P, NOT1, QTOK], F8, tag="h8e")
            # ots 0..3 run kp-major across four live psums so each matmul
            # group consumes x8/x8e kt-pairs as LN1 streams them out --
            # otherwise the first psum group alone needs the full x8 tile
            # and the PE idles through the whole LN1 quant trench
            ps03 = []
            for ot in range(4):
                w1h, w1l = w1bufs[(ot // 2) % NW1B]
                if ot == 3:
                    psi = ps_x.tile([P, 512], F32, tag="x", name=f"ps03_{ot}")
                else:
                    psi = ps_mm.tile([P, 512], F32, tag="mm", name=f"ps03_{ot}")
                ps03.append((psi, w1h, w1l))
            load_w1(3)
            load_w1(4)
            for kp in range(NKT // 2):
                for ot in range(4):
                    psi, w1h, w1l = ps03[ot]
                    for pi, xq_w in enumerate(((w1h, x8), (w1l, x8), (w1h, x8e))):
                        wt, xq = xq_w
                        nc.tensor.matmul(
                            psi[:],
                            wt[:, ot % 2, 2 * kp : 2 * kp + 2],
                            xq[:, 2 * kp : 2 * kp + 2],
                            start=(kp == 0 and pi == 0),
                            stop=(kp == NKT // 2 - 1 and pi == 2),
                            perf_mode=DR,
                        )
            for ot in range(NOT1):
                if ot < 4:
                    ps = ps03[ot][0]
                else:
                    w1h, w1l = w1bufs[(ot // 2) % NW1B]
                    if ot % 2 == 0:
                        load_w1(ot // 2 + 3)
                    if ot % 4 == 3:
                        ps = ps_x.tile([P, 512], F32, tag="x")
                    else:
                        ps = ps_mm.tile([P, 512], F32, tag="mm")
                    passes = [(w1h, x8), (w1l, x8), (w1h, x8e)]
                    for pi, (wt, xq) in enumerate(passes):
                        for kp in range(NKT // 2):
                            nc.tensor.matmul(
                                ps[:],
                                wt[:, ot % 2, 2 * kp : 2 * kp + 2],
                                xq[:, 2 * kp : 2 * kp + 2],
                                start=(pi == 0 and kp == 0),
                                stop=(pi == 2 and kp == NKT // 2 - 1),
                                perf_mode=DR,
                            )
                p_t = pselu.tile([P, QTOK], F32, tag="selup")
                nc.scalar.activation(
                    p_t[:],
                    ps[:],
                    AF.Relu,
                    scale=SELU_S / WS,
                    bias=b1r_s[:, ot : ot + 1],
                )
                e_t = pselu.tile([P, QTOK], F32, tag="selue")
                nc.scalar.activation(
                    e_t[:], ps[:], AF.Exp, scale=1.0 / WS, bias=b1e_s[:, ot : ot + 1]
                )
                # selu(z) = min(sa*e^z - sa, s*relu(z))
                nc.vector.scalar_tensor_tensor(
                    p_t[:], e_t[:], SELU_SA, p_t[:], OP.subtract, OP.min
                )
                nc.gpsimd.tensor_copy(h8[:, ot], p_t[:])
                nc.vector.tensor_tensor(h8e[:, ot], p_t[:], h8[:, ot], OP.subtract)

            # ---- phase 5: fc2 + residual + LN2 + store (full 512 width) ----
            ps_stat2 = pffn_stack.enter_context(
                tc.tile_pool(name="ps_stat2", bufs=2, space="PSUM")
            )
            res2 = pffn.tile([P, NKT, QTOK], F32, tag="res2")
            ps0_2 = ps_stat2.tile([P, 512], F32, tag="stat2")
            ps1_2 = ps_stat2.tile([P, 512], F32, tag="stat2")
            for ot in range(NKT):
                w2h, w2l = w2bufs[ot % NW2B]
                load_w2(ot + 2)  # ots 0,1 preloaded in attention
                if ot % 4 == 3:
                    ps = ps_x.tile([P, 512], F32, tag="x")
                else:
                    ps = ps_mm.tile([P, 512], F32, tag="mm")
                passes = [(w2h, h8), (w2l, h8), (w2h, h8e)]
                for pi, (wt, hq) in enumerate(passes):
                    for kp in range(NOT1 // 2):
                        nc.tensor.matmul(
                            ps[:],
                            wt[:, 2 * kp : 2 * kp + 2],
                            hq[:, 2 * kp : 2 * kp + 2],
                            start=(pi == 0 and kp == 0),
                            stop=(pi == 2 and kp == NOT1 // 2 - 1),
                            perf_mode=DR,
                        )
                t1 = tmp2.tile([P, QTOK], F32, tag="r2t")
                nc.scalar.activation(
                    t1[:],
                    ps[:],
                    AF.Identity,
                    scale=1.0 / WS,
                    bias=small["b2t"][:, ot : ot + 1],
                )
                nc.vector.tensor_tensor(
                    res2[:, ot], t1[:], ln1_bf[:, ot], OP.add
                )
                cast_t = tmp.tile([P, QTOK], BF16, tag="lncast2")
                sq_t = tmp.tile([P, QTOK], BF16, tag="lnsq2")
                nc.vector.tensor_copy(cast_t[:], res2[:, ot])
                nc.scalar.activation(sq_t[:], res2[:, ot], AF.Square)
                ln_stats_mm(ps0_2, ps1_2, cast_t, sq_t, ot)
            mean2, rstd2 = ln_meanvar(ps0_2, ps1_2)
            for kt in range(NKT):
                # spread the tail normalize across DVE and Pool so the
                # final 8-tile chain isn't serialized on one engine; the
                # scale-bias always runs on the otherwise-idle Act engine
                eng = nc.gpsimd if kt in (2, 5) else nc.vector
                ln_apply(
                    res2[:, kt], mean2, rstd2,
                    small["g2t"], small["be2t"], res2[:, kt], kt,
                    eng=eng,
                )
                nc.sync.dma_start(out=out[:, kt], in_=res2[:, kt])
            pffn_stack.close()

    _legalize_waits(nc)
    return nc


_NC_CACHE = None
TRACE = False
LAST_EXEC_NS = None


def _get_nc():
    global _NC_CACHE
    if _NC_CACHE is None:
        _NC_CACHE = _build_nc()
    return _NC_CACHE


def _tile_w(a):
    """[Din, O] -> [P, O//P(ot), Din//P(kt), P] with ot-contiguous DMA slices."""
    Din, O = a.shape
    return np.ascontiguousarray(
        a.reshape(Din // P, P, O // P, P).transpose(1, 2, 0, 3)
    )


def _pp(v, n):
    """[n*P] -> [P, n] per-partition layout."""
    return np.ascontiguousarray(v.reshape(n, P).T)


def kernel(X, wq, wk, wv, ln1_g, ln1_b, w1, b1, w2, b2, ln2_g, ln2_b):
    from concourse.bass_utils import run_bass_kernel_spmd

    X = np.asarray(X, np.float32)
    bf = ml_dtypes.bfloat16
    f8 = ml_dtypes.float8_e4m3  # IEEE flavor — matches bass float8e4

    def hilo(wt):
        hi = wt.astype(f8)
        lo = (wt - hi.astype(np.float32)).astype(f8)
        return hi, lo

    # scores-DR out-dim permutation: slot (ot, i) holds projection row
    # head*64 + dh with head = 4*(ot//2) + i//32, dh = 2*(i%32) + ot%2,
    # so head h sits at partition group 32*(h%4) of tiles (2u, 2u+1)
    # with the dh parity split across the tile pair (DoubleRow Ko dim).
    qperm = np.empty(D, np.int64)
    for _ot in range(NKT):
        for _i in range(P):
            _h = 4 * (_ot // 2) + _i // 32
            _dh = 2 * (_i % 32) + (_ot % 2)
            qperm[_ot * P + _i] = _h * 64 + _dh
    wqT = _tile_w((WSQK * np.asarray(wq, np.float32).T)[:, qperm]).astype(f8)
    wkT = _tile_w((WSQK * np.asarray(wk, np.float32).T)[:, qperm]).astype(f8)
    wvT = np.ascontiguousarray(
        WS * np.asarray(wv, np.float32).T.reshape(NKT, P, D).transpose(1, 0, 2)
    ).astype(f8)
    w1hi, w1lo = hilo(_tile_w(WS * np.asarray(w1, np.float32).T))
    w2hi, w2lo = hilo(_tile_w(WS * np.asarray(w2, np.float32).T))
    b1 = np.asarray(b1, np.float32)
    shared = dict(
        wq=wqT,
        wk=wkT,
        wv=wvT,
        w1=w1hi,
        w1e=w1lo,
        w2=w2hi,
        w2e=w2lo,
        b1r=_pp(SELU_S * b1, NOT1),
        b1e=_pp(b1 + LN_SA, NOT1),
        b2t=_pp(np.asarray(b2, np.float32), NKT),
        g1t=_pp(np.asarray(ln1_g, np.float32), NKT),
        be1t=_pp(np.asarray(ln1_b, np.float32), NKT),
        g2t=_pp(np.asarray(ln2_g, np.float32), NKT),
        be2t=_pp(np.asarray(ln2_b, np.float32), NKT),
    )

    in_maps = []
    for c in range(8):
        b, hf = c // 2, c % 2
        if hf == 1:
            xkv = X[b].T  # [D, L]
            valid = np.full(KV, WS, np.float32)
            xq = X[b, 512:]
        else:
            xkv = np.concatenate(
                [np.zeros((D, 512), np.float32), X[b, :512].T], axis=1
            )
            valid = np.concatenate(
                [np.zeros(512, np.float32), np.full(512, WS, np.float32)]
            )
            xq = X[b, :512]
        xt = (
            np.ascontiguousarray(xkv.reshape(NKT, P, KV).transpose(1, 0, 2))
        ).astype(f8)
        xres = np.ascontiguousarray(xq.T.reshape(NKT, P, QTOK).transpose(1, 0, 2))
        vt = valid.reshape(NKT, P).T  # [P, NKT]
        val16 = (
            np.repeat(vt[:, :, None], H, axis=2).reshape(P, NKT, NKT, 2).astype(bf)
        )
        m = dict(shared)
        m.update(xt=xt, xres=xres, valid16=np.ascontiguousarray(val16))
        in_maps.append(m)

    nc = _get_nc()
    global LAST_EXEC_NS
    if TRACE:
        res = run_bass_kernel_spmd(nc, in_maps, list(range(8)), trace=True)
        LAST_EXEC_NS = res.exec_time_ns
    else:
        res = run_bass_kernel_spmd(nc, in_maps, list(range(8)))

    out = np.empty((B, L, D), np.float32)
    for c in range(8):
        b, hf = c // 2, c % 2
        o = res.results[c]["out"]  # [P, NKT, QTOK]
        o = o.transpose(1, 0, 2).reshape(D, QTOK).T  # [QTOK, D]
        out[b, hf * 512 : hf * 512 + 512] = o
    return out



# revision 93
# speedup vs baseline: 1.2129x; 1.0007x over previous
"""Decoder-layer Trainium2 kernel: 8-core SPMD, single launch, no collectives.

Sharding: core c -> (batch b = c // 2, sequence-half hf = c % 2). Each core
computes the full decoder layer for 512 query tokens of one sequence.
All cores run ONE identical program over a canonical virtual sequence of
1024 kv tokens with queries at virtual positions 512..1023; first-half cores
get their 512 real tokens placed at virtual 512..1023 with zero-padded kv
prefix and a `valid` vector that zeroes the pad contribution to the softmax
denominator.

v2 changes vs baseline:
- softmax denominators ride along in the ctx matmul via an augmented V
  (per head-pair V layout [Edims|Eden|Oden|zeros63|Odims], 193 wide): even
  heads matmul M=65 -> dims at psum rows 0..63 + den at row 64; odd heads
  M=128 with a zero block -> den at row 0 + dims at rows 64..127. Kills the
  65536 rows of separate [1,512] denominator matmuls.
- ctx matmuls are causally restricted to the live query range per kv tile
  (like scores), saving another 12288 rows.
- LN1 stats matmuls run inline as each head pair finishes its xres tile.
- fc2 + LN2 run in two token-half passes so the final normalize/store of
  half 0 overlaps the fc2 matmuls of half 1.
"""

import sys

sys.path.insert(0, "/opt/trn_rl_repo")

import math

import numpy as np
import ml_dtypes

import concourse.bass as bass
import concourse.mybir as mybir
from concourse.tile import TileContext, TilePool
from concourse.vector_clock import ScopedClock

BF16 = mybir.dt.bfloat16
F8 = mybir.dt.float8e4
F32 = mybir.dt.float32
AF = mybir.ActivationFunctionType
OP = mybir.AluOpType
DR = mybir.MatmulPerfMode.DoubleRow
WS = 64.0  # fp8 weight pre-scale (wv/w1/w2)
# Q/K projections use a smaller pre-scale: bass float8e4 is IEEE e4m3
# (max finite 240, saturates to inf) and |K|*64 reaches ~290 on some
# batches; *32 keeps the fp8 Q/K copies comfortably finite.
WSQK = 32.0

B, L, D = 4, 1024, 1024
H, DH = 16, 64
DFF = 4 * D
P = 128
QTOK = 512  # query tokens per core
KV = 1024  # canonical kv length (virtual)
NKT = D // P  # 8 d-tiles
NOT1 = DFF // P  # 32 fc1 out tiles
MASK_NEG = -1.0e9
VW = 193  # augmented V width per head pair: [Ed 64|Eden|Oden|z 63|Od 64]

SELU_S = 1.0507009873554804934193349852946
SELU_A = 1.6732632423543772848170429916717
SELU_SA = SELU_S * SELU_A
LN_SA = math.log(SELU_SA)
LN_EPS = 1e-5


class PatchedTileContext(TileContext):
    """TileContext whose exit drain respects this walrus build's limit of
    ONE semaphore wait per instruction: the global-clock waits are spread
    across standalone NOPs and the butterfly barrier (whose sem-eq waits
    walrus rejects) is replaced by the NRT-expanded pseudo barrier."""

    def _drain_and_barrier(self, tick_clock, wait_clock):
        nc = self.nc
        carrier = nc.sync.nop()
        wait_clock.add_sem_waits(
            carrier.ins, ScopedClock({None: tick_clock.global_clock})
        )
        waits = list(carrier.ins.sync_info.on_wait)
        ups = list(carrier.ins.sync_info.on_update)
        if len(waits) > 1:
            carrier.ins.sync_info = mybir.SyncInfo(on_wait=[waits[0]], on_update=ups)
            for w in waits[1:]:
                extra = nc.sync.nop()
                extra.ins.sync_info = mybir.SyncInfo(on_wait=[w], on_update=[])
        for eng in nc.engines.values():
            eng.drain()
        nc._nrt_pseudo_barrier()
        popped = nc._tile_sem_poison_stack.pop()
        assert popped is self._sem_poison
        nc.clear_and_free_semaphores(list(self.sems.allocated().values()))
        nc._nrt_pseudo_barrier()


def _legalize_waits(nc):
    """This walrus build accepts at most ONE semaphore wait per instruction.
    Tile's sem-assignment can attach several; hoist the extras onto same-engine
    NOPs inserted immediately before the instruction (waits are a conjunction,
    so a sequence of single-wait stalls is equivalent)."""
    n = 0
    for fn in nc.m.functions:
        for blk in fn.blocks:
            out = []
            changed = False
            for inst in blk.instructions:
                si = getattr(inst, "sync_info", None)
                if si is not None and len(si.on_wait) > 1:
                    waits = list(si.on_wait)
                    for w in waits[:-1]:
                        nop = mybir.InstNoOp(name=f"waitnop_{n}", ins=[], outs=[])
                        n += 1
                        nop.engine = inst.engine
                        nop.sync_info = mybir.SyncInfo(on_wait=[w], on_update=[])
                        out.append(nop)
                    inst.sync_info = mybir.SyncInfo(
                        on_wait=[waits[-1]], on_update=list(si.on_update)
                    )
                    changed = True
                out.append(inst)
            if changed:
                blk.instructions = out
    return n


DEBUG_TAPS = False
DBG_HEAD = 0


def _build_nc():
    nc = bass.Bass("TRN2", target_bir_lowering=False, debug=False, num_devices=8)

    def din(name, shape, dt):
        return nc.dram_tensor(name, shape, dt, kind="ExternalInput").ap()

    xt = din("xt", [P, NKT, KV], F8)  # X[b].T tiled, virtual-padded
    xres = din("xres", [P, NKT, QTOK], F32)  # q tokens transposed, fp32
    valid16 = din("valid16", [P, NKT, NKT, 2], BF16)  # WS flag, [8hp x 2]
    wq = din("wq", [P, NKT, NKT, P], F8)  # [dpart, ot, kt, o], x WS
    wk = din("wk", [P, NKT, NKT, P], F8)
    wv = din("wv", [P, NKT, D], F8)  # rhs layout [dpart, kt, o], x WS
    w1 = din("w1", [P, NOT1, NKT, P], F8)  # fp8(WS*w1^T)
    w1e = din("w1e", [P, NOT1, NKT, P], F8)  # fp8 residual of the above
    w2 = din("w2", [P, NKT, NOT1, P], F8)
    w2e = din("w2e", [P, NKT, NOT1, P], F8)
    b1r = din("b1r", [P, NOT1], F32)  # SELU_S * b1
    b1e = din("b1e", [P, NOT1], F32)  # b1 + ln(SELU_S*SELU_A)
    b2t = din("b2t", [P, NKT], F32)
    g1t = din("g1t", [P, NKT], F32)
    be1t = din("be1t", [P, NKT], F32)
    g2t = din("g2t", [P, NKT], F32)
    be2t = din("be2t", [P, NKT], F32)
    out = nc.dram_tensor("out", [P, NKT, QTOK], F32, kind="ExternalOutput").ap()

    with PatchedTileContext(nc) as tc:
        import contextlib

        with contextlib.ExitStack() as ctx:
            persist = ctx.enter_context(tc.tile_pool(name="persist", bufs=1))
            bc = ctx.enter_context(tc.tile_pool(name="bc", bufs=1))
            wpool = ctx.enter_context(tc.tile_pool(name="wpool", bufs=4))
            tmp = ctx.enter_context(tc.tile_pool(name="tmp", bufs=2))
            tmp2 = ctx.enter_context(tc.tile_pool(name="tmp2", bufs=2))
            lnp = ctx.enter_context(tc.tile_pool(name="lnp", bufs=1))
            w1pool = ctx.enter_context(tc.tile_pool(name="w1pool", bufs=1))

            # ---- constants ----
            NW1B, NW2B = 5, 4
            w1bufs = [
                (
                    w1pool.tile(
                        [P, 2, NKT, P], F8, tag=f"w1{i}h", name=f"w1{i}h"
                    ),
                    w1pool.tile(
                        [P, 2, NKT, P], F8, tag=f"w1{i}e", name=f"w1{i}e"
                    ),
                )
                for i in range(NW1B)
            ]
            w2bufs = [
                (
                    w1pool.tile(
                        [P, NOT1, P], F8, tag=f"w2{i}h", name=f"w2{i}h"
                    ),
                    w1pool.tile(
                        [P, NOT1, P], F8, tag=f"w2{i}e", name=f"w2{i}e"
                    ),
                )
                for i in range(NW2B)
            ]

            def load_w1(chunk):
                if chunk < NOT1 // 2:
                    hb, lb = w1bufs[chunk % NW1B]
                    nc.sync.dma_start(out=hb[:], in_=w1[:, 2 * chunk : 2 * chunk + 2])
                    nc.sync.dma_start(out=lb[:], in_=w1e[:, 2 * chunk : 2 * chunk + 2])

            def load_w2(ot):
                if ot < NKT:
                    hb, lb = w2bufs[ot % NW2B]
                    nc.sync.dma_start(out=hb[:], in_=w2[:, ot])
                    nc.sync.dma_start(out=lb[:], in_=w2e[:, ot])
            ones128 = persist.tile([P, P], BF16, tag="ones128")
            nc.gpsimd.memset(ones128[:], 1.0)
            ones_r0 = persist.tile([P, P], BF16, tag="ones_r0")
            nc.gpsimd.memset(ones_r0[:], 0.0)
            nc.gpsimd.memset(ones_r0[0:1, :], 1.0)
            ones_r64 = persist.tile([P, P], BF16, tag="ones_r64")
            nc.gpsimd.memset(ones_r64[:], 0.0)
            nc.gpsimd.memset(ones_r64[64:65, :], 1.0)
            srowE_bf = persist.tile([P, QTOK], BF16, tag="srowEbf")
            nc.vector.memset(srowE_bf[:], 0.0)
            srowO_bf = persist.tile([P, QTOK], BF16, tag="srowObf")
            nc.vector.memset(srowO_bf[:], 0.0)
            eps_ap = persist.tile([P, 1], F32, tag="eps")
            nc.gpsimd.memset(eps_ap[:], LN_EPS)

            def ln_stats_mm(ps0, ps1, cast_t, sq_t, kt, n=NKT, ncols=QTOK):
                nc.tensor.matmul(
                    ps0[:, 0:ncols],
                    ones128[:],
                    cast_t[:],
                    start=(kt == 0),
                    stop=(kt == n - 1),
                )
                nc.tensor.matmul(
                    ps1[:, 0:ncols],
                    ones128[:],
                    sq_t[:],
                    start=(kt == 0),
                    stop=(kt == n - 1),
                )

            def ln_meanvar(ps0, ps1, ncols=QTOK):
                """stats psums -> (mean, rstd) broadcast tiles.

                rstd = exp(-0.5*ln(var+eps)): Ln and Exp share an Act table
                (natural_log_exp_and_others) with Relu/Identity/Square, so
                this never forces the 1.3us act-table reload that Sqrt would.
                """
                if ps1 is None:
                    # ps0 is a [P, 2*ncols] psum holding [sum | sumsq]:
                    # scale both with one DVE op into an adjacent pair
                    mv = bc.tile([P, 2 * ncols], F32, tag="meanvar")
                    nc.vector.tensor_scalar_mul(mv[:], ps0[:, 0 : 2 * ncols], 1.0 / D)
                    mean_bc, var_bc = mv[:, 0:ncols], mv[:, ncols : 2 * ncols]
                else:
                    mean_t = bc.tile([P, ncols], F32, tag="mean", name="mean_t")
                    mean_bc = mean_t[:]
                    nc.vector.tensor_scalar_mul(mean_bc, ps0[:, 0:ncols], 1.0 / D)
                    var_t = bc.tile([P, ncols], F32, tag="var", name="var_t")
                    var_bc = var_t[:]
                    nc.vector.tensor_scalar_mul(var_bc, ps1[:, 0:ncols], 1.0 / D)
                m2 = tmp2.tile([P, ncols], F32, tag="lnt")
                nc.vector.tensor_tensor(m2[:], mean_bc, mean_bc, OP.mult)
                nc.vector.tensor_tensor(var_bc, var_bc, m2[:], OP.subtract)
                nc.scalar.activation(var_bc, var_bc, AF.Ln, bias=eps_ap[:])
                nc.scalar.activation(var_bc, var_bc, AF.Exp, scale=-0.5)
                return mean_bc, var_bc

            def ln_apply(
                src_kt, mean_bc, var_bc, g_ap, b_ap, dst_kt, kt, ncols=QTOK,
                eng=None, fin_act=None,
            ):
                eng = eng or nc.vector
                t1 = tmp2.tile([P, ncols], F32, tag="lnt")
                eng.tensor_tensor(t1[:], src_kt, mean_bc[:], OP.subtract)
                eng.tensor_tensor(t1[:], t1[:], var_bc[:], OP.mult)
                if fin_act if fin_act is not None else (kt % 2 == 0):
                    nc.scalar.activation(
                        dst_kt,
                        t1[:],
                        AF.Identity,
                        scale=g_ap[:, kt : kt + 1],
                        bias=b_ap[:, kt : kt + 1],
                    )
                else:
                    nc.vector.tensor_scalar(
                        dst_kt,
                        t1[:],
                        g_ap[:, kt : kt + 1],
                        b_ap[:, kt : kt + 1],
                        OP.mult,
                        OP.add,
                    )

            # ---- phase 1+2 fused: QKV projections + attention ----
            with tc.tile_pool(name="pproj", bufs=1) as pproj:
                import contextlib as _ctl

                attn_stack = _ctl.ExitStack()
                pattn = attn_stack.enter_context(tc.tile_pool(name="pattn", bufs=3))
                ps_ctx = attn_stack.enter_context(
                    tc.tile_pool(name="ps_ctx", bufs=2, space="PSUM")
                )
                ps_sc = attn_stack.enter_context(
                    tc.tile_pool(name="ps_sc", bufs=2, space="PSUM")
                )
                ps_fill = attn_stack.enter_context(
                    tc.tile_pool(name="ps_fill", bufs=2, space="PSUM")
                )
                pxstack = _ctl.ExitStack()
                px = pxstack.enter_context(tc.tile_pool(name="px", bufs=1))
                wq_t0 = wpool.tile([P, NKT, P], F8, tag="wqkv")
                nc.sync.dma_start(out=wq_t0[:], in_=wq[:, 0])
                xt_s = px.tile([P, NKT, KV], F8, tag="xt")
                nc.sync.dma_start(out=xt_s[:, 0:4], in_=xt[:, 0:4])
                nc.sync.dma_start(out=xt_s[:, 4:8], in_=xt[:, 4:8])
                wk_t0 = wpool.tile([P, NKT, P], F8, tag="wqkv")
                nc.sync.dma_start(out=wk_t0[:], in_=wk[:, 0])
                # Q/K in fp8, scores-DR grouped layout: tile ot = (u, c)
                # with u = ot//2 (head group 4u..4u+3), c = ot%2 (dh parity);
                # partition 32*g+ki holds head 4u+g, dh = 2*ki + c.
                qt_s = pproj.tile([P, NKT, QTOK], F8, tag="qt")
                kt_s = pproj.tile([P, NKT, KV], F8, tag="kt")
                # augmented V: per (kv-tile j, head pair hp) 193 cols:
                # [Edims 64 | Eden 1 | Oden 1 | zeros 63 | Odims 64]
                vaug = pproj.tile([P, NKT, NKT, VW], BF16, tag="vaug")
                nc.gpsimd.memset(vaug[:, :, :, 66:129], 0.0)

                wv_s = px.tile([P, NKT, D], F8, tag="wv")
                nc.sync.dma_start(out=wv_s[:], in_=wv[:])
                val_s = pproj.tile([P, NKT, NKT, 2], BF16, tag="val")
                nc.sync.dma_start(out=val_s[:], in_=valid16[:])
                xres_s = pproj.tile([P, NKT, QTOK], F32, tag="xres")
                b1r_s = persist.tile([P, NOT1], F32, tag="b1r")
                b1e_s = persist.tile([P, NOT1], F32, tag="b1e")
                small = {}
                sum1_sb = pproj.tile([P, 512], F32, tag="sum1")
                sumsq1_sb = pproj.tile([P, 512], F32, tag="sumsq1")

                qk_done = set()
                v_done = {0: 0, 1: 0}

                def emit_qproj(ot):
                    if ot == 0:
                        wq_t = wq_t0
                    else:
                        wq_t = wpool.tile([P, NKT, P], F8, tag="wqkv")
                        nc.sync.dma_start(out=wq_t[:], in_=wq[:, ot])
                    ps = ps_fill.tile([P, 512], F32, tag="fill")
                    for kp in range(NKT // 2):
                        nc.tensor.matmul(
                            ps[:],
                            wq_t[:, 2 * kp : 2 * kp + 2],
                            xt_s[:, 2 * kp : 2 * kp + 2, 512:1024],
                            start=(kp == 0),
                            stop=(kp == NKT // 2 - 1),
                            perf_mode=DR,
                        )
                    nc.vector.tensor_copy(qt_s[:, ot], ps[:])

                wk_ts = {0: wk_t0}

                def emit_kproj(ot, tb):
                    if tb == 0 and ot not in wk_ts:
                        wk_t = wpool.tile([P, NKT, P], F8, tag="wqkv")
                        nc.sync.dma_start(out=wk_t[:], in_=wk[:, ot])
                        wk_ts[ot] = wk_t
                    wk_t = wk_ts[ot]
                    ps = ps_fill.tile([P, 512], F32, tag="fill")
                    for kp in range(NKT // 2):
                        nc.tensor.matmul(
                            ps[:],
                            wk_t[:, 2 * kp : 2 * kp + 2],
                            xt_s[:, 2 * kp : 2 * kp + 2, tb * 512 : (tb + 1) * 512],
                            start=(kp == 0),
                            stop=(kp == NKT // 2 - 1),
                            perf_mode=DR,
                        )
                    if tb == 0:
                        nc.scalar.copy(kt_s[:, ot, 0:512], ps[:])
                    else:
                        nc.vector.tensor_copy(kt_s[:, ot, 512:1024], ps[:])
                        qk_done.add(ot)

                def emit_vproj(db, tk):
                    if db == 0:
                        # den columns for all 8 pairs x 2 parities
                        nc.vector.tensor_copy(vaug[:, tk, :, 64:66], val_s[:, tk])
                    ps = ps_fill.tile([P, 4, P], F32, tag="fill")
                    for kp in range(NKT // 2):
                        nc.tensor.matmul(
                            ps[:, :, :],
                            xt_s[:, 2 * kp : 2 * kp + 2, tk * P : (tk + 1) * P],
                            wv_s[:, 2 * kp : 2 * kp + 2, db * 512 : (db + 1) * 512],
                            start=(kp == 0),
                            stop=(kp == NKT // 2 - 1),
                            perf_mode=DR,
                        )
                    hp0 = db * 4
                    nc.vector.tensor_copy(
                        vaug[:, tk, hp0 : hp0 + 4, 0:64], ps[:, :, 0:64]
                    )
                    nc.vector.tensor_copy(
                        vaug[:, tk, hp0 : hp0 + 4, 129:193], ps[:, :, 64:128]
                    )
                    v_done[db] += 1

                stats_pending = []

                def flush_stats():
                    while stats_pending:
                        cast_t, sq_t, hp = stats_pending.pop(0)
                        pss = ps_sc.tile([P, 1024], F32, tag="sc")
                        nc.tensor.matmul(
                            pss[:, 0:512], ones128[:], cast_t[:], start=True, stop=True
                        )
                        nc.tensor.matmul(
                            pss[:, 512:1024], ones128[:], sq_t[:], start=True, stop=True
                        )
                        if hp == 0:
                            nc.vector.tensor_copy(sum1_sb[:], pss[:, 0:512])
                            nc.vector.tensor_copy(sumsq1_sb[:], pss[:, 512:1024])
                        else:
                            nc.vector.tensor_tensor(
                                sum1_sb[:], sum1_sb[:], pss[:, 0:512], OP.add
                            )
                            nc.vector.tensor_tensor(
                                sumsq1_sb[:], sumsq1_sb[:], pss[:, 512:1024], OP.add
                            )

                def emit_post(p):
                    h, cps, ctxn = p
                    hp, par = h // 2, h % 2
                    po = 64 * par
                    flush_stats()
                    # broadcast the bf16 reciprocal row across the 64 ctx
                    # partitions via a ones-matmul (bcp shares the fill pool)
                    bcp = ps_fill.tile([P, 512], F32, tag="fill")
                    if par == 0:
                        nc.tensor.matmul(
                            bcp[0:64],
                            ones_r64[:, 0:64],
                            srowE_bf[:],
                            start=True,
                            stop=True,
                        )
                    else:
                        nc.tensor.matmul(
                            bcp[64:128],
                            ones_r0[:, 0:64],
                            srowO_bf[:],
                            start=True,
                            stop=True,
                        )
                    bc_sb = tmp2.tile([P, 512], BF16, tag="bcsb")
                    nc.vector.tensor_copy(bc_sb[po : po + 64], bcp[po : po + 64])
                    nc.vector.tensor_tensor(
                        ctxn[po : po + 64],
                        cps[po : po + 64],
                        bc_sb[po : po + 64],
                        OP.mult,
                    )
                    if par == 1:
                        # pair finished: residual add + LN1 stats (Pool
                        # takes the add + cast + square; PE the stats,
                        # deferred one slot so PE never waits on Pool).
                        # Last pair runs on DVE: Pool's 0.42 efficiency
                        # would sit on the attention->LN1 critical path.
                        eng = nc.vector if hp == H // 2 - 1 else nc.gpsimd
                        eng.tensor_add(
                            xres_s[:, hp], xres_s[:, hp], ctxn[:]
                        )
                        cast_t = tmp.tile([P, 512], BF16, tag="lncast")
                        sq_t = tmp.tile([P, 512], BF16, tag="lnsq")
                        eng.tensor_copy(cast_t[:], xres_s[:, hp])
                        eng.tensor_mul(
                            sq_t[:], xres_s[:, hp], xres_s[:, hp]
                        )
                        stats_pending.append((cast_t, sq_t, hp))

                ctxn = None
                # packed expt: per-j live query range [off_j, 512) stored
                # contiguously; POFF[j] is the packed start, NCOL[j] the width
                NCOL = [512 - max(0, j - 4) * P for j in range(NKT)]
                POFF = [0] * NKT
                for j in range(1, NKT):
                    POFF[j] = POFF[j - 1] + NCOL[j - 1]

                def emit_ctx(pr):
                    nonlocal ctxn
                    h, expt = pr
                    hp, par = h // 2, h % 2
                    cps = ps_ctx.tile([P, 512], F32, tag="ctx")
                    lsl = (0, 65) if par == 0 else (65, VW)
                    m = lsl[1] - lsl[0]
                    for j in range(NKT):
                        off = max(0, j - 4) * P
                        nc.tensor.matmul(
                            cps[0:m, off:512],
                            vaug[:, j, hp, lsl[0] : lsl[1]],
                            expt[:, POFF[j] : POFF[j] + NCOL[j]],
                            start=(j == 0),
                            stop=(j == NKT - 1),
                        )
                    with nc.allow_low_precision(
                        reason="softmax denominator reciprocal to bf16"
                    ):
                        if par == 0:
                            nc.vector.reciprocal(srowE_bf[64:65], cps[64:65])
                        else:
                            nc.vector.reciprocal(srowO_bf[0:1], cps[0:1])
                    if DEBUG_TAPS and h == DBG_HEAD:
                        dbg_cps = nc.dram_tensor(
                            "dbg_cps", [P, 512], F32, kind="ExternalOutput"
                        ).ap()
                        dbg_sb = persist.tile([P, 512], F32, tag="dbgsb")
                        nc.vector.memset(dbg_sb[:], 0.0)
                        _r0, _r1 = (0, 65) if par == 0 else (64, 128)
                        nc.vector.tensor_copy(dbg_sb[_r0:_r1], cps[_r0:_r1])
                        if par == 1:
                            nc.vector.tensor_copy(dbg_sb[0:1], cps[0:1])
                        nc.sync.dma_start(out=dbg_cps[:], in_=dbg_sb[:])
                        dbg_expt = nc.dram_tensor(
                            "dbg_expt", [P, 3328], BF16, kind="ExternalOutput"
                        ).ap()
                        nc.sync.dma_start(out=dbg_expt[:], in_=expt[:])
                        dbg_vaug = nc.dram_tensor(
                            "dbg_vaug", [P, NKT, VW], BF16, kind="ExternalOutput"
                        ).ap()
                        nc.sync.dma_start(out=dbg_vaug[:], in_=vaug[:, :, hp])
                    if par == 0:
                        ctxn = tmp2.tile([P, 512], F32, tag="ctxn")
                    return (h, cps, ctxn)

                def emit_scores(h):
                    # DoubleRow over dh: contraction (ki 32, parity 2); head
                    # h lives at partition group 32*(h%4) of ot pair
                    # (2*(h//4), 2*(h//4)+1). j-tiles are computed two per
                    # 2-bank psum so each Exp covers a pair in one shot.
                    u, sub = h // 4, h % 4
                    b0 = 32 * sub
                    expt = pattn.tile([P, 3328], BF16, tag="expt")
                    for pj in range(4):
                        j0 = 2 * pj
                        w0, w1 = NCOL[j0], NCOL[j0 + 1]
                        # two independent accumulation groups must not share
                        # a PSUM bank: place the second j at a 512 offset
                        po1 = max(w0, 512)
                        ps = ps_sc.tile([P, 1024], F32, tag="sc")
                        for j, w, po in ((j0, w0, 0), (j0 + 1, w1, po1)):
                            off = 512 - w
                            nc.tensor.matmul(
                                ps[:, po : po + w],
                                kt_s[b0 : b0 + 32, 2 * u : 2 * u + 2,
                                     j * P : (j + 1) * P],
                                qt_s[b0 : b0 + 32, 2 * u : 2 * u + 2, off:512],
                                start=True,
                                stop=True,
                                perf_mode=DR,
                                tile_position=(b0, 0),
                            )
                        if po1 == w0:
                            nc.scalar.activation(
                                expt[:, POFF[j0] : POFF[j0] + w0 + w1],
                                ps[:, 0 : w0 + w1],
                                AF.Exp,
                                scale=0.125 / (WSQK * WSQK),
                            )
                        else:
                            nc.scalar.activation(
                                expt[:, POFF[j0] : POFF[j0] + w0],
                                ps[:, 0:w0],
                                AF.Exp,
                                scale=0.125 / (WSQK * WSQK),
                            )
                            nc.scalar.activation(
                                expt[:, POFF[j0 + 1] : POFF[j0 + 1] + w1],
                                ps[:, po1 : po1 + w1],
                                AF.Exp,
                                scale=0.125 / (WSQK * WSQK),
                            )
                    for j in range(4, NKT):
                        # zero the masked upper triangle of the diagonal
                        # query block post-exp (Pool, off the hot engines)
                        nc.gpsimd.affine_select(
                            out=expt[:, POFF[j] : POFF[j] + P],
                            in_=expt[:, POFF[j] : POFF[j] + P],
                            compare_op=OP.is_ge,
                            fill=0.0,
                            base=0,
                            pattern=[[1, P]],
                            channel_multiplier=-1,
                        )
                    return (h, expt)

                # filler units: Q/K projections + V-proj tiles, ordered by
                # consumption deadline, drained during the head loop
                fillers = []
                for ot in (2, 3):
                    fillers += [
                        lambda o=ot: emit_qproj(o),
                        lambda o=ot: emit_kproj(o, 0),
                        lambda o=ot: emit_kproj(o, 1),
                    ]
                fillers += [lambda t=tk: emit_vproj(0, t) for tk in range(NKT)]
                for ot in (4, 5):
                    fillers += [
                        lambda o=ot: emit_qproj(o),
                        lambda o=ot: emit_kproj(o, 0),
                        lambda o=ot: emit_kproj(o, 1),
                    ]
                fillers += [lambda t=tk: emit_vproj(1, t) for tk in range(NKT)]
                for ot in (6, 7):
                    fillers += [
                        lambda o=ot: emit_qproj(o),
                        lambda o=ot: emit_kproj(o, 0),
                        lambda o=ot: emit_kproj(o, 1),
                    ]
                fillers.reverse()  # consume via pop()

                def drain(n):
                    for _ in range(n):
                        if fillers:
                            fillers.pop()()

                def need_qk(u):
                    # scores for head group u needs both parity tiles 2u, 2u+1
                    while not ({2 * u, 2 * u + 1} <= qk_done):
                        assert fillers, f"filler queue dry before qk pair {u}"
                        fillers.pop()()

                def need_v(db):
                    while v_done[db] < NKT:
                        assert fillers, f"filler queue dry before v {db}"
                        fillers.pop()()

                # warmup: head-group-0 Q/K, then 3 heads of scores while the
                # filler queue builds V/QK state; first ctx after V db0 done
                emit_qproj(0)
                emit_kproj(0, 0)
                emit_kproj(0, 1)
                emit_qproj(1)
                emit_kproj(1, 0)
                emit_kproj(1, 1)
                from collections import deque

                prevs = deque()
                prevs.append(emit_scores(0))
                drain(3)
                prevs.append(emit_scores(1))
                nc.sync.dma_start(out=xres_s[:], in_=xres[:])
                drain(3)
                prevs.append(emit_scores(2))
                drain(4)
                need_v(0)
                nc.sync.dma_start(out=b1r_s[:], in_=b1r[:])
                nc.sync.dma_start(out=b1e_s[:], in_=b1e[:])
                for nm, _src in (
                    ("b2t", b2t),
                    ("g1t", g1t),
                    ("be1t", be1t),
                    ("g2t", g2t),
                    ("be2t", be2t),
                ):
                    t = persist.tile([P, NKT], F32, tag=nm)
                    nc.sync.dma_start(out=t[:], in_=_src[:])
                    small[nm] = t
                # prefetch the first fc1/fc2 weight chunks during attention
                # so the FFN phases never wait on the serial SP DMA queue
                for _c in range(3):
                    load_w1(_c)
                load_w2(0)
                load_w2(1)
                pending = emit_ctx(prevs.popleft())
                for h in range(3, H):
                    need_qk(h // 4)
                    prevs.append(emit_scores(h))
                    drain(1)
                    emit_post(pending)
                    nh = prevs[0][0]
                    need_v(nh // 8)
                    pending = emit_ctx(prevs.popleft())
                drain(len(fillers))
                while prevs:
                    emit_post(pending)
                    need_v(1)
                    pending = emit_ctx(prevs.popleft())
                emit_post(pending)
                flush_stats()

                if DEBUG_TAPS:
                    dbg_xres = nc.dram_tensor(
                        "dbg_xres", [P, NKT, QTOK], F32, kind="ExternalOutput"
                    ).ap()
                    nc.sync.dma_start(out=dbg_xres[:], in_=xres_s[:])
                    dbg_xt = nc.dram_tensor(
                        "dbg_xt", [P, NKT, KV], F8, kind="ExternalOutput"
                    ).ap()
                    nc.sync.dma_start(out=dbg_xt[:], in_=xt_s[:])
                    dbg_kt = nc.dram_tensor(
                        "dbg_kt", [P, NKT, KV], F8, kind="ExternalOutput"
                    ).ap()
                    nc.sync.dma_start(out=dbg_kt[:], in_=kt_s[:])
                    dbg_stats = nc.dram_tensor(
                        "dbg_stats", [P, 2, 512], F32, kind="ExternalOutput"
                    ).ap()
                    nc.sync.dma_start(out=dbg_stats[:, 0], in_=sum1_sb[:])
                    nc.sync.dma_start(out=dbg_stats[:, 1], in_=sumsq1_sb[:])
                    dbg_srow = nc.dram_tensor(
                        "dbg_srow", [P, 2, QTOK], BF16, kind="ExternalOutput"
                    ).ap()
                    nc.sync.dma_start(out=dbg_srow[:, 0], in_=srowE_bf[:])
                    nc.sync.dma_start(out=dbg_srow[:, 1], in_=srowO_bf[:])

                # ---- phase 3: LN1 (stats already accumulated) ----
                # ln1_bf (bf16) is the fc2 residual; x8/x8e are the fp8
                # hi/lo pair feeding the compensated fc1 DoubleRow passes.
                ln1_bf = lnp.tile([P, NKT, QTOK], BF16, tag="ln1")
                x8 = lnp.tile([P, NKT, QTOK], F8, tag="x8")
                x8e = lnp.tile([P, NKT, QTOK], F8, tag="x8e")
                mean1, rstd1 = ln_meanvar(sum1_sb, sumsq1_sb)
                for kt in range(NKT):
                    ln_apply(
                        xres_s[:, kt], mean1, rstd1,
                        small["g1t"], small["be1t"], ln1_bf[:, kt], kt,
                    )
                    nc.gpsimd.tensor_copy(x8[:, kt], ln1_bf[:, kt])
                    nc.vector.tensor_tensor(
                        x8e[:, kt], ln1_bf[:, kt], x8[:, kt], OP.subtract
                    )
                pxstack.close()
                attn_stack.close()

            # ---- phase 4: fc1 + selu (w1 in JIT 4-ot chunks, depth 2) ----
            pffn_stack = contextlib.ExitStack()
            pffn = pffn_stack.enter_context(tc.tile_pool(name="pffn", bufs=1))
            ps_mm = pffn_stack.enter_context(
                tc.tile_pool(name="ps_mm", bufs=3, space="PSUM")
            )
            ps_x = pffn_stack.enter_context(
                tc.tile_pool(name="ps_x", bufs=1, space="PSUM")
            )
            # deep ring for the selu temporaries: with only 2 bufs the
            # Relu of ot must wait for Pool's h8 copy of ot-2 (slot reuse),
            # putting ~1us of Pool/Act latency on the PE critical path
            pselu = pffn_stack.enter_context(tc.tile_pool(name="pselu", bufs=4))
            h8 = pffn.tile([P, NOT1, QTOK], F8, tag="h8")
            h8e = pffn.tile([P, NOT1, QTOK], F8, tag="h8e")
            # ots 0..3 run kp-major across four live psums so each matmul
            # group consumes x8/x8e kt-pairs as LN1 streams them out --
            # otherwise the first psum group alone needs the full x8 tile
            # and the PE idles through the whole LN1 quant trench
            ps03 = []
            for ot in range(4):
                w1h, w1l = w1bufs[(ot // 2) % NW1B]
                if ot == 3:
                    psi = ps_x.tile([P, 512], F32, tag="x", name=f"ps03_{ot}")
                else:
                    psi = ps_mm.tile([P, 512], F32, tag="mm", name=f"ps03_{ot}")
                ps03.append((psi, w1h, w1l))
            load_w1(3)
            load_w1(4)
            for kp in range(NKT // 2):
                for ot in range(4):
                    psi, w1h, w1l = ps03[ot]
                    for pi, xq_w in enumerate(((w1h, x8), (w1l, x8), (w1h, x8e))):
                        wt, xq = xq_w
                        nc.tensor.matmul(
                            psi[:],
                            wt[:, ot % 2, 2 * kp : 2 * kp + 2],
                            xq[:, 2 * kp : 2 * kp + 2],
                            start=(kp == 0 and pi == 0),
                            stop=(kp == NKT // 2 - 1 and pi == 2),
                            perf_mode=DR,
                        )
            for ot in range(NOT1):
                if ot < 4:
                    ps = ps03[ot][0]
                else:
                    w1h, w1l = w1bufs[(ot // 2) % NW1B]
                    if ot % 2 == 0:
                        load_w1(ot // 2 + 3)
                    if ot % 4 == 3:
                        ps = ps_x.tile([P, 512], F32, tag="x")
                    else:
                        ps = ps_mm.tile([P, 512], F32, tag="mm")
                    passes = [(w1h, x8), (w1l, x8), (w1h, x8e)]
                    for pi, (wt, xq) in enumerate(passes):
                        for kp in range(NKT // 2):
                            nc.tensor.matmul(
                                ps[:],
                                wt[:, ot % 2, 2 * kp : 2 * kp + 2],
                                xq[:, 2 * kp : 2 * kp + 2],
                                start=(pi == 0 and kp == 0),
                                stop=(pi == 2 and kp == NKT // 2 - 1),
                                perf_mode=DR,
                            )
                p_t = pselu.tile([P, QTOK], F32, tag="selup")
                nc.scalar.activation(
                    p_t[:],
                    ps[:],
                    AF.Relu,
                    scale=SELU_S / WS,
                    bias=b1r_s[:, ot : ot + 1],
                )
                e_t = pselu.tile([P, QTOK], F32, tag="selue")
                nc.scalar.activation(
                    e_t[:], ps[:], AF.Exp, scale=1.0 / WS, bias=b1e_s[:, ot : ot + 1]
                )
                # selu(z) = min(sa*e^z - sa, s*relu(z))
                nc.vector.scalar_tensor_tensor(
                    p_t[:], e_t[:], SELU_SA, p_t[:], OP.subtract, OP.min
                )
                nc.gpsimd.tensor_copy(h8[:, ot], p_t[:])
                nc.vector.tensor_tensor(h8e[:, ot], p_t[:], h8[:, ot], OP.subtract)

            # ---- phase 5: fc2 + residual + LN2 + store (full 512 width) ----
            ps_stat2 = pffn_stack.enter_context(
                tc.tile_pool(name="ps_stat2", bufs=2, space="PSUM")
            )
            res2 = pffn.tile([P, NKT, QTOK], F32, tag="res2")
            ps0_2 = ps_stat2.tile([P, 512], F32, tag="stat2")
            ps1_2 = ps_stat2.tile([P, 512], F32, tag="stat2")
            for ot in range(NKT):
                w2h, w2l = w2bufs[ot % NW2B]
                load_w2(ot + 2)  # ots 0,1 preloaded in attention
                if ot % 4 == 3:
                    ps = ps_x.tile([P, 512], F32, tag="x")
                else:
                    ps = ps_mm.tile([P, 512], F32, tag="mm")
                passes = [(w2h, h8), (w2l, h8), (w2h, h8e)]
                for pi, (wt, hq) in enumerate(passes):
                    for kp in range(NOT1 // 2):
                        nc.tensor.matmul(
                            ps[:],
                            wt[:, 2 * kp : 2 * kp + 2],
                            hq[:, 2 * kp : 2 * kp + 2],
                            start=(pi == 0 and kp == 0),
                            stop=(pi == 2 and kp == NOT1 // 2 - 1),
                            perf_mode=DR,
                        )
                t1 = tmp2.tile([P, QTOK], F32, tag="r2t")
                nc.scalar.activation(
                    t1[:],
                    ps[:],
                    AF.Identity,
                    scale=1.0 / WS,
                    bias=small["b2t"][:, ot : ot + 1],
                )
                nc.vector.tensor_tensor(
                    res2[:, ot], t1[:], ln1_bf[:, ot], OP.add
                )
                cast_t = tmp.tile([P, QTOK], BF16, tag="lncast2")
                sq_t = tmp.tile([P, QTOK], BF16, tag="lnsq2")
                nc.vector.tensor_copy(cast_t[:], res2[:, ot])
                nc.scalar.activation(sq_t[:], res2[:, ot], AF.Square)
                ln_stats_mm(ps0_2, ps1_2, cast_t, sq_t, ot)
            mean2, rstd2 = ln_meanvar(ps0_2, ps1_2)
            for kt in range(NKT):
                # spread the tail normalize across DVE and Pool so the
                # final 8-tile chain isn't serialized on one engine; the
                # scale-bias always runs on the otherwise-idle Act engine
                eng = nc.gpsimd if kt in (2, 5) else nc.vector
                ln_apply(
                    res2[:, kt], mean2, rstd2,
                    small["g2t"], small["be2t"], res2[:, kt], kt,
                    eng=eng,
                )
                nc.sync.dma_start(out=out[:, kt], in_=res2[:, kt])
            pffn_stack.close()

    _legalize_waits(nc)
    return nc


_NC_CACHE = None
TRACE = False
LAST_EXEC_NS = None


def _get_nc():
    global _NC_CACHE
    if _NC_CACHE is None:
        _NC_CACHE = _build_nc()
    return _NC_CACHE


def _tile_w(a):
    """[Din, O] -> [P, O//P(ot), Din//P(kt), P] with ot-contiguous DMA slices."""
    Din, O = a.shape
    return np.ascontiguousarray(
        a.reshape(Din // P, P, O // P, P).transpose(1, 2, 0, 3)
    )


def _pp(v, n):
    """[n*P] -> [P, n] per-partition layout."""
    return np.ascontiguousarray(v.reshape(n, P).T)


def kernel(X, wq, wk, wv, ln1_g, ln1_b, w1, b1, w2, b2, ln2_g, ln2_b):
    from concourse.bass_utils import run_bass_kernel_spmd

    X = np.asarray(X, np.float32)
    bf = ml_dtypes.bfloat16
    f8 = ml_dtypes.float8_e4m3  # IEEE flavor — matches bass float8e4

    def hilo(wt):
        hi = wt.astype(f8)
        lo = (wt - hi.astype(np.float32)).astype(f8)
        return hi, lo

    # scores-DR out-dim permutation: slot (ot, i) holds projection row
    # head*64 + dh with head = 4*(ot//2) + i//32, dh = 2*(i%32) + ot%2,
    # so head h sits at partition group 32*(h%4) of tiles (2u, 2u+1)
    # with the dh parity split across the tile pair (DoubleRow Ko dim).
    qperm = np.empty(D, np.int64)
    for _ot in range(NKT):
        for _i in range(P):
            _h = 4 * (_ot // 2) + _i // 32
            _dh = 2 * (_i % 32) + (_ot % 2)
            qperm[_ot * P + _i] = _h * 64 + _dh
    wqT = _tile_w((WSQK * np.asarray(wq, np.float32).T)[:, qperm]).astype(f8)
    wkT = _tile_w((WSQK * np.asarray(wk, np.float32).T)[:, qperm]).astype(f8)
    wvT = np.ascontiguousarray(
        WS * np.asarray(wv, np.float32).T.reshape(NKT, P, D).transpose(1, 0, 2)
    ).astype(f8)
    w1hi, w1lo = hilo(_tile_w(WS * np.asarray(w1, np.float32).T))
    w2hi, w2lo = hilo(_tile_w(WS * np.asarray(w2, np.float32).T))
    b1 = np.asarray(b1, np.float32)
    shared = dict(
        wq=wqT,
        wk=wkT,
        wv=wvT,
        w1=w1hi,
        w1e=w1lo,
        w2=w2hi,
        w2e=w2lo,
        b1r=_pp(SELU_S * b1, NOT1),
        b1e=_pp(b1 + LN_SA, NOT1),
        b2t=_pp(np.asarray(b2, np.float32), NKT),
        g1t=_pp(np.asarray(ln1_g, np.float32), NKT),
        be1t=_pp(np.asarray(ln1_b, np.float32), NKT),
        g2t=_pp(np.asarray(ln2_g, np.float32), NKT),
        be2t=_pp(np.asarray(ln2_b, np.float32), NKT),
    )

    in_maps = []
    for c in range(8):
        b, hf = c // 2, c % 2
        if hf == 1:
            xkv = X[b].T  # [D, L]
            valid = np.full(KV, WS, np.float32)
            xq = X[b, 512:]
        else:
            xkv = np.concatenate(
                [np.zeros((D, 512), np.float32), X[b, :512].T], axis=1
            )
            valid = np.concatenate(
                [np.zeros(512, np.float32), np.full(512, WS, np.float32)]
            )
            xq = X[b, :512]
        xt = (
            np.ascontiguousarray(xkv.reshape(NKT, P, KV).transpose(1, 0, 2))
        ).astype(f8)
        xres = np.ascontiguousarray(xq.T.reshape(NKT, P, QTOK).transpose(1, 0, 2))
        vt = valid.reshape(NKT, P).T  # [P, NKT]
        val16 = (
            np.repeat(vt[:, :, None], H, axis=2).reshape(P, NKT, NKT, 2).astype(bf)
        )
        m = dict(shared)
        m.update(xt=xt, xres=xres, valid16=np.ascontiguousarray(val16))
        in_maps.append(m)

    nc = _get_nc()
    global LAST_EXEC_NS
    if TRACE:
        res = run_bass_kernel_spmd(nc, in_maps, list(range(8)), trace=True)
        LAST_EXEC_NS = res.exec_time_ns
    else:
        res = run_bass_kernel_spmd(nc, in_maps, list(range(8)))

    out = np.empty((B, L, D), np.float32)
    for c in range(8):
        b, hf = c // 2, c % 2
        o = res.results[c]["out"]  # [P, NKT, QTOK]
        o = o.transpose(1, 0, 2).reshape(D, QTOK).T  # [QTOK, D]
        out[b, hf * 512 : hf * 512 + 512] = o
    return out



# revision 95
# speedup vs baseline: 1.2241x; 1.0092x over previous
"""Decoder-layer Trainium2 kernel: 8-core SPMD, single launch, no collectives.

Sharding: core c -> (batch b = c // 2, sequence-half hf = c % 2). Each core
computes the full decoder layer for 512 query tokens of one sequence.
All cores run ONE identical program over a canonical virtual sequence of
1024 kv tokens with queries at virtual positions 512..1023; first-half cores
get their 512 real tokens placed at virtual 512..1023 with zero-padded kv
prefix and a `valid` vector that zeroes the pad contribution to the softmax
denominator.

v2 changes vs baseline:
- softmax denominators ride along in the ctx matmul via an augmented V
  (per head-pair V layout [Edims|Eden|Oden|zeros63|Odims], 193 wide): even
  heads matmul M=65 -> dims at psum rows 0..63 + den at row 64; odd heads
  M=128 with a zero block -> den at row 0 + dims at rows 64..127. Kills the
  65536 rows of separate [1,512] denominator matmuls.
- ctx matmuls are causally restricted to the live query range per kv tile
  (like scores), saving another 12288 rows.
- LN1 stats matmuls run inline as each head pair finishes its xres tile.
- fc2 + LN2 run in two token-half passes so the final normalize/store of
  half 0 overlaps the fc2 matmuls of half 1.
"""

import sys

sys.path.insert(0, "/opt/trn_rl_repo")

import math

import numpy as np
import ml_dtypes

import concourse.bass as bass
import concourse.mybir as mybir
from concourse.tile import TileContext, TilePool
from concourse.vector_clock import ScopedClock

BF16 = mybir.dt.bfloat16
F8 = mybir.dt.float8e4
F32 = mybir.dt.float32
AF = mybir.ActivationFunctionType
OP = mybir.AluOpType
DR = mybir.MatmulPerfMode.DoubleRow
WS = 64.0  # fp8 weight pre-scale (wv/w1/w2)
# Q/K projections use a smaller pre-scale: bass float8e4 is IEEE e4m3
# (max finite 240, saturates to inf) and |K|*64 reaches ~290 on some
# batches; *32 keeps the fp8 Q/K copies comfortably finite.
WSQK = 32.0

B, L, D = 4, 1024, 1024
H, DH = 16, 64
DFF = 4 * D
P = 128
QTOK = 512  # query tokens per core
KV = 1024  # canonical kv length (virtual)
NKT = D // P  # 8 d-tiles
NOT1 = DFF // P  # 32 fc1 out tiles
MASK_NEG = -1.0e9
VW = 193  # augmented V width per head pair: [Ed 64|Eden|Oden|z 63|Od 64]

SELU_S = 1.0507009873554804934193349852946
SELU_A = 1.6732632423543772848170429916717
SELU_SA = SELU_S * SELU_A
LN_SA = math.log(SELU_SA)
LN_EPS = 1e-5


class PatchedTileContext(TileContext):
    """TileContext whose exit drain respects this walrus build's limit of
    ONE semaphore wait per instruction: the global-clock waits are spread
    across standalone NOPs and the butterfly barrier (whose sem-eq waits
    walrus rejects) is replaced by the NRT-expanded pseudo barrier."""

    def _drain_and_barrier(self, tick_clock, wait_clock):
        nc = self.nc
        carrier = nc.sync.nop()
        wait_clock.add_sem_waits(
            carrier.ins, ScopedClock({None: tick_clock.global_clock})
        )
        waits = list(carrier.ins.sync_info.on_wait)
        ups = list(carrier.ins.sync_info.on_update)
        if len(waits) > 1:
            carrier.ins.sync_info = mybir.SyncInfo(on_wait=[waits[0]], on_update=ups)
            for w in waits[1:]:
                extra = nc.sync.nop()
                extra.ins.sync_info = mybir.SyncInfo(on_wait=[w], on_update=[])
        for eng in nc.engines.values():
            eng.drain()
        nc._nrt_pseudo_barrier()
        popped = nc._tile_sem_poison_stack.pop()
        assert popped is self._sem_poison
        nc.clear_and_free_semaphores(list(self.sems.allocated().values()))
        nc._nrt_pseudo_barrier()


def _legalize_waits(nc):
    """This walrus build accepts at most ONE semaphore wait per instruction.
    Tile's sem-assignment can attach several; hoist the extras onto same-engine
    NOPs inserted immediately before the instruction (waits are a conjunction,
    so a sequence of single-wait stalls is equivalent)."""
    n = 0
    for fn in nc.m.functions:
        for blk in fn.blocks:
            out = []
            changed = False
            for inst in blk.instructions:
                si = getattr(inst, "sync_info", None)
                if si is not None and len(si.on_wait) > 1:
                    waits = list(si.on_wait)
                    for w in waits[:-1]:
                        nop = mybir.InstNoOp(name=f"waitnop_{n}", ins=[], outs=[])
                        n += 1
                        nop.engine = inst.engine
                        nop.sync_info = mybir.SyncInfo(on_wait=[w], on_update=[])
                        out.append(nop)
                    inst.sync_info = mybir.SyncInfo(
                        on_wait=[waits[-1]], on_update=list(si.on_update)
                    )
                    changed = True
                out.append(inst)
            if changed:
                blk.instructions = out
    return n


DEBUG_TAPS = False
DBG_HEAD = 0


def _build_nc():
    nc = bass.Bass("TRN2", target_bir_lowering=False, debug=False, num_devices=8)

    def din(name, shape, dt):
        return nc.dram_tensor(name, shape, dt, kind="ExternalInput").ap()

    xt = din("xt", [P, NKT, KV], F8)  # X[b].T tiled, virtual-padded
    xres = din("xres", [P, NKT, QTOK], F32)  # q tokens transposed, fp32
    valid16 = din("valid16", [P, NKT, NKT, 2], BF16)  # WS flag, [8hp x 2]
    wq = din("wq", [P, NKT, NKT, P], F8)  # [dpart, ot, kt, o], x WS
    wk = din("wk", [P, NKT, NKT, P], F8)
    wv = din("wv", [P, NKT, D], F8)  # rhs layout [dpart, kt, o], x WS
    w1 = din("w1", [P, NOT1, NKT, P], F8)  # fp8(WS*w1^T)
    w1e = din("w1e", [P, NOT1, NKT, P], F8)  # fp8 residual of the above
    w2 = din("w2", [P, NKT, NOT1, P], F8)
    w2e = din("w2e", [P, NKT, NOT1, P], F8)
    b1r = din("b1r", [P, NOT1], F32)  # SELU_S * b1
    b1e = din("b1e", [P, NOT1], F32)  # b1 + ln(SELU_S*SELU_A)
    b2t = din("b2t", [P, NKT], F32)
    g1t = din("g1t", [P, NKT], F32)
    be1t = din("be1t", [P, NKT], F32)
    g2t = din("g2t", [P, NKT], F32)
    be2t = din("be2t", [P, NKT], F32)
    out = nc.dram_tensor("out", [P, NKT, QTOK], F32, kind="ExternalOutput").ap()

    with PatchedTileContext(nc) as tc:
        import contextlib

        with contextlib.ExitStack() as ctx:
            persist = ctx.enter_context(tc.tile_pool(name="persist", bufs=1))
            bc = ctx.enter_context(tc.tile_pool(name="bc", bufs=1))
            wpool = ctx.enter_context(tc.tile_pool(name="wpool", bufs=4))
            tmp = ctx.enter_context(tc.tile_pool(name="tmp", bufs=2))
            tmp2 = ctx.enter_context(tc.tile_pool(name="tmp2", bufs=2))
            lnp = ctx.enter_context(tc.tile_pool(name="lnp", bufs=1))
            w1pool = ctx.enter_context(tc.tile_pool(name="w1pool", bufs=1))

            # ---- constants ----
            NW1B, NW2B = 5, 4
            w1bufs = [
                (
                    w1pool.tile(
                        [P, 2, NKT, P], F8, tag=f"w1{i}h", name=f"w1{i}h"
                    ),
                    w1pool.tile(
                        [P, 2, NKT, P], F8, tag=f"w1{i}e", name=f"w1{i}e"
                    ),
                )
                for i in range(NW1B)
            ]
            w2bufs = [
                (
                    w1pool.tile(
                        [P, NOT1, P], F8, tag=f"w2{i}h", name=f"w2{i}h"
                    ),
                    w1pool.tile(
                        [P, NOT1, P], F8, tag=f"w2{i}e", name=f"w2{i}e"
                    ),
                )
                for i in range(NW2B)
            ]

            def load_w1(chunk):
                if chunk < NOT1 // 2:
                    hb, lb = w1bufs[chunk % NW1B]
                    nc.sync.dma_start(out=hb[:], in_=w1[:, 2 * chunk : 2 * chunk + 2])
                    nc.sync.dma_start(out=lb[:], in_=w1e[:, 2 * chunk : 2 * chunk + 2])

            def load_w2(ot):
                if ot < NKT:
                    hb, lb = w2bufs[ot % NW2B]
                    nc.sync.dma_start(out=hb[:], in_=w2[:, ot])
                    nc.sync.dma_start(out=lb[:], in_=w2e[:, ot])
            ones128 = persist.tile([P, P], BF16, tag="ones128")
            nc.gpsimd.memset(ones128[:], 1.0)
            ones_r0 = persist.tile([P, P], BF16, tag="ones_r0")
            nc.gpsimd.memset(ones_r0[:], 0.0)
            nc.gpsimd.memset(ones_r0[0:1, :], 1.0)
            ones_r64 = persist.tile([P, P], BF16, tag="ones_r64")
            nc.gpsimd.memset(ones_r64[:], 0.0)
            nc.gpsimd.memset(ones_r64[64:65, :], 1.0)
            srowE_bf = persist.tile([P, QTOK], BF16, tag="srowEbf")
            nc.vector.memset(srowE_bf[:], 0.0)
            srowO_bf = persist.tile([P, QTOK], BF16, tag="srowObf")
            nc.vector.memset(srowO_bf[:], 0.0)
            eps_ap = persist.tile([P, 1], F32, tag="eps")
            nc.gpsimd.memset(eps_ap[:], LN_EPS)

            def ln_stats_mm(ps0, ps1, cast_t, sq_t, kt, n=NKT, ncols=QTOK):
                nc.tensor.matmul(
                    ps0[:, 0:ncols],
                    ones128[:],
                    cast_t[:],
                    start=(kt == 0),
                    stop=(kt == n - 1),
                )
                nc.tensor.matmul(
                    ps1[:, 0:ncols],
                    ones128[:],
                    sq_t[:],
                    start=(kt == 0),
                    stop=(kt == n - 1),
                )

            def ln_meanvar(ps0, ps1, ncols=QTOK):
                """stats psums -> (mean, rstd) broadcast tiles.

                rstd = exp(-0.5*ln(var+eps)): Ln and Exp share an Act table
                (natural_log_exp_and_others) with Relu/Identity/Square, so
                this never forces the 1.3us act-table reload that Sqrt would.
                """
                if ps1 is None:
                    # ps0 is a [P, 2*ncols] psum holding [sum | sumsq]:
                    # scale both with one DVE op into an adjacent pair
                    mv = bc.tile([P, 2 * ncols], F32, tag="meanvar")
                    nc.vector.tensor_scalar_mul(mv[:], ps0[:, 0 : 2 * ncols], 1.0 / D)
                    mean_bc, var_bc = mv[:, 0:ncols], mv[:, ncols : 2 * ncols]
                else:
                    mean_t = bc.tile([P, ncols], F32, tag="mean", name="mean_t")
                    mean_bc = mean_t[:]
                    nc.vector.tensor_scalar_mul(mean_bc, ps0[:, 0:ncols], 1.0 / D)
                    var_t = bc.tile([P, ncols], F32, tag="var", name="var_t")
                    var_bc = var_t[:]
                    nc.vector.tensor_scalar_mul(var_bc, ps1[:, 0:ncols], 1.0 / D)
                m2 = tmp2.tile([P, ncols], F32, tag="lnt")
                nc.vector.tensor_tensor(m2[:], mean_bc, mean_bc, OP.mult)
                nc.vector.tensor_tensor(var_bc, var_bc, m2[:], OP.subtract)
                nc.scalar.activation(var_bc, var_bc, AF.Ln, bias=eps_ap[:])
                nc.scalar.activation(var_bc, var_bc, AF.Exp, scale=-0.5)
                return mean_bc, var_bc

            def ln_apply(
                src_kt, mean_bc, var_bc, g_ap, b_ap, dst_kt, kt, ncols=QTOK,
                eng=None, fin_act=None,
            ):
                eng = eng or nc.vector
                t1 = tmp2.tile([P, ncols], F32, tag="lnt")
                eng.tensor_tensor(t1[:], src_kt, mean_bc[:], OP.subtract)
                eng.tensor_tensor(t1[:], t1[:], var_bc[:], OP.mult)
                if fin_act if fin_act is not None else (kt % 2 == 0):
                    nc.scalar.activation(
                        dst_kt,
                        t1[:],
                        AF.Identity,
                        scale=g_ap[:, kt : kt + 1],
                        bias=b_ap[:, kt : kt + 1],
                    )
                else:
                    nc.vector.tensor_scalar(
                        dst_kt,
                        t1[:],
                        g_ap[:, kt : kt + 1],
                        b_ap[:, kt : kt + 1],
                        OP.mult,
                        OP.add,
                    )

            # ---- phase 1+2 fused: QKV projections + attention ----
            with tc.tile_pool(name="pproj", bufs=1) as pproj:
                import contextlib as _ctl

                attn_stack = _ctl.ExitStack()
                pattn = attn_stack.enter_context(tc.tile_pool(name="pattn", bufs=3))
                ps_ctx = attn_stack.enter_context(
                    tc.tile_pool(name="ps_ctx", bufs=2, space="PSUM")
                )
                ps_sc = attn_stack.enter_context(
                    tc.tile_pool(name="ps_sc", bufs=2, space="PSUM")
                )
                ps_fill = attn_stack.enter_context(
                    tc.tile_pool(name="ps_fill", bufs=2, space="PSUM")
                )
                pxstack = _ctl.ExitStack()
                px = pxstack.enter_context(tc.tile_pool(name="px", bufs=1))
                wq_t0 = wpool.tile([P, NKT, P], F8, tag="wqkv")
                nc.sync.dma_start(out=wq_t0[:], in_=wq[:, 0])
                xt_s = px.tile([P, NKT, KV], F8, tag="xt")
                nc.sync.dma_start(out=xt_s[:, 0:4], in_=xt[:, 0:4])
                nc.sync.dma_start(out=xt_s[:, 4:8], in_=xt[:, 4:8])
                wk_t0 = wpool.tile([P, NKT, P], F8, tag="wqkv")
                nc.sync.dma_start(out=wk_t0[:], in_=wk[:, 0])
                # Q/K in fp8, scores-DR grouped layout: tile ot = (u, c)
                # with u = ot//2 (head group 4u..4u+3), c = ot%2 (dh parity);
                # partition 32*g+ki holds head 4u+g, dh = 2*ki + c.
                qt_s = pproj.tile([P, NKT, QTOK], F8, tag="qt")
                kt_s = pproj.tile([P, NKT, KV], F8, tag="kt")
                # augmented V: per (kv-tile j, head pair hp) 193 cols:
                # [Edims 64 | Eden 1 | Oden 1 | zeros 63 | Odims 64]
                vaug = pproj.tile([P, NKT, NKT, VW], BF16, tag="vaug")
                nc.gpsimd.memset(vaug[:, :, :, 66:129], 0.0)

                wv_s = px.tile([P, NKT, D], F8, tag="wv")
                nc.sync.dma_start(out=wv_s[:], in_=wv[:])
                val_s = pproj.tile([P, NKT, NKT, 2], BF16, tag="val")
                nc.sync.dma_start(out=val_s[:], in_=valid16[:])
                xres_s = pproj.tile([P, NKT, QTOK], F32, tag="xres")
                b1r_s = persist.tile([P, NOT1], F32, tag="b1r")
                b1e_s = persist.tile([P, NOT1], F32, tag="b1e")
                small = {}
                sum1_sb = pproj.tile([P, 512], F32, tag="sum1")
                sumsq1_sb = pproj.tile([P, 512], F32, tag="sumsq1")

                qk_done = set()
                v_done = {0: 0, 1: 0}

                def emit_qproj(ot):
                    if ot == 0:
                        wq_t = wq_t0
                    else:
                        wq_t = wpool.tile([P, NKT, P], F8, tag="wqkv")
                        nc.sync.dma_start(out=wq_t[:], in_=wq[:, ot])
                    ps = ps_fill.tile([P, 512], F32, tag="fill")
                    for kp in range(NKT // 2):
                        nc.tensor.matmul(
                            ps[:],
                            wq_t[:, 2 * kp : 2 * kp + 2],
                            xt_s[:, 2 * kp : 2 * kp + 2, 512:1024],
                            start=(kp == 0),
                            stop=(kp == NKT // 2 - 1),
                            perf_mode=DR,
                        )
                    nc.vector.tensor_copy(qt_s[:, ot], ps[:])

                wk_ts = {0: wk_t0}

                def emit_kproj(ot, tb):
                    if tb == 0 and ot not in wk_ts:
                        wk_t = wpool.tile([P, NKT, P], F8, tag="wqkv")
                        nc.sync.dma_start(out=wk_t[:], in_=wk[:, ot])
                        wk_ts[ot] = wk_t
                    wk_t = wk_ts[ot]
                    ps = ps_fill.tile([P, 512], F32, tag="fill")
                    for kp in range(NKT // 2):
                        nc.tensor.matmul(
                            ps[:],
                            wk_t[:, 2 * kp : 2 * kp + 2],
                            xt_s[:, 2 * kp : 2 * kp + 2, tb * 512 : (tb + 1) * 512],
                            start=(kp == 0),
                            stop=(kp == NKT // 2 - 1),
                            perf_mode=DR,
                        )
                    if tb == 0:
                        nc.scalar.copy(kt_s[:, ot, 0:512], ps[:])
                    else:
                        nc.vector.tensor_copy(kt_s[:, ot, 512:1024], ps[:])
                        qk_done.add(ot)

                def emit_vproj(db, tk):
                    if db == 0:
                        # den columns for all 8 pairs x 2 parities
                        nc.vector.tensor_copy(vaug[:, tk, :, 64:66], val_s[:, tk])
                    ps = ps_fill.tile([P, 4, P], F32, tag="fill")
                    for kp in range(NKT // 2):
                        nc.tensor.matmul(
                            ps[:, :, :],
                            xt_s[:, 2 * kp : 2 * kp + 2, tk * P : (tk + 1) * P],
                            wv_s[:, 2 * kp : 2 * kp + 2, db * 512 : (db + 1) * 512],
                            start=(kp == 0),
                            stop=(kp == NKT // 2 - 1),
                            perf_mode=DR,
                        )
                    hp0 = db * 4
                    nc.vector.tensor_copy(
                        vaug[:, tk, hp0 : hp0 + 4, 0:64], ps[:, :, 0:64]
                    )
                    nc.vector.tensor_copy(
                        vaug[:, tk, hp0 : hp0 + 4, 129:193], ps[:, :, 64:128]
                    )
                    v_done[db] += 1

                stats_pending = []

                def flush_stats():
                    while stats_pending:
                        cast_t, sq_t, hp = stats_pending.pop(0)
                        pss = ps_sc.tile([P, 1024], F32, tag="sc")
                        nc.tensor.matmul(
                            pss[:, 0:512], ones128[:], cast_t[:], start=True, stop=True
                        )
                        nc.tensor.matmul(
                            pss[:, 512:1024], ones128[:], sq_t[:], start=True, stop=True
                        )
                        if hp == 0:
                            nc.vector.tensor_copy(sum1_sb[:], pss[:, 0:512])
                            nc.vector.tensor_copy(sumsq1_sb[:], pss[:, 512:1024])
                        else:
                            nc.vector.tensor_tensor(
                                sum1_sb[:], sum1_sb[:], pss[:, 0:512], OP.add
                            )
                            nc.vector.tensor_tensor(
                                sumsq1_sb[:], sumsq1_sb[:], pss[:, 512:1024], OP.add
                            )

                def emit_post(p):
                    h, cps, ctxn = p
                    hp, par = h // 2, h % 2
                    po = 64 * par
                    flush_stats()
                    # broadcast the bf16 reciprocal row across the 64 ctx
                    # partitions via a ones-matmul (bcp shares the fill pool)
                    bcp = ps_fill.tile([P, 512], F32, tag="fill")
                    if par == 0:
                        nc.tensor.matmul(
                            bcp[0:64],
                            ones_r64[:, 0:64],
                            srowE_bf[:],
                            start=True,
                            stop=True,
                        )
                    else:
                        nc.tensor.matmul(
                            bcp[64:128],
                            ones_r0[:, 0:64],
                            srowO_bf[:],
                            start=True,
                            stop=True,
                        )
                    bc_sb = tmp2.tile([P, 512], BF16, tag="bcsb")
                    nc.vector.tensor_copy(bc_sb[po : po + 64], bcp[po : po + 64])
                    nc.vector.tensor_tensor(
                        ctxn[po : po + 64],
                        cps[po : po + 64],
                        bc_sb[po : po + 64],
                        OP.mult,
                    )
                    if par == 1:
                        # pair finished: residual add + LN1 stats (Pool
                        # takes the add + cast + square; PE the stats,
                        # deferred one slot so PE never waits on Pool).
                        # Last pair runs on DVE: Pool's 0.42 efficiency
                        # would sit on the attention->LN1 critical path.
                        eng = nc.vector if hp == H // 2 - 1 else nc.gpsimd
                        eng.tensor_add(
                            xres_s[:, hp], xres_s[:, hp], ctxn[:]
                        )
                        cast_t = tmp.tile([P, 512], BF16, tag="lncast")
                        sq_t = tmp.tile([P, 512], BF16, tag="lnsq")
                        eng.tensor_copy(cast_t[:], xres_s[:, hp])
                        eng.tensor_mul(
                            sq_t[:], xres_s[:, hp], xres_s[:, hp]
                        )
                        stats_pending.append((cast_t, sq_t, hp))

                ctxn = None
                # packed expt: per-j live query range [off_j, 512) stored
                # contiguously; POFF[j] is the packed start, NCOL[j] the width
                NCOL = [512 - max(0, j - 4) * P for j in range(NKT)]
                POFF = [0] * NKT
                for j in range(1, NKT):
                    POFF[j] = POFF[j - 1] + NCOL[j - 1]

                def emit_ctx(pr):
                    nonlocal ctxn
                    h, expt = pr
                    hp, par = h // 2, h % 2
                    cps = ps_ctx.tile([P, 512], F32, tag="ctx")
                    lsl = (0, 65) if par == 0 else (65, VW)
                    m = lsl[1] - lsl[0]
                    for j in range(NKT):
                        off = max(0, j - 4) * P
                        nc.tensor.matmul(
                            cps[0:m, off:512],
                            vaug[:, j, hp, lsl[0] : lsl[1]],
                            expt[:, POFF[j] : POFF[j] + NCOL[j]],
                            start=(j == 0),
                            stop=(j == NKT - 1),
                        )
                    with nc.allow_low_precision(
                        reason="softmax denominator reciprocal to bf16"
                    ):
                        if par == 0:
                            nc.vector.reciprocal(srowE_bf[64:65], cps[64:65])
                        else:
                            nc.vector.reciprocal(srowO_bf[0:1], cps[0:1])
                    if DEBUG_TAPS and h == DBG_HEAD:
                        dbg_cps = nc.dram_tensor(
                            "dbg_cps", [P, 512], F32, kind="ExternalOutput"
                        ).ap()
                        dbg_sb = persist.tile([P, 512], F32, tag="dbgsb")
                        nc.vector.memset(dbg_sb[:], 0.0)
                        _r0, _r1 = (0, 65) if par == 0 else (64, 128)
                        nc.vector.tensor_copy(dbg_sb[_r0:_r1], cps[_r0:_r1])
                        if par == 1:
                            nc.vector.tensor_copy(dbg_sb[0:1], cps[0:1])
                        nc.sync.dma_start(out=dbg_cps[:], in_=dbg_sb[:])
                        dbg_expt = nc.dram_tensor(
                            "dbg_expt", [P, 3328], BF16, kind="ExternalOutput"
                        ).ap()
                        nc.sync.dma_start(out=dbg_expt[:], in_=expt[:])
                        dbg_vaug = nc.dram_tensor(
                            "dbg_vaug", [P, NKT, VW], BF16, kind="ExternalOutput"
                        ).ap()
                        nc.sync.dma_start(out=dbg_vaug[:], in_=vaug[:, :, hp])
                    if par == 0:
                        ctxn = tmp2.tile([P, 512], F32, tag="ctxn")
                    return (h, cps, ctxn)

                def emit_scores(h):
                    # DoubleRow over dh: contraction (ki 32, parity 2); head
                    # h lives at partition group 32*(h%4) of ot pair
                    # (2*(h//4), 2*(h//4)+1). j-tiles are computed two per
                    # 2-bank psum so each Exp covers a pair in one shot.
                    u, sub = h // 4, h % 4
                    b0 = 32 * sub
                    expt = pattn.tile([P, 3328], BF16, tag="expt")
                    for pj in range(4):
                        j0 = 2 * pj
                        w0, w1 = NCOL[j0], NCOL[j0 + 1]
                        # two independent accumulation groups must not share
                        # a PSUM bank: place the second j at a 512 offset
                        po1 = max(w0, 512)
                        ps = ps_sc.tile([P, 1024], F32, tag="sc")
                        for j, w, po in ((j0, w0, 0), (j0 + 1, w1, po1)):
                            off = 512 - w
                            nc.tensor.matmul(
                                ps[:, po : po + w],
                                kt_s[b0 : b0 + 32, 2 * u : 2 * u + 2,
                                     j * P : (j + 1) * P],
                                qt_s[b0 : b0 + 32, 2 * u : 2 * u + 2, off:512],
                                start=True,
                                stop=True,
                                perf_mode=DR,
                                tile_position=(b0, 0),
                            )
                        if po1 == w0:
                            nc.scalar.activation(
                                expt[:, POFF[j0] : POFF[j0] + w0 + w1],
                                ps[:, 0 : w0 + w1],
                                AF.Exp,
                                scale=0.125 / (WSQK * WSQK),
                            )
                        else:
                            nc.scalar.activation(
                                expt[:, POFF[j0] : POFF[j0] + w0],
                                ps[:, 0:w0],
                                AF.Exp,
                                scale=0.125 / (WSQK * WSQK),
                            )
                            nc.scalar.activation(
                                expt[:, POFF[j0 + 1] : POFF[j0 + 1] + w1],
                                ps[:, po1 : po1 + w1],
                                AF.Exp,
                                scale=0.125 / (WSQK * WSQK),
                            )
                    for j in range(4, NKT):
                        # zero the masked upper triangle of the diagonal
                        # query block post-exp (Pool, off the hot engines)
                        nc.gpsimd.affine_select(
                            out=expt[:, POFF[j] : POFF[j] + P],
                            in_=expt[:, POFF[j] : POFF[j] + P],
                            compare_op=OP.is_ge,
                            fill=0.0,
                            base=0,
                            pattern=[[1, P]],
                            channel_multiplier=-1,
                        )
                    return (h, expt)

                # filler units: Q/K projections + V-proj tiles, ordered by
                # consumption deadline, drained during the head loop
                fillers = []
                for ot in (2, 3):
                    fillers += [
                        lambda o=ot: emit_qproj(o),
                        lambda o=ot: emit_kproj(o, 0),
                        lambda o=ot: emit_kproj(o, 1),
                    ]
                fillers += [lambda t=tk: emit_vproj(0, t) for tk in range(NKT)]
                for ot in (4, 5):
                    fillers += [
                        lambda o=ot: emit_qproj(o),
                        lambda o=ot: emit_kproj(o, 0),
                        lambda o=ot: emit_kproj(o, 1),
                    ]
                fillers += [lambda t=tk: emit_vproj(1, t) for tk in range(NKT)]
                for ot in (6, 7):
                    fillers += [
                        lambda o=ot: emit_qproj(o),
                        lambda o=ot: emit_kproj(o, 0),
                        lambda o=ot: emit_kproj(o, 1),
                    ]
                fillers.reverse()  # consume via pop()

                def drain(n):
                    for _ in range(n):
                        if fillers:
                            fillers.pop()()

                def need_qk(u):
                    # scores for head group u needs both parity tiles 2u, 2u+1
                    while not ({2 * u, 2 * u + 1} <= qk_done):
                        assert fillers, f"filler queue dry before qk pair {u}"
                        fillers.pop()()

                def need_v(db):
                    while v_done[db] < NKT:
                        assert fillers, f"filler queue dry before v {db}"
                        fillers.pop()()

                # warmup: head-group-0 Q/K, then 3 heads of scores while the
                # filler queue builds V/QK state; first ctx after V db0 done
                emit_qproj(0)
                emit_kproj(0, 0)
                emit_kproj(0, 1)
                emit_qproj(1)
                emit_kproj(1, 0)
                emit_kproj(1, 1)
                from collections import deque

                prevs = deque()
                prevs.append(emit_scores(0))
                drain(3)
                prevs.append(emit_scores(1))
                nc.sync.dma_start(out=xres_s[:], in_=xres[:])
                drain(3)
                prevs.append(emit_scores(2))
                drain(4)
                need_v(0)
                nc.sync.dma_start(out=b1r_s[:], in_=b1r[:])
                nc.sync.dma_start(out=b1e_s[:], in_=b1e[:])
                for nm, _src in (
                    ("b2t", b2t),
                    ("g1t", g1t),
                    ("be1t", be1t),
                    ("g2t", g2t),
                    ("be2t", be2t),
                ):
                    t = persist.tile([P, NKT], F32, tag=nm)
                    nc.sync.dma_start(out=t[:], in_=_src[:])
                    small[nm] = t
                # prefetch the first fc1/fc2 weight chunks during attention
                # so the FFN phases never wait on the serial SP DMA queue
                for _c in range(3):
                    load_w1(_c)
                load_w2(0)
                load_w2(1)
                pending = emit_ctx(prevs.popleft())
                for h in range(3, H):
                    need_qk(h // 4)
                    prevs.append(emit_scores(h))
                    drain(1)
                    emit_post(pending)
                    nh = prevs[0][0]
                    need_v(nh // 8)
                    pending = emit_ctx(prevs.popleft())
                drain(len(fillers))
                while prevs:
                    emit_post(pending)
                    need_v(1)
                    pending = emit_ctx(prevs.popleft())
                emit_post(pending)
                flush_stats()

                if DEBUG_TAPS:
                    dbg_xres = nc.dram_tensor(
                        "dbg_xres", [P, NKT, QTOK], F32, kind="ExternalOutput"
                    ).ap()
                    nc.sync.dma_start(out=dbg_xres[:], in_=xres_s[:])
                    dbg_xt = nc.dram_tensor(
                        "dbg_xt", [P, NKT, KV], F8, kind="ExternalOutput"
                    ).ap()
                    nc.sync.dma_start(out=dbg_xt[:], in_=xt_s[:])
                    dbg_kt = nc.dram_tensor(
                        "dbg_kt", [P, NKT, KV], F8, kind="ExternalOutput"
                    ).ap()
                    nc.sync.dma_start(out=dbg_kt[:], in_=kt_s[:])
                    dbg_stats = nc.dram_tensor(
                        "dbg_stats", [P, 2, 512], F32, kind="ExternalOutput"
                    ).ap()
                    nc.sync.dma_start(out=dbg_stats[:, 0], in_=sum1_sb[:])
                    nc.sync.dma_start(out=dbg_stats[:, 1], in_=sumsq1_sb[:])
                    dbg_srow = nc.dram_tensor(
                        "dbg_srow", [P, 2, QTOK], BF16, kind="ExternalOutput"
                    ).ap()
                    nc.sync.dma_start(out=dbg_srow[:, 0], in_=srowE_bf[:])
                    nc.sync.dma_start(out=dbg_srow[:, 1], in_=srowO_bf[:])

                # ---- phase 3: LN1 (stats already accumulated) ----
                # ln1_bf (bf16) is the fc2 residual; x8/x8e are the fp8
                # hi/lo pair feeding the compensated fc1 DoubleRow passes.
                ln1_bf = lnp.tile([P, NKT, QTOK], BF16, tag="ln1")
                x8 = lnp.tile([P, NKT, QTOK], F8, tag="x8")
                x8e = lnp.tile([P, NKT, QTOK], F8, tag="x8e")
                mean1, rstd1 = ln_meanvar(sum1_sb, sumsq1_sb)
                for kt in range(NKT):
                    ln_apply(
                        xres_s[:, kt], mean1, rstd1,
                        small["g1t"], small["be1t"], ln1_bf[:, kt], kt,
                    )
                    nc.gpsimd.tensor_copy(x8[:, kt], ln1_bf[:, kt])
                    nc.vector.tensor_tensor(
                        x8e[:, kt], ln1_bf[:, kt], x8[:, kt], OP.subtract
                    )
                pxstack.close()
                attn_stack.close()

            # ---- phase 4: fc1 + selu (w1 in JIT 4-ot chunks, depth 2) ----
            pffn_stack = contextlib.ExitStack()
            pffn = pffn_stack.enter_context(tc.tile_pool(name="pffn", bufs=1))
            ps_mm = pffn_stack.enter_context(
                tc.tile_pool(name="ps_mm", bufs=3, space="PSUM")
            )
            ps_x = pffn_stack.enter_context(
                tc.tile_pool(name="ps_x", bufs=1, space="PSUM")
            )
            # deep ring for the selu temporaries: with only 2 bufs the
            # Relu of ot must wait for Pool's h8 copy of ot-2 (slot reuse),
            # putting ~1us of Pool/Act latency on the PE critical path
            pselu = pffn_stack.enter_context(tc.tile_pool(name="pselu", bufs=4))
            h8 = pffn.tile([P, NOT1, QTOK], F8, tag="h8")
            h8e = pffn.tile([P, NOT1, QTOK], F8, tag="h8e")
            # ots 0..3 run kp-major across four live psums so each matmul
            # group consumes x8/x8e kt-pairs as LN1 streams them out --
            # otherwise the first psum group alone needs the full x8 tile
            # and the PE idles through the whole LN1 quant trench
            ps03 = []
            for ot in range(4):
                w1h, w1l = w1bufs[(ot // 2) % NW1B]
                if ot == 3:
                    psi = ps_x.tile([P, 512], F32, tag="x", name=f"ps03_{ot}")
                else:
                    psi = ps_mm.tile([P, 512], F32, tag="mm", name=f"ps03_{ot}")
                ps03.append((psi, w1h, w1l))
            load_w1(3)
            load_w1(4)
            for kp in range(NKT // 2):
                for ot in range(4):
                    psi, w1h, w1l = ps03[ot]
                    for pi, xq_w in enumerate(((w1h, x8), (w1l, x8), (w1h, x8e))):
                        wt, xq = xq_w
                        nc.tensor.matmul(
                            psi[:],
                            wt[:, ot % 2, 2 * kp : 2 * kp + 2],
                            xq[:, 2 * kp : 2 * kp + 2],
                            start=(kp == 0 and pi == 0),
                            stop=(kp == NKT // 2 - 1 and pi == 2),
                            perf_mode=DR,
                        )
            for ot in range(NOT1):
                if ot < 4:
                    ps = ps03[ot][0]
                else:
                    w1h, w1l = w1bufs[(ot // 2) % NW1B]
                    if ot % 2 == 0:
                        load_w1(ot // 2 + 3)
                    if ot % 4 == 3:
                        ps = ps_x.tile([P, 512], F32, tag="x")
                    else:
                        ps = ps_mm.tile([P, 512], F32, tag="mm")
                    passes = [(w1h, x8), (w1l, x8), (w1h, x8e)]
                    for pi, (wt, xq) in enumerate(passes):
                        for kp in range(NKT // 2):
                            nc.tensor.matmul(
                                ps[:],
                                wt[:, ot % 2, 2 * kp : 2 * kp + 2],
                                xq[:, 2 * kp : 2 * kp + 2],
                                start=(pi == 0 and kp == 0),
                                stop=(pi == 2 and kp == NKT // 2 - 1),
                                perf_mode=DR,
                            )
                p_t = pselu.tile([P, QTOK], F32, tag="selup")
                nc.scalar.activation(
                    p_t[:],
                    ps[:],
                    AF.Relu,
                    scale=SELU_S / WS,
                    bias=b1r_s[:, ot : ot + 1],
                )
                e_t = pselu.tile([P, QTOK], F32, tag="selue")
                nc.scalar.activation(
                    e_t[:], ps[:], AF.Exp, scale=1.0 / WS, bias=b1e_s[:, ot : ot + 1]
                )
                # selu(z) = min(sa*e^z - sa, s*relu(z))
                nc.vector.scalar_tensor_tensor(
                    p_t[:], e_t[:], SELU_SA, p_t[:], OP.subtract, OP.min
                )
                nc.gpsimd.tensor_copy(h8[:, ot], p_t[:])
                nc.vector.tensor_tensor(h8e[:, ot], p_t[:], h8[:, ot], OP.subtract)

            # ---- phase 5: fc2 + residual + LN2 + store (full 512 width) ----
            ps_stat2 = pffn_stack.enter_context(
                tc.tile_pool(name="ps_stat2", bufs=2, space="PSUM")
            )
            res2 = pffn.tile([P, NKT, QTOK], F32, tag="res2")
            ps0_2 = ps_stat2.tile([P, 512], F32, tag="stat2")
            ps1_2 = ps_stat2.tile([P, 512], F32, tag="stat2")
            for ot in range(NKT):
                w2h, w2l = w2bufs[ot % NW2B]
                load_w2(ot + 2)  # ots 0,1 preloaded in attention
                if ot % 4 == 3:
                    ps = ps_x.tile([P, 512], F32, tag="x")
                else:
                    ps = ps_mm.tile([P, 512], F32, tag="mm")
                passes = [(w2h, h8), (w2l, h8), (w2h, h8e)]
                for pi, (wt, hq) in enumerate(passes):
                    for kp in range(NOT1 // 2):
                        nc.tensor.matmul(
                            ps[:],
                            wt[:, 2 * kp : 2 * kp + 2],
                            hq[:, 2 * kp : 2 * kp + 2],
                            start=(pi == 0 and kp == 0),
                            stop=(pi == 2 and kp == NOT1 // 2 - 1),
                            perf_mode=DR,
                        )
                t1 = tmp2.tile([P, QTOK], F32, tag="r2t")
                nc.scalar.activation(
                    t1[:],
                    ps[:],
                    AF.Identity,
                    scale=1.0 / WS,
                    bias=small["b2t"][:, ot : ot + 1],
                )
                nc.vector.tensor_tensor(
                    res2[:, ot], t1[:], ln1_bf[:, ot], OP.add
                )
                cast_t = tmp.tile([P, QTOK], BF16, tag="lncast2")
                sq_t = tmp.tile([P, QTOK], BF16, tag="lnsq2")
                nc.vector.tensor_copy(cast_t[:], res2[:, ot])
                nc.scalar.activation(sq_t[:], res2[:, ot], AF.Square)
                ln_stats_mm(ps0_2, ps1_2, cast_t, sq_t, ot)
            mean2, rstd2 = ln_meanvar(ps0_2, ps1_2)
            for kt in range(NKT):
                # spread the tail normalize across DVE and Pool so the
                # final 8-tile chain isn't serialized on one engine; the
                # scale-bias always runs on the otherwise-idle Act engine
                eng = nc.gpsimd if kt in (2, 5) else nc.vector
                ln_apply(
                    res2[:, kt], mean2, rstd2,
                    small["g2t"], small["be2t"], res2[:, kt], kt,
                    eng=eng,
                )
                nc.sync.dma_start(out=out[:, kt], in_=res2[:, kt])
            pffn_stack.close()

    _legalize_waits(nc)
    return nc


_NC_CACHE = None
TRACE = False
LAST_EXEC_NS = None


def _get_nc():
    global _NC_CACHE
    if _NC_CACHE is None:
        _NC_CACHE = _build_nc()
    return _NC_CACHE


def _tile_w(a):
    """[Din, O] -> [P, O//P(ot), Din//P(kt), P] with ot-contiguous DMA slices."""
    Din, O = a.shape
    return np.ascontiguousarray(
        a.reshape(Din // P, P, O // P, P).transpose(1, 2, 0, 3)
    )


def _pp(v, n):
    """[n*P] -> [P, n] per-partition layout."""
    return np.ascontiguousarray(v.reshape(n, P).T)


def kernel(X, wq, wk, wv, ln1_g, ln1_b, w1, b1, w2, b2, ln2_g, ln2_b):
    from concourse.bass_utils import run_bass_kernel_spmd

    X = np.asarray(X, np.float32)
    bf = ml_dtypes.bfloat16
    f8 = ml_dtypes.float8_e4m3  # IEEE flavor — matches bass float8e4

    def hilo(wt):
        hi = wt.astype(f8)
        lo = (wt - hi.astype(np.float32)).astype(f8)
        return hi, lo

    # scores-DR out-dim permutation: slot (ot, i) holds projection row
    # head*64 + dh with head = 4*(ot//2) + i//32, dh = 2*(i%32) + ot%2,
    # so head h sits at partition group 32*(h%4) of tiles (2u, 2u+1)
    # with the dh parity split across the tile pair (DoubleRow Ko dim).
    qperm = np.empty(D, np.int64)
    for _ot in range(NKT):
        for _i in range(P):
            _h = 4 * (_ot // 2) + _i // 32
            _dh = 2 * (_i % 32) + (_ot % 2)
            qperm[_ot * P + _i] = _h * 64 + _dh
    wqT = _tile_w((WSQK * np.asarray(wq, np.float32).T)[:, qperm]).astype(f8)
    wkT = _tile_w((WSQK * np.asarray(wk, np.float32).T)[:, qperm]).astype(f8)
    wvT = np.ascontiguousarray(
        WS * np.asarray(wv, np.float32).T.reshape(NKT, P, D).transpose(1, 0, 2)
    ).astype(f8)
    w1hi, w1lo = hilo(_tile_w(WS * np.asarray(w1, np.float32).T))
    w2hi, w2lo = hilo(_tile_w(WS * np.asarray(w2, np.float32).T))
    b1 = np.asarray(b1, np.float32)
    shared = dict(
        wq=wqT,
        wk=wkT,
        wv=wvT,
        w1=w1hi,
        w1e=w1lo,
        w2=w2hi,
        w2e=w2lo,
        b1r=_pp(SELU_S * b1, NOT1),
        b1e=_pp(b1 + LN_SA, NOT1),
        b2t=_pp(np.asarray(b2, np.float32), NKT),
        g1t=_pp(np.asarray(ln1_g, np.float32), NKT),
        be1t=_pp(np.asarray(ln1_b, np.float32), NKT),
        g2t=_pp(np.asarray(ln2_g, np.float32), NKT),
        be2t=_pp(np.asarray(ln2_b, np.float32), NKT),
    )

    in_maps = []
    for c in range(8):
        b, hf = c // 2, c % 2
        if hf == 1:
            xkv = X[b].T  # [D, L]
            valid = np.full(KV, WS, np.float32)
            xq = X[b, 512:]
        else:
            xkv = np.concatenate(
                [np.zeros((D, 512), np.float32), X[b, :512].T], axis=1
            )
            valid = np.concatenate(
                [np.zeros(512, np.float32), np.full(512, WS, np.float32)]
            )
            xq = X[b, :512]
        xt = (
            np.ascontiguousarray(xkv.reshape(NKT, P, KV).transpose(1, 0, 2))
        ).astype(f8)
        xres = np.ascontiguousarray(xq.T.reshape(NKT, P, QTOK).transpose(1, 0, 2))
        vt = valid.reshape(NKT, P).T  # [P, NKT]
        val16 = (
            np.repeat(vt[:, :, None], H, axis=2).reshape(P, NKT, NKT, 2).astype(bf)
        )
        m = dict(shared)
        m.update(xt=xt, xres=xres, valid16=np.ascontiguousarray(val16))
        in_maps.append(m)

    nc = _get_nc()
    global LAST_EXEC_NS
    if TRACE:
        res = run_bass_kernel_spmd(nc, in_maps, list(range(8)), trace=True)
        LAST_EXEC_NS = res.exec_time_ns
    else:
        res = run_bass_kernel_spmd(nc, in_maps, list(range(8)))

    out = np.empty((B, L, D), np.float32)
    for c in range(8):
        b, hf = c // 2, c % 2
        o = res.results[c]["out"]  # [P, NKT, QTOK]
        o = o.transpose(1, 0, 2).reshape(D, QTOK).T  # [QTOK, D]
        out[b, hf * 512 : hf * 512 + 512] = o
    return out



# revision 97
# speedup vs baseline: 1.2257x; 1.0014x over previous
"""Decoder-layer Trainium2 kernel: 8-core SPMD, single launch, no collectives.

Sharding: core c -> (batch b = c // 2, sequence-half hf = c % 2). Each core
computes the full decoder layer for 512 query tokens of one sequence.
All cores run ONE identical program over a canonical virtual sequence of
1024 kv tokens with queries at virtual positions 512..1023; first-half cores
get their 512 real tokens placed at virtual 512..1023 with zero-padded kv
prefix and a `valid` vector that zeroes the pad contribution to the softmax
denominator.

v2 changes vs baseline:
- softmax denominators ride along in the ctx matmul via an augmented V
  (per head-pair V layout [Edims|Eden|Oden|zeros63|Odims], 193 wide): even
  heads matmul M=65 -> dims at psum rows 0..63 + den at row 64; odd heads
  M=128 with a zero block -> den at row 0 + dims at rows 64..127. Kills the
  65536 rows of separate [1,512] denominator matmuls.
- ctx matmuls are causally restricted to the live query range per kv tile
  (like scores), saving another 12288 rows.
- LN1 stats matmuls run inline as each head pair finishes its xres tile.
- fc2 + LN2 run in two token-half passes so the final normalize/store of
  half 0 overlaps the fc2 matmuls of half 1.
"""

import sys

sys.path.insert(0, "/opt/trn_rl_repo")

import math

import numpy as np
import ml_dtypes

import concourse.bass as bass
import concourse.mybir as mybir
from concourse.tile import TileContext, TilePool
from concourse.vector_clock import ScopedClock

BF16 = mybir.dt.bfloat16
F8 = mybir.dt.float8e4
F32 = mybir.dt.float32
AF = mybir.ActivationFunctionType
OP = mybir.AluOpType
DR = mybir.MatmulPerfMode.DoubleRow
WS = 64.0  # fp8 weight pre-scale (wv/w1/w2)
# Q/K projections use a smaller pre-scale: bass float8e4 is IEEE e4m3
# (max finite 240, saturates to inf) and |K|*64 reaches ~290 on some
# batches; *32 keeps the fp8 Q/K copies comfortably finite.
WSQK = 32.0

B, L, D = 4, 1024, 1024
H, DH = 16, 64
DFF = 4 * D
P = 128
QTOK = 512  # query tokens per core
KV = 1024  # canonical kv length (virtual)
NKT = D // P  # 8 d-tiles
NOT1 = DFF // P  # 32 fc1 out tiles
MASK_NEG = -1.0e9
VW = 193  # augmented V width per head pair: [Ed 64|Eden|Oden|z 63|Od 64]

SELU_S = 1.0507009873554804934193349852946
SELU_A = 1.6732632423543772848170429916717
SELU_SA = SELU_S * SELU_A
LN_SA = math.log(SELU_SA)
LN_EPS = 1e-5


class PatchedTileContext(TileContext):
    """TileContext whose exit drain respects this walrus build's limit of
    ONE semaphore wait per instruction: the global-clock waits are spread
    across standalone NOPs and the butterfly barrier (whose sem-eq waits
    walrus rejects) is replaced by the NRT-expanded pseudo barrier."""

    def _drain_and_barrier(self, tick_clock, wait_clock):
        nc = self.nc
        carrier = nc.sync.nop()
        wait_clock.add_sem_waits(
            carrier.ins, ScopedClock({None: tick_clock.global_clock})
        )
        waits = list(carrier.ins.sync_info.on_wait)
        ups = list(carrier.ins.sync_info.on_update)
        if len(waits) > 1:
            carrier.ins.sync_info = mybir.SyncInfo(on_wait=[waits[0]], on_update=ups)
            for w in waits[1:]:
                extra = nc.sync.nop()
                extra.ins.sync_info = mybir.SyncInfo(on_wait=[w], on_update=[])
        for eng in nc.engines.values():
            eng.drain()
        nc._nrt_pseudo_barrier()
        popped = nc._tile_sem_poison_stack.pop()
        assert popped is self._sem_poison
        nc.clear_and_free_semaphores(list(self.sems.allocated().values()))
        nc._nrt_pseudo_barrier()


def _legalize_waits(nc):
    """This walrus build accepts at most ONE semaphore wait per instruction.
    Tile's sem-assignment can attach several; hoist the extras onto same-engine
    NOPs inserted immediately before the instruction (waits are a conjunction,
    so a sequence of single-wait stalls is equivalent)."""
    n = 0
    for fn in nc.m.functions:
        for blk in fn.blocks:
            out = []
            changed = False
            for inst in blk.instructions:
                si = getattr(inst, "sync_info", None)
                if si is not None and len(si.on_wait) > 1:
                    waits = list(si.on_wait)
                    for w in waits[:-1]:
                        nop = mybir.InstNoOp(name=f"waitnop_{n}", ins=[], outs=[])
                        n += 1
                        nop.engine = inst.engine
                        nop.sync_info = mybir.SyncInfo(on_wait=[w], on_update=[])
                        out.append(nop)
                    inst.sync_info = mybir.SyncInfo(
                        on_wait=[waits[-1]], on_update=list(si.on_update)
                    )
                    changed = True
                out.append(inst)
            if changed:
                blk.instructions = out
    return n


DEBUG_TAPS = False
DBG_HEAD = 0


def _build_nc():
    nc = bass.Bass("TRN2", target_bir_lowering=False, debug=False, num_devices=8)

    def din(name, shape, dt):
        return nc.dram_tensor(name, shape, dt, kind="ExternalInput").ap()

    xt = din("xt", [P, NKT, KV], F8)  # X[b].T tiled, virtual-padded
    xres = din("xres", [P, NKT, QTOK], F32)  # q tokens transposed, fp32
    valid16 = din("valid16", [P, NKT, NKT, 2], BF16)  # WS flag, [8hp x 2]
    wq = din("wq", [P, NKT, NKT, P], F8)  # [dpart, ot, kt, o], x WS
    wk = din("wk", [P, NKT, NKT, P], F8)
    wv = din("wv", [P, NKT, D], F8)  # rhs layout [dpart, kt, o], x WS
    w1 = din("w1", [P, NOT1, NKT, P], F8)  # fp8(WS*w1^T)
    w1e = din("w1e", [P, NOT1, NKT, P], F8)  # fp8 residual of the above
    w2 = din("w2", [P, NKT, NOT1, P], F8)
    w2e = din("w2e", [P, NKT, NOT1, P], F8)
    b1r = din("b1r", [P, NOT1], F32)  # SELU_S * b1
    b1e = din("b1e", [P, NOT1], F32)  # b1 + ln(SELU_S*SELU_A)
    b2t = din("b2t", [P, NKT], F32)
    g1t = din("g1t", [P, NKT], F32)
    be1t = din("be1t", [P, NKT], F32)
    g2t = din("g2t", [P, NKT], F32)
    be2t = din("be2t", [P, NKT], F32)
    out = nc.dram_tensor("out", [P, NKT, QTOK], F32, kind="ExternalOutput").ap()

    with PatchedTileContext(nc) as tc:
        import contextlib

        with contextlib.ExitStack() as ctx:
            persist = ctx.enter_context(tc.tile_pool(name="persist", bufs=1))
            bc = ctx.enter_context(tc.tile_pool(name="bc", bufs=1))
            wpool = ctx.enter_context(tc.tile_pool(name="wpool", bufs=4))
            tmp = ctx.enter_context(tc.tile_pool(name="tmp", bufs=2))
            tmp2 = ctx.enter_context(tc.tile_pool(name="tmp2", bufs=2))
            lnp = ctx.enter_context(tc.tile_pool(name="lnp", bufs=1))
            w1pool = ctx.enter_context(tc.tile_pool(name="w1pool", bufs=1))

            # ---- constants ----
            NW1B, NW2B = 5, 4
            w1bufs = [
                (
                    w1pool.tile(
                        [P, 2, NKT, P], F8, tag=f"w1{i}h", name=f"w1{i}h"
                    ),
                    w1pool.tile(
                        [P, 2, NKT, P], F8, tag=f"w1{i}e", name=f"w1{i}e"
                    ),
                )
                for i in range(NW1B)
            ]
            w2bufs = [
                (
                    w1pool.tile(
                        [P, NOT1, P], F8, tag=f"w2{i}h", name=f"w2{i}h"
                    ),
                    w1pool.tile(
                        [P, NOT1, P], F8, tag=f"w2{i}e", name=f"w2{i}e"
                    ),
                )
                for i in range(NW2B)
            ]

            def load_w1(chunk):
                if chunk < NOT1 // 2:
                    hb, lb = w1bufs[chunk % NW1B]
                    nc.sync.dma_start(out=hb[:], in_=w1[:, 2 * chunk : 2 * chunk + 2])
                    nc.sync.dma_start(out=lb[:], in_=w1e[:, 2 * chunk : 2 * chunk + 2])

            def load_w2(ot):
                if ot < NKT:
                    hb, lb = w2bufs[ot % NW2B]
                    nc.sync.dma_start(out=hb[:], in_=w2[:, ot])
                    nc.sync.dma_start(out=lb[:], in_=w2e[:, ot])
            ones128 = persist.tile([P, P], BF16, tag="ones128")
            nc.gpsimd.memset(ones128[:], 1.0)
            ones_r0 = persist.tile([P, P], BF16, tag="ones_r0")
            nc.gpsimd.memset(ones_r0[:], 0.0)
            nc.gpsimd.memset(ones_r0[0:1, :], 1.0)
            ones_r64 = persist.tile([P, P], BF16, tag="ones_r64")
            nc.gpsimd.memset(ones_r64[:], 0.0)
            nc.gpsimd.memset(ones_r64[64:65, :], 1.0)
            srowE_bf = persist.tile([P, QTOK], BF16, tag="srowEbf")
            nc.vector.memset(srowE_bf[:], 0.0)
            srowO_bf = persist.tile([P, QTOK], BF16, tag="srowObf")
            nc.vector.memset(srowO_bf[:], 0.0)
            eps_ap = persist.tile([P, 1], F32, tag="eps")
            nc.gpsimd.memset(eps_ap[:], LN_EPS)

            def ln_stats_mm(ps0, ps1, cast_t, sq_t, kt, n=NKT, ncols=QTOK):
                nc.tensor.matmul(
                    ps0[:, 0:ncols],
                    ones128[:],
                    cast_t[:],
                    start=(kt == 0),
                    stop=(kt == n - 1),
                )
                nc.tensor.matmul(
                    ps1[:, 0:ncols],
                    ones128[:],
                    sq_t[:],
                    start=(kt == 0),
                    stop=(kt == n - 1),
                )

            def ln_meanvar(ps0, ps1, ncols=QTOK):
                """stats psums -> (mean, rstd) broadcast tiles.

                rstd = exp(-0.5*ln(var+eps)): Ln and Exp share an Act table
                (natural_log_exp_and_others) with Relu/Identity/Square, so
                this never forces the 1.3us act-table reload that Sqrt would.
                """
                if ps1 is None:
                    # ps0 is a [P, 2*ncols] psum holding [sum | sumsq]:
                    # scale both with one DVE op into an adjacent pair
                    mv = bc.tile([P, 2 * ncols], F32, tag="meanvar")
                    nc.vector.tensor_scalar_mul(mv[:], ps0[:, 0 : 2 * ncols], 1.0 / D)
                    mean_bc, var_bc = mv[:, 0:ncols], mv[:, ncols : 2 * ncols]
                else:
                    mean_t = bc.tile([P, ncols], F32, tag="mean", name="mean_t")
                    mean_bc = mean_t[:]
                    nc.vector.tensor_scalar_mul(mean_bc, ps0[:, 0:ncols], 1.0 / D)
                    var_t = bc.tile([P, ncols], F32, tag="var", name="var_t")
                    var_bc = var_t[:]
                    nc.vector.tensor_scalar_mul(var_bc, ps1[:, 0:ncols], 1.0 / D)
                m2 = tmp2.tile([P, ncols], F32, tag="lnt")
                nc.vector.tensor_tensor(m2[:], mean_bc, mean_bc, OP.mult)
                nc.vector.tensor_tensor(var_bc, var_bc, m2[:], OP.subtract)
                nc.scalar.activation(var_bc, var_bc, AF.Ln, bias=eps_ap[:])
                nc.scalar.activation(var_bc, var_bc, AF.Exp, scale=-0.5)
                return mean_bc, var_bc

            def ln_apply(
                src_kt, mean_bc, var_bc, g_ap, b_ap, dst_kt, kt, ncols=QTOK,
                eng=None, fin_act=None,
            ):
                eng = eng or nc.vector
                t1 = tmp2.tile([P, ncols], F32, tag="lnt")
                eng.tensor_tensor(t1[:], src_kt, mean_bc[:], OP.subtract)
                eng.tensor_tensor(t1[:], t1[:], var_bc[:], OP.mult)
                if fin_act if fin_act is not None else (kt % 2 == 0):
                    nc.scalar.activation(
                        dst_kt,
                        t1[:],
                        AF.Identity,
                        scale=g_ap[:, kt : kt + 1],
                        bias=b_ap[:, kt : kt + 1],
                    )
                else:
                    nc.vector.tensor_scalar(
                        dst_kt,
                        t1[:],
                        g_ap[:, kt : kt + 1],
                        b_ap[:, kt : kt + 1],
                        OP.mult,
                        OP.add,
                    )

            # ---- phase 1+2 fused: QKV projections + attention ----
            with tc.tile_pool(name="pproj", bufs=1) as pproj:
                import contextlib as _ctl

                attn_stack = _ctl.ExitStack()
                pattn = attn_stack.enter_context(tc.tile_pool(name="pattn", bufs=3))
                ps_ctx = attn_stack.enter_context(
                    tc.tile_pool(name="ps_ctx", bufs=2, space="PSUM")
                )
                ps_sc = attn_stack.enter_context(
                    tc.tile_pool(name="ps_sc", bufs=2, space="PSUM")
                )
                ps_fill = attn_stack.enter_context(
                    tc.tile_pool(name="ps_fill", bufs=2, space="PSUM")
                )
                pxstack = _ctl.ExitStack()
                px = pxstack.enter_context(tc.tile_pool(name="px", bufs=1))
                wq_t0 = wpool.tile([P, NKT, P], F8, tag="wqkv")
                nc.sync.dma_start(out=wq_t0[:], in_=wq[:, 0])
                xt_s = px.tile([P, NKT, KV], F8, tag="xt")
                nc.sync.dma_start(out=xt_s[:, 0:4], in_=xt[:, 0:4])
                nc.sync.dma_start(out=xt_s[:, 4:8], in_=xt[:, 4:8])
                wk_t0 = wpool.tile([P, NKT, P], F8, tag="wqkv")
                nc.sync.dma_start(out=wk_t0[:], in_=wk[:, 0])
                # Q/K in fp8, scores-DR grouped layout: tile ot = (u, c)
                # with u = ot//2 (head group 4u..4u+3), c = ot%2 (dh parity);
                # partition 32*g+ki holds head 4u+g, dh = 2*ki + c.
                qt_s = pproj.tile([P, NKT, QTOK], F8, tag="qt")
                kt_s = pproj.tile([P, NKT, KV], F8, tag="kt")
                # augmented V: per (kv-tile j, head pair hp) 193 cols:
                # [Edims 64 | Eden 1 | Oden 1 | zeros 63 | Odims 64]
                vaug = pproj.tile([P, NKT, NKT, VW], BF16, tag="vaug")
                nc.gpsimd.memset(vaug[:, :, :, 66:129], 0.0)

                wv_s = px.tile([P, NKT, D], F8, tag="wv")
                nc.sync.dma_start(out=wv_s[:], in_=wv[:])
                val_s = pproj.tile([P, NKT, NKT, 2], BF16, tag="val")
                nc.sync.dma_start(out=val_s[:], in_=valid16[:])
                xres_s = pproj.tile([P, NKT, QTOK], F32, tag="xres")
                b1r_s = persist.tile([P, NOT1], F32, tag="b1r")
                b1e_s = persist.tile([P, NOT1], F32, tag="b1e")
                small = {}
                sum1_sb = pproj.tile([P, 512], F32, tag="sum1")
                sumsq1_sb = pproj.tile([P, 512], F32, tag="sumsq1")

                qk_done = set()
                v_done = {0: 0, 1: 0}

                def emit_qproj(ot):
                    if ot == 0:
                        wq_t = wq_t0
                    else:
                        wq_t = wpool.tile([P, NKT, P], F8, tag="wqkv")
                        nc.sync.dma_start(out=wq_t[:], in_=wq[:, ot])
                    ps = ps_fill.tile([P, 512], F32, tag="fill")
                    for kp in range(NKT // 2):
                        nc.tensor.matmul(
                            ps[:],
                            wq_t[:, 2 * kp : 2 * kp + 2],
                            xt_s[:, 2 * kp : 2 * kp + 2, 512:1024],
                            start=(kp == 0),
                            stop=(kp == NKT // 2 - 1),
                            perf_mode=DR,
                        )
                    nc.vector.tensor_copy(qt_s[:, ot], ps[:])

                wk_ts = {0: wk_t0}

                def emit_kproj(ot, tb):
                    if tb == 0 and ot not in wk_ts:
                        wk_t = wpool.tile([P, NKT, P], F8, tag="wqkv")
                        nc.sync.dma_start(out=wk_t[:], in_=wk[:, ot])
                        wk_ts[ot] = wk_t
                    wk_t = wk_ts[ot]
                    ps = ps_fill.tile([P, 512], F32, tag="fill")
                    for kp in range(NKT // 2):
                        nc.tensor.matmul(
                            ps[:],
                            wk_t[:, 2 * kp : 2 * kp + 2],
                            xt_s[:, 2 * kp : 2 * kp + 2, tb * 512 : (tb + 1) * 512],
                            start=(kp == 0),
                            stop=(kp == NKT // 2 - 1),
                            perf_mode=DR,
                        )
                    if tb == 0:
                        nc.scalar.copy(kt_s[:, ot, 0:512], ps[:])
                    else:
                        nc.vector.tensor_copy(kt_s[:, ot, 512:1024], ps[:])
                        qk_done.add(ot)

                def emit_vproj(db, tk):
                    if db == 0:
                        # den columns for all 8 pairs x 2 parities
                        nc.vector.tensor_copy(vaug[:, tk, :, 64:66], val_s[:, tk])
                    ps = ps_fill.tile([P, 4, P], F32, tag="fill")
                    for kp in range(NKT // 2):
                        nc.tensor.matmul(
                            ps[:, :, :],
                            xt_s[:, 2 * kp : 2 * kp + 2, tk * P : (tk + 1) * P],
                            wv_s[:, 2 * kp : 2 * kp + 2, db * 512 : (db + 1) * 512],
                            start=(kp == 0),
                            stop=(kp == NKT // 2 - 1),
                            perf_mode=DR,
                        )
                    hp0 = db * 4
                    nc.vector.tensor_copy(
                        vaug[:, tk, hp0 : hp0 + 4, 0:64], ps[:, :, 0:64]
                    )
                    nc.vector.tensor_copy(
                        vaug[:, tk, hp0 : hp0 + 4, 129:193], ps[:, :, 64:128]
                    )
                    v_done[db] += 1

                stats_pending = []

                def flush_stats():
                    while stats_pending:
                        cast_t, sq_t, hp = stats_pending.pop(0)
                        pss = ps_sc.tile([P, 1024], F32, tag="sc")
                        nc.tensor.matmul(
                            pss[:, 0:512], ones128[:], cast_t[:], start=True, stop=True
                        )
                        nc.tensor.matmul(
                            pss[:, 512:1024], ones128[:], sq_t[:], start=True, stop=True
                        )
                        if hp == 0:
                            nc.vector.tensor_copy(sum1_sb[:], pss[:, 0:512])
                            nc.vector.tensor_copy(sumsq1_sb[:], pss[:, 512:1024])
                        else:
                            nc.vector.tensor_tensor(
                                sum1_sb[:], sum1_sb[:], pss[:, 0:512], OP.add
                            )
                            nc.vector.tensor_tensor(
                                sumsq1_sb[:], sumsq1_sb[:], pss[:, 512:1024], OP.add
                            )

                def emit_post(p):
                    h, cps, ctxn = p
                    hp, par = h // 2, h % 2
                    po = 64 * par
                    flush_stats()
                    # broadcast the bf16 reciprocal row across the 64 ctx
                    # partitions via a ones-matmul (bcp shares the fill pool)
                    bcp = ps_fill.tile([P, 512], F32, tag="fill")
                    if par == 0:
                        nc.tensor.matmul(
                            bcp[0:64],
                            ones_r64[:, 0:64],
                            srowE_bf[:],
                            start=True,
                            stop=True,
                        )
                    else:
                        nc.tensor.matmul(
                            bcp[64:128],
                            ones_r0[:, 0:64],
                            srowO_bf[:],
                            start=True,
                            stop=True,
                        )
                    bc_sb = tmp2.tile([P, 512], BF16, tag="bcsb")
                    nc.vector.tensor_copy(bc_sb[po : po + 64], bcp[po : po + 64])
                    nc.vector.tensor_tensor(
                        ctxn[po : po + 64],
                        cps[po : po + 64],
                        bc_sb[po : po + 64],
                        OP.mult,
                    )
                    if par == 1:
                        # pair finished: residual add + LN1 stats (Pool
                        # takes the add + cast + square; PE the stats,
                        # deferred one slot so PE never waits on Pool).
                        # Last pair runs on DVE: Pool's 0.42 efficiency
                        # would sit on the attention->LN1 critical path.
                        eng = nc.vector if hp == H // 2 - 1 else nc.gpsimd
                        eng.tensor_add(
                            xres_s[:, hp], xres_s[:, hp], ctxn[:]
                        )
                        cast_t = tmp.tile([P, 512], BF16, tag="lncast")
                        sq_t = tmp.tile([P, 512], BF16, tag="lnsq")
                        eng.tensor_copy(cast_t[:], xres_s[:, hp])
                        eng.tensor_mul(
                            sq_t[:], xres_s[:, hp], xres_s[:, hp]
                        )
                        stats_pending.append((cast_t, sq_t, hp))

                ctxn = None
                # packed expt: per-j live query range [off_j, 512) stored
                # contiguously; POFF[j] is the packed start, NCOL[j] the width
                NCOL = [512 - max(0, j - 4) * P for j in range(NKT)]
                POFF = [0] * NKT
                for j in range(1, NKT):
                    POFF[j] = POFF[j - 1] + NCOL[j - 1]

                def emit_ctx(pr):
                    nonlocal ctxn
                    h, expt = pr
                    hp, par = h // 2, h % 2
                    cps = ps_ctx.tile([P, 512], F32, tag="ctx")
                    lsl = (0, 65) if par == 0 else (65, VW)
                    m = lsl[1] - lsl[0]
                    for j in range(NKT):
                        off = max(0, j - 4) * P
                        nc.tensor.matmul(
                            cps[0:m, off:512],
                            vaug[:, j, hp, lsl[0] : lsl[1]],
                            expt[:, POFF[j] : POFF[j] + NCOL[j]],
                            start=(j == 0),
                            stop=(j == NKT - 1),
                        )
                    with nc.allow_low_precision(
                        reason="softmax denominator reciprocal to bf16"
                    ):
                        if par == 0:
                            nc.vector.reciprocal(srowE_bf[64:65], cps[64:65])
                        else:
                            nc.vector.reciprocal(srowO_bf[0:1], cps[0:1])
                    if DEBUG_TAPS and h == DBG_HEAD:
                        dbg_cps = nc.dram_tensor(
                            "dbg_cps", [P, 512], F32, kind="ExternalOutput"
                        ).ap()
                        dbg_sb = persist.tile([P, 512], F32, tag="dbgsb")
                        nc.vector.memset(dbg_sb[:], 0.0)
                        _r0, _r1 = (0, 65) if par == 0 else (64, 128)
                        nc.vector.tensor_copy(dbg_sb[_r0:_r1], cps[_r0:_r1])
                        if par == 1:
                            nc.vector.tensor_copy(dbg_sb[0:1], cps[0:1])
                        nc.sync.dma_start(out=dbg_cps[:], in_=dbg_sb[:])
                        dbg_expt = nc.dram_tensor(
                            "dbg_expt", [P, 3328], BF16, kind="ExternalOutput"
                        ).ap()
                        nc.sync.dma_start(out=dbg_expt[:], in_=expt[:])
                        dbg_vaug = nc.dram_tensor(
                            "dbg_vaug", [P, NKT, VW], BF16, kind="ExternalOutput"
                        ).ap()
                        nc.sync.dma_start(out=dbg_vaug[:], in_=vaug[:, :, hp])
                    if par == 0:
                        ctxn = tmp2.tile([P, 512], F32, tag="ctxn")
                    return (h, cps, ctxn)

                def emit_scores(h):
                    # DoubleRow over dh: contraction (ki 32, parity 2); head
                    # h lives at partition group 32*(h%4) of ot pair
                    # (2*(h//4), 2*(h//4)+1). j-tiles are computed two per
                    # 2-bank psum so each Exp covers a pair in one shot.
                    u, sub = h // 4, h % 4
                    b0 = 32 * sub
                    expt = pattn.tile([P, 3328], BF16, tag="expt")
                    for pj in range(4):
                        j0 = 2 * pj
                        w0, w1 = NCOL[j0], NCOL[j0 + 1]
                        # two independent accumulation groups must not share
                        # a PSUM bank: place the second j at a 512 offset
                        po1 = max(w0, 512)
                        ps = ps_sc.tile([P, 1024], F32, tag="sc")
                        for j, w, po in ((j0, w0, 0), (j0 + 1, w1, po1)):
                            off = 512 - w
                            nc.tensor.matmul(
                                ps[:, po : po + w],
                                kt_s[b0 : b0 + 32, 2 * u : 2 * u + 2,
                                     j * P : (j + 1) * P],
                                qt_s[b0 : b0 + 32, 2 * u : 2 * u + 2, off:512],
                                start=True,
                                stop=True,
                                perf_mode=DR,
                                tile_position=(b0, 0),
                            )
                        if po1 == w0:
                            nc.scalar.activation(
                                expt[:, POFF[j0] : POFF[j0] + w0 + w1],
                                ps[:, 0 : w0 + w1],
                                AF.Exp,
                                scale=0.125 / (WSQK * WSQK),
                            )
                        else:
                            nc.scalar.activation(
                                expt[:, POFF[j0] : POFF[j0] + w0],
                                ps[:, 0:w0],
                                AF.Exp,
                                scale=0.125 / (WSQK * WSQK),
                            )
                            nc.scalar.activation(
                                expt[:, POFF[j0 + 1] : POFF[j0 + 1] + w1],
                                ps[:, po1 : po1 + w1],
                                AF.Exp,
                                scale=0.125 / (WSQK * WSQK),
                            )
                    for j in range(4, NKT):
                        # zero the masked upper triangle of the diagonal
                        # query block post-exp (Pool, off the hot engines)
                        nc.gpsimd.affine_select(
                            out=expt[:, POFF[j] : POFF[j] + P],
                            in_=expt[:, POFF[j] : POFF[j] + P],
                            compare_op=OP.is_ge,
                            fill=0.0,
                            base=0,
                            pattern=[[1, P]],
                            channel_multiplier=-1,
                        )
                    return (h, expt)

                # filler units: Q/K projections + V-proj tiles, ordered by
                # consumption deadline, drained during the head loop
                fillers = []
                for ot in (2, 3):
                    fillers += [
                        lambda o=ot: emit_qproj(o),
                        lambda o=ot: emit_kproj(o, 0),
                        lambda o=ot: emit_kproj(o, 1),
                    ]
                fillers += [lambda t=tk: emit_vproj(0, t) for tk in range(NKT)]
                for ot in (4, 5):
                    fillers += [
                        lambda o=ot: emit_qproj(o),
                        lambda o=ot: emit_kproj(o, 0),
                        lambda o=ot: emit_kproj(o, 1),
                    ]
                fillers += [lambda t=tk: emit_vproj(1, t) for tk in range(NKT)]
                for ot in (6, 7):
                    fillers += [
                        lambda o=ot: emit_qproj(o),
                        lambda o=ot: emit_kproj(o, 0),
                        lambda o=ot: emit_kproj(o, 1),
                    ]
                fillers.reverse()  # consume via pop()

                def drain(n):
                    for _ in range(n):
                        if fillers:
                            fillers.pop()()

                def need_qk(u):
                    # scores for head group u needs both parity tiles 2u, 2u+1
                    while not ({2 * u, 2 * u + 1} <= qk_done):
                        assert fillers, f"filler queue dry before qk pair {u}"
                        fillers.pop()()

                def need_v(db):
                    while v_done[db] < NKT:
                        assert fillers, f"filler queue dry before v {db}"
                        fillers.pop()()

                # warmup: head-group-0 Q/K, then 3 heads of scores while the
                # filler queue builds V/QK state; first ctx after V db0 done
                emit_qproj(0)
                emit_kproj(0, 0)
                emit_kproj(0, 1)
                emit_qproj(1)
                emit_kproj(1, 0)
                emit_kproj(1, 1)
                from collections import deque

                prevs = deque()
                prevs.append(emit_scores(0))
                drain(3)
                prevs.append(emit_scores(1))
                nc.sync.dma_start(out=xres_s[:], in_=xres[:])
                drain(3)
                prevs.append(emit_scores(2))
                drain(4)
                need_v(0)
                nc.sync.dma_start(out=b1r_s[:], in_=b1r[:])
                nc.sync.dma_start(out=b1e_s[:], in_=b1e[:])
                for nm, _src in (
                    ("b2t", b2t),
                    ("g1t", g1t),
                    ("be1t", be1t),
                    ("g2t", g2t),
                    ("be2t", be2t),
                ):
                    t = persist.tile([P, NKT], F32, tag=nm)
                    nc.sync.dma_start(out=t[:], in_=_src[:])
                    small[nm] = t
                # prefetch the first fc1/fc2 weight chunks during attention
                # so the FFN phases never wait on the serial SP DMA queue
                for _c in range(3):
                    load_w1(_c)
                load_w2(0)
                load_w2(1)
                pending = emit_ctx(prevs.popleft())
                for h in range(3, H):
                    need_qk(h // 4)
                    prevs.append(emit_scores(h))
                    drain(1)
                    emit_post(pending)
                    nh = prevs[0][0]
                    need_v(nh // 8)
                    pending = emit_ctx(prevs.popleft())
                drain(len(fillers))
                while prevs:
                    emit_post(pending)
                    need_v(1)
                    pending = emit_ctx(prevs.popleft())
                emit_post(pending)
                flush_stats()

                if DEBUG_TAPS:
                    dbg_xres = nc.dram_tensor(
                        "dbg_xres", [P, NKT, QTOK], F32, kind="ExternalOutput"
                    ).ap()
                    nc.sync.dma_start(out=dbg_xres[:], in_=xres_s[:])
                    dbg_xt = nc.dram_tensor(
                        "dbg_xt", [P, NKT, KV], F8, kind="ExternalOutput"
                    ).ap()
                    nc.sync.dma_start(out=dbg_xt[:], in_=xt_s[:])
                    dbg_kt = nc.dram_tensor(
                        "dbg_kt", [P, NKT, KV], F8, kind="ExternalOutput"
                    ).ap()
                    nc.sync.dma_start(out=dbg_kt[:], in_=kt_s[:])
                    dbg_stats = nc.dram_tensor(
                        "dbg_stats", [P, 2, 512], F32, kind="ExternalOutput"
                    ).ap()
                    nc.sync.dma_start(out=dbg_stats[:, 0], in_=sum1_sb[:])
                    nc.sync.dma_start(out=dbg_stats[:, 1], in_=sumsq1_sb[:])
                    dbg_srow = nc.dram_tensor(
                        "dbg_srow", [P, 2, QTOK], BF16, kind="ExternalOutput"
                    ).ap()
                    nc.sync.dma_start(out=dbg_srow[:, 0], in_=srowE_bf[:])
                    nc.sync.dma_start(out=dbg_srow[:, 1], in_=srowO_bf[:])

                # ---- phase 3: LN1 (stats already accumulated) ----
                # ln1_bf (bf16) is the fc2 residual; x8/x8e are the fp8
                # hi/lo pair feeding the compensated fc1 DoubleRow passes.
                ln1_bf = lnp.tile([P, NKT, QTOK], BF16, tag="ln1")
                x8 = lnp.tile([P, NKT, QTOK], F8, tag="x8")
                x8e = lnp.tile([P, NKT, QTOK], F8, tag="x8e")
                mean1, rstd1 = ln_meanvar(sum1_sb, sumsq1_sb)
                for kt in range(NKT):
                    ln_apply(
                        xres_s[:, kt], mean1, rstd1,
                        small["g1t"], small["be1t"], ln1_bf[:, kt], kt,
                    )
                    nc.gpsimd.tensor_copy(x8[:, kt], ln1_bf[:, kt])
                    nc.vector.tensor_tensor(
                        x8e[:, kt], ln1_bf[:, kt], x8[:, kt], OP.subtract
                    )
                pxstack.close()
                attn_stack.close()

            # ---- phase 4: fc1 + selu (w1 in JIT 4-ot chunks, depth 2) ----
            pffn_stack = contextlib.ExitStack()
            pffn = pffn_stack.enter_context(tc.tile_pool(name="pffn", bufs=1))
            ps_mm = pffn_stack.enter_context(
                tc.tile_pool(name="ps_mm", bufs=3, space="PSUM")
            )
            ps_x = pffn_stack.enter_context(
                tc.tile_pool(name="ps_x", bufs=1, space="PSUM")
            )
            # deep ring for the selu temporaries: with only 2 bufs the
            # Relu of ot must wait for Pool's h8 copy of ot-2 (slot reuse),
            # putting ~1us of Pool/Act latency on the PE critical path
            pselu = pffn_stack.enter_context(tc.tile_pool(name="pselu", bufs=4))
            h8 = pffn.tile([P, NOT1, QTOK], F8, tag="h8")
            h8e = pffn.tile([P, NOT1, QTOK], F8, tag="h8e")
            # ots 0..3 run kp-major across four live psums so each matmul
            # group consumes x8/x8e kt-pairs as LN1 streams them out --
            # otherwise the first psum group alone needs the full x8 tile
            # and the PE idles through the whole LN1 quant trench
            ps03 = []
            for ot in range(4):
                w1h, w1l = w1bufs[(ot // 2) % NW1B]
                if ot == 3:
                    psi = ps_x.tile([P, 512], F32, tag="x", name=f"ps03_{ot}")
                else:
                    psi = ps_mm.tile([P, 512], F32, tag="mm", name=f"ps03_{ot}")
                ps03.append((psi, w1h, w1l))
            load_w1(3)
            load_w1(4)
            for kp in range(NKT // 2):
                for ot in range(4):
                    psi, w1h, w1l = ps03[ot]
                    for pi, xq_w in enumerate(((w1h, x8), (w1l, x8), (w1h, x8e))):
                        wt, xq = xq_w
                        nc.tensor.matmul(
                            psi[:],
                            wt[:, ot % 2, 2 * kp : 2 * kp + 2],
                            xq[:, 2 * kp : 2 * kp + 2],
                            start=(kp == 0 and pi == 0),
                            stop=(kp == NKT // 2 - 1 and pi == 2),
                            perf_mode=DR,
                        )
            for ot in range(NOT1):
                if ot < 4:
                    ps = ps03[ot][0]
                else:
                    w1h, w1l = w1bufs[(ot // 2) % NW1B]
                    if ot % 2 == 0:
                        load_w1(ot // 2 + 3)
                    if ot % 4 == 3:
                        ps = ps_x.tile([P, 512], F32, tag="x")
                    else:
                        ps = ps_mm.tile([P, 512], F32, tag="mm")
                    passes = [(w1h, x8), (w1l, x8), (w1h, x8e)]
                    for pi, (wt, xq) in enumerate(passes):
                        for kp in range(NKT // 2):
                            nc.tensor.matmul(
                                ps[:],
                                wt[:, ot % 2, 2 * kp : 2 * kp + 2],
                                xq[:, 2 * kp : 2 * kp + 2],
                                start=(pi == 0 and kp == 0),
                                stop=(pi == 2 and kp == NKT // 2 - 1),
                                perf_mode=DR,
                            )
                p_t = pselu.tile([P, QTOK], F32, tag="selup")
                nc.scalar.activation(
                    p_t[:],
                    ps[:],
                    AF.Relu,
                    scale=SELU_S / WS,
                    bias=b1r_s[:, ot : ot + 1],
                )
                e_t = pselu.tile([P, QTOK], F32, tag="selue")
                nc.scalar.activation(
                    e_t[:], ps[:], AF.Exp, scale=1.0 / WS, bias=b1e_s[:, ot : ot + 1]
                )
                # selu(z) = min(sa*e^z - sa, s*relu(z))
                nc.vector.scalar_tensor_tensor(
                    p_t[:], e_t[:], SELU_SA, p_t[:], OP.subtract, OP.min
                )
                nc.gpsimd.tensor_copy(h8[:, ot], p_t[:])
                nc.vector.tensor_tensor(h8e[:, ot], p_t[:], h8[:, ot], OP.subtract)

            # ---- phase 5: fc2 + residual + LN2 + store (full 512 width) ----
            ps_stat2 = pffn_stack.enter_context(
                tc.tile_pool(name="ps_stat2", bufs=2, space="PSUM")
            )
            res2 = pffn.tile([P, NKT, QTOK], F32, tag="res2")
            ps0_2 = ps_stat2.tile([P, 512], F32, tag="stat2")
            ps1_2 = ps_stat2.tile([P, 512], F32, tag="stat2")
            for ot in range(NKT):
                w2h, w2l = w2bufs[ot % NW2B]
                load_w2(ot + 2)  # ots 0,1 preloaded in attention
                if ot % 4 == 3:
                    ps = ps_x.tile([P, 512], F32, tag="x")
                else:
                    ps = ps_mm.tile([P, 512], F32, tag="mm")
                passes = [(w2h, h8), (w2l, h8), (w2h, h8e)]
                for pi, (wt, hq) in enumerate(passes):
                    for kp in range(NOT1 // 2):
                        nc.tensor.matmul(
                            ps[:],
                            wt[:, 2 * kp : 2 * kp + 2],
                            hq[:, 2 * kp : 2 * kp + 2],
                            start=(pi == 0 and kp == 0),
                            stop=(pi == 2 and kp == NOT1 // 2 - 1),
                            perf_mode=DR,
                        )
                t1 = tmp2.tile([P, QTOK], F32, tag="r2t")
                nc.scalar.activation(
                    t1[:],
                    ps[:],
                    AF.Identity,
                    scale=1.0 / WS,
                    bias=small["b2t"][:, ot : ot + 1],
                )
                nc.vector.tensor_tensor(
                    res2[:, ot], t1[:], ln1_bf[:, ot], OP.add
                )
                cast_t = tmp.tile([P, QTOK], BF16, tag="lncast2")
                sq_t = tmp.tile([P, QTOK], BF16, tag="lnsq2")
                nc.vector.tensor_copy(cast_t[:], res2[:, ot])
                nc.scalar.activation(sq_t[:], res2[:, ot], AF.Square)
                ln_stats_mm(ps0_2, ps1_2, cast_t, sq_t, ot)
            mean2, rstd2 = ln_meanvar(ps0_2, ps1_2)
            for kt in range(NKT):
                # spread the tail normalize across DVE and Pool so the
                # final 8-tile chain isn't serialized on one engine; the
                # scale-bias always runs on the otherwise-idle Act engine
                eng = nc.gpsimd if kt in (2, 5) else nc.vector
                ln_apply(
                    res2[:, kt], mean2, rstd2,
                    small["g2t"], small["be2t"], res2[:, kt], kt,
                    eng=eng,
                )
                nc.sync.dma_start(out=out[:, kt], in_=res2[:, kt])
            pffn_stack.close()

    _legalize_waits(nc)
    return nc


_NC_CACHE = None
TRACE = False
LAST_EXEC_NS = None


def _get_nc():
    global _NC_CACHE
    if _NC_CACHE is None:
        _NC_CACHE = _build_nc()
    return _NC_CACHE


def _tile_w(a):
    """[Din, O] -> [P, O//P(ot), Din//P(kt), P] with ot-contiguous DMA slices."""
    Din, O = a.shape
    return np.ascontiguousarray(
        a.reshape(Din // P, P, O // P, P).transpose(1, 2, 0, 3)
    )


def _pp(v, n):
    """[n*P] -> [P, n] per-partition layout."""
    return np.ascontiguousarray(v.reshape(n, P).T)


def kernel(X, wq, wk, wv, ln1_g, ln1_b, w1, b1, w2, b2, ln2_g, ln2_b):
    from concourse.bass_utils import run_bass_kernel_spmd

    X = np.asarray(X, np.float32)
    bf = ml_dtypes.bfloat16
    f8 = ml_dtypes.float8_e4m3  # IEEE flavor — matches bass float8e4

    def hilo(wt):
        hi = wt.astype(f8)
        lo = (wt - hi.astype(np.float32)).astype(f8)
        return hi, lo

    # scores-DR out-dim permutation: slot (ot, i) holds projection row
    # head*64 + dh with head = 4*(ot//2) + i//32, dh = 2*(i%32) + ot%2,
    # so head h sits at partition group 32*(h%4) of tiles (2u, 2u+1)
    # with the dh parity split across the tile pair (DoubleRow Ko dim).
    qperm = np.empty(D, np.int64)
    for _ot in range(NKT):
        for _i in range(P):
            _h = 4 * (_ot // 2) + _i // 32
            _dh = 2 * (_i % 32) + (_ot % 2)
            qperm[_ot * P + _i] = _h * 64 + _dh
    wqT = _tile_w((WSQK * np.asarray(wq, np.float32).T)[:, qperm]).astype(f8)
    wkT = _tile_w((WSQK * np.asarray(wk, np.float32).T)[:, qperm]).astype(f8)
    wvT = np.ascontiguousarray(
        WS * np.asarray(wv, np.float32).T.reshape(NKT, P, D).transpose(1, 0, 2)
    ).astype(f8)
    w1hi, w1lo = hilo(_tile_w(WS * np.asarray(w1, np.float32).T))
    w2hi, w2lo = hilo(_tile_w(WS * np.asarray(w2, np.float32).T))
    b1 = np.asarray(b1, np.float32)
    shared = dict(
        wq=wqT,
        wk=wkT,
        wv=wvT,
        w1=w1hi,
        w1e=w1lo,
        w2=w2hi,
        w2e=w2lo,
        b1r=_pp(SELU_S * b1, NOT1),
        b1e=_pp(b1 + LN_SA, NOT1),
        b2t=_pp(np.asarray(b2, np.float32), NKT),
        g1t=_pp(np.asarray(ln1_g, np.float32), NKT),
        be1t=_pp(np.asarray(ln1_b, np.float32), NKT),
        g2t=_pp(np.asarray(ln2_g, np.float32), NKT),
        be2t=_pp(np.asarray(ln2_b, np.float32), NKT),
    )

    in_maps = []
    for c in range(8):
        b, hf = c // 2, c % 2
        if hf == 1:
            xkv = X[b].T  # [D, L]
            valid = np.full(KV, WS, np.float32)
            xq = X[b, 512:]
        else:
            xkv = np.concatenate(
                [np.zeros((D, 512), np.float32), X[b, :512].T], axis=1
            )
            valid = np.concatenate(
                [np.zeros(512, np.float32), np.full(512, WS, np.float32)]
            )
            xq = X[b, :512]
        xt = (
            np.ascontiguousarray(xkv.reshape(NKT, P, KV).transpose(1, 0, 2))
        ).astype(f8)
        xres = np.ascontiguousarray(xq.T.reshape(NKT, P, QTOK).transpose(1, 0, 2))
        vt = valid.reshape(NKT, P).T  # [P, NKT]
        val16 = (
            np.repeat(vt[:, :, None], H, axis=2).reshape(P, NKT, NKT, 2).astype(bf)
        )
        m = dict(shared)
        m.update(xt=xt, xres=xres, valid16=np.ascontiguousarray(val16))
        in_maps.append(m)

    nc = _get_nc()
    global LAST_EXEC_NS
    if TRACE:
        res = run_bass_kernel_spmd(nc, in_maps, list(range(8)), trace=True)
        LAST_EXEC_NS = res.exec_time_ns
    else:
        res = run_bass_kernel_spmd(nc, in_maps, list(range(8)))

    out = np.empty((B, L, D), np.float32)
    for c in range(8):
        b, hf = c // 2, c % 2
        o = res.results[c]["out"]  # [P, NKT, QTOK]
        o = o.transpose(1, 0, 2).reshape(D, QTOK).T  # [QTOK, D]
        out[b, hf * 512 : hf * 512 + 512] = o
    return out



# revision 104
# speedup vs baseline: 1.2271x; 1.0011x over previous
"""Decoder-layer Trainium2 kernel: 8-core SPMD, single launch, no collectives.

Sharding: core c -> (batch b = c // 2, sequence-half hf = c % 2). Each core
computes the full decoder layer for 512 query tokens of one sequence.
All cores run ONE identical program over a canonical virtual sequence of
1024 kv tokens with queries at virtual positions 512..1023; first-half cores
get their 512 real tokens placed at virtual 512..1023 with zero-padded kv
prefix and a `valid` vector that zeroes the pad contribution to the softmax
denominator.

v2 changes vs baseline:
- softmax denominators ride along in the ctx matmul via an augmented V
  (per head-pair V layout [Edims|Eden|Oden|zeros63|Odims], 193 wide): even
  heads matmul M=65 -> dims at psum rows 0..63 + den at row 64; odd heads
  M=128 with a zero block -> den at row 0 + dims at rows 64..127. Kills the
  65536 rows of separate [1,512] denominator matmuls.
- ctx matmuls are causally restricted to the live query range per kv tile
  (like scores), saving another 12288 rows.
- LN1 stats matmuls run inline as each head pair finishes its xres tile.
- fc2 + LN2 run in two token-half passes so the final normalize/store of
  half 0 overlaps the fc2 matmuls of half 1.
"""

import sys

sys.path.insert(0, "/opt/trn_rl_repo")

import math

import numpy as np
import ml_dtypes

import concourse.bass as bass
import concourse.mybir as mybir
from concourse.tile import TileContext, TilePool
from concourse.vector_clock import ScopedClock

BF16 = mybir.dt.bfloat16
F8 = mybir.dt.float8e4
F32 = mybir.dt.float32
AF = mybir.ActivationFunctionType
OP = mybir.AluOpType
DR = mybir.MatmulPerfMode.DoubleRow
WS = 64.0  # fp8 weight pre-scale (wv/w1/w2)
# Q/K projections use a smaller pre-scale: bass float8e4 is IEEE e4m3
# (max finite 240, saturates to inf) and |K|*64 reaches ~290 on some
# batches; *32 keeps the fp8 Q/K copies comfortably finite.
WSQK = 32.0

B, L, D = 4, 1024, 1024
H, DH = 16, 64
DFF = 4 * D
P = 128
QTOK = 512  # query tokens per core
KV = 1024  # canonical kv length (virtual)
NKT = D // P  # 8 d-tiles
NOT1 = DFF // P  # 32 fc1 out tiles
MASK_NEG = -1.0e9
VW = 193  # augmented V width per head pair: [Ed 64|Eden|Oden|z 63|Od 64]

SELU_S = 1.0507009873554804934193349852946
SELU_A = 1.6732632423543772848170429916717
SELU_SA = SELU_S * SELU_A
LN_SA = math.log(SELU_SA)
LN_EPS = 1e-5


class PatchedTileContext(TileContext):
    """TileContext whose exit drain respects this walrus build's limit of
    ONE semaphore wait per instruction: the global-clock waits are spread
    across standalone NOPs and the butterfly barrier (whose sem-eq waits
    walrus rejects) is replaced by the NRT-expanded pseudo barrier."""

    def _drain_and_barrier(self, tick_clock, wait_clock):
        nc = self.nc
        carrier = nc.sync.nop()
        wait_clock.add_sem_waits(
            carrier.ins, ScopedClock({None: tick_clock.global_clock})
        )
        waits = list(carrier.ins.sync_info.on_wait)
        ups = list(carrier.ins.sync_info.on_update)
        if len(waits) > 1:
            carrier.ins.sync_info = mybir.SyncInfo(on_wait=[waits[0]], on_update=ups)
            for w in waits[1:]:
                extra = nc.sync.nop()
                extra.ins.sync_info = mybir.SyncInfo(on_wait=[w], on_update=[])
        for eng in nc.engines.values():
            eng.drain()
        nc._nrt_pseudo_barrier()
        popped = nc._tile_sem_poison_stack.pop()
        assert popped is self._sem_poison
        nc.clear_and_free_semaphores(list(self.sems.allocated().values()))
        nc._nrt_pseudo_barrier()


def _legalize_waits(nc):
    """This walrus build accepts at most ONE semaphore wait per instruction.
    Tile's sem-assignment can attach several; hoist the extras onto same-engine
    NOPs inserted immediately before the instruction (waits are a conjunction,
    so a sequence of single-wait stalls is equivalent)."""
    n = 0
    for fn in nc.m.functions:
        for blk in fn.blocks:
            out = []
            changed = False
            for inst in blk.instructions:
                si = getattr(inst, "sync_info", None)
                if si is not None and len(si.on_wait) > 1:
                    waits = list(si.on_wait)
                    for w in waits[:-1]:
                        nop = mybir.InstNoOp(name=f"waitnop_{n}", ins=[], outs=[])
                        n += 1
                        nop.engine = inst.engine
                        nop.sync_info = mybir.SyncInfo(on_wait=[w], on_update=[])
                        out.append(nop)
                    inst.sync_info = mybir.SyncInfo(
                        on_wait=[waits[-1]], on_update=list(si.on_update)
                    )
                    changed = True
                out.append(inst)
            if changed:
                blk.instructions = out
    return n


DEBUG_TAPS = False
DBG_HEAD = 0


def _build_nc():
    nc = bass.Bass("TRN2", target_bir_lowering=False, debug=False, num_devices=8)

    def din(name, shape, dt):
        return nc.dram_tensor(name, shape, dt, kind="ExternalInput").ap()

    xt = din("xt", [P, NKT, KV], F8)  # X[b].T tiled, virtual-padded
    xres = din("xres", [P, NKT, QTOK], F32)  # q tokens transposed, fp32
    valid16 = din("valid16", [P, NKT, NKT, 2], BF16)  # WS flag, [8hp x 2]
    wq = din("wq", [P, NKT, NKT, P], F8)  # [dpart, ot, kt, o], x WS
    wk = din("wk", [P, NKT, NKT, P], F8)
    wv = din("wv", [P, NKT, D], F8)  # rhs layout [dpart, kt, o], x WS
    w1 = din("w1", [P, NOT1, NKT, P], F8)  # fp8(WS*w1^T)
    w1e = din("w1e", [P, NOT1, NKT, P], F8)  # fp8 residual of the above
    w2 = din("w2", [P, NKT, NOT1, P], F8)
    w2e = din("w2e", [P, NKT, NOT1, P], F8)
    b1r = din("b1r", [P, NOT1], F32)  # SELU_S * b1
    b1e = din("b1e", [P, NOT1], F32)  # b1 + ln(SELU_S*SELU_A)
    b2t = din("b2t", [P, NKT], F32)
    g1t = din("g1t", [P, NKT], F32)
    be1t = din("be1t", [P, NKT], F32)
    g2t = din("g2t", [P, NKT], F32)
    be2t = din("be2t", [P, NKT], F32)
    out = nc.dram_tensor("out", [P, NKT, QTOK], F32, kind="ExternalOutput").ap()

    with PatchedTileContext(nc) as tc:
        import contextlib

        with contextlib.ExitStack() as ctx:
            persist = ctx.enter_context(tc.tile_pool(name="persist", bufs=1))
            bc = ctx.enter_context(tc.tile_pool(name="bc", bufs=1))
            wpool = ctx.enter_context(tc.tile_pool(name="wpool", bufs=4))
            tmp = ctx.enter_context(tc.tile_pool(name="tmp", bufs=2))
            tmp2 = ctx.enter_context(tc.tile_pool(name="tmp2", bufs=2))
            lnp = ctx.enter_context(tc.tile_pool(name="lnp", bufs=1))
            w1pool = ctx.enter_context(tc.tile_pool(name="w1pool", bufs=1))

            # ---- constants ----
            NW1B, NW2B = 5, 4
            w1bufs = [
                (
                    w1pool.tile(
                        [P, 2, NKT, P], F8, tag=f"w1{i}h", name=f"w1{i}h"
                    ),
                    w1pool.tile(
                        [P, 2, NKT, P], F8, tag=f"w1{i}e", name=f"w1{i}e"
                    ),
                )
                for i in range(NW1B)
            ]
            w2bufs = [
                (
                    w1pool.tile(
                        [P, NOT1, P], F8, tag=f"w2{i}h", name=f"w2{i}h"
                    ),
                    w1pool.tile(
                        [P, NOT1, P], F8, tag=f"w2{i}e", name=f"w2{i}e"
                    ),
                )
                for i in range(NW2B)
            ]

            def load_w1(chunk):
                if chunk < NOT1 // 2:
                    hb, lb = w1bufs[chunk % NW1B]
                    nc.sync.dma_start(out=hb[:], in_=w1[:, 2 * chunk : 2 * chunk + 2])
                    nc.sync.dma_start(out=lb[:], in_=w1e[:, 2 * chunk : 2 * chunk + 2])

            def load_w2(ot):
                if ot < NKT:
                    hb, lb = w2bufs[ot % NW2B]
                    nc.sync.dma_start(out=hb[:], in_=w2[:, ot])
                    nc.sync.dma_start(out=lb[:], in_=w2e[:, ot])
            ones128 = persist.tile([P, P], BF16, tag="ones128")
            nc.gpsimd.memset(ones128[:], 1.0)
            ones_r0 = persist.tile([P, P], BF16, tag="ones_r0")
            nc.gpsimd.memset(ones_r0[:], 0.0)
            nc.gpsimd.memset(ones_r0[0:1, :], 1.0)
            ones_r64 = persist.tile([P, P], BF16, tag="ones_r64")
            nc.gpsimd.memset(ones_r64[:], 0.0)
            nc.gpsimd.memset(ones_r64[64:65, :], 1.0)
            srowE_bf = persist.tile([P, QTOK], BF16, tag="srowEbf")
            nc.vector.memset(srowE_bf[:], 0.0)
            srowO_bf = persist.tile([P, QTOK], BF16, tag="srowObf")
            nc.vector.memset(srowO_bf[:], 0.0)
            eps_ap = persist.tile([P, 1], F32, tag="eps")
            nc.gpsimd.memset(eps_ap[:], LN_EPS)

            def ln_stats_mm(ps0, ps1, cast_t, sq_t, kt, n=NKT, ncols=QTOK):
                nc.tensor.matmul(
                    ps0[:, 0:ncols],
                    ones128[:],
                    cast_t[:],
                    start=(kt == 0),
                    stop=(kt == n - 1),
                )
                nc.tensor.matmul(
                    ps1[:, 0:ncols],
                    ones128[:],
                    sq_t[:],
                    start=(kt == 0),
                    stop=(kt == n - 1),
                )

            def ln_meanvar(ps0, ps1, ncols=QTOK):
                """stats psums -> (mean, rstd) broadcast tiles.

                rstd = exp(-0.5*ln(var+eps)): Ln and Exp share an Act table
                (natural_log_exp_and_others) with Relu/Identity/Square, so
                this never forces the 1.3us act-table reload that Sqrt would.
                """
                if ps1 is None:
                    # ps0 is a [P, 2*ncols] psum holding [sum | sumsq]:
                    # scale both with one DVE op into an adjacent pair
                    mv = bc.tile([P, 2 * ncols], F32, tag="meanvar")
                    nc.vector.tensor_scalar_mul(mv[:], ps0[:, 0 : 2 * ncols], 1.0 / D)
                    mean_bc, var_bc = mv[:, 0:ncols], mv[:, ncols : 2 * ncols]
                else:
                    mean_t = bc.tile([P, ncols], F32, tag="mean", name="mean_t")
                    mean_bc = mean_t[:]
                    nc.vector.tensor_scalar_mul(mean_bc, ps0[:, 0:ncols], 1.0 / D)
                    var_t = bc.tile([P, ncols], F32, tag="var", name="var_t")
                    var_bc = var_t[:]
                    nc.vector.tensor_scalar_mul(var_bc, ps1[:, 0:ncols], 1.0 / D)
                m2 = tmp2.tile([P, ncols], F32, tag="lnt")
                nc.vector.tensor_tensor(m2[:], mean_bc, mean_bc, OP.mult)
                nc.vector.tensor_tensor(var_bc, var_bc, m2[:], OP.subtract)
                nc.scalar.activation(var_bc, var_bc, AF.Ln, bias=eps_ap[:])
                nc.scalar.activation(var_bc, var_bc, AF.Exp, scale=-0.5)
                return mean_bc, var_bc

            def ln_apply(
                src_kt, mean_bc, var_bc, g_ap, b_ap, dst_kt, kt, ncols=QTOK,
                eng=None, fin_act=None,
            ):
                eng = eng or nc.vector
                t1 = tmp2.tile([P, ncols], F32, tag="lnt")
                eng.tensor_tensor(t1[:], src_kt, mean_bc[:], OP.subtract)
                eng.tensor_tensor(t1[:], t1[:], var_bc[:], OP.mult)
                if fin_act if fin_act is not None else (kt % 2 == 0):
                    nc.scalar.activation(
                        dst_kt,
                        t1[:],
                        AF.Identity,
                        scale=g_ap[:, kt : kt + 1],
                        bias=b_ap[:, kt : kt + 1],
                    )
                else:
                    nc.vector.tensor_scalar(
                        dst_kt,
                        t1[:],
                        g_ap[:, kt : kt + 1],
                        b_ap[:, kt : kt + 1],
                        OP.mult,
                        OP.add,
                    )

            # ---- phase 1+2 fused: QKV projections + attention ----
            with tc.tile_pool(name="pproj", bufs=1) as pproj:
                import contextlib as _ctl

                attn_stack = _ctl.ExitStack()
                pattn = attn_stack.enter_context(tc.tile_pool(name="pattn", bufs=3))
                ps_ctx = attn_stack.enter_context(
                    tc.tile_pool(name="ps_ctx", bufs=2, space="PSUM")
                )
                ps_sc = attn_stack.enter_context(
                    tc.tile_pool(name="ps_sc", bufs=2, space="PSUM")
                )
                ps_fill = attn_stack.enter_context(
                    tc.tile_pool(name="ps_fill", bufs=2, space="PSUM")
                )
                pxstack = _ctl.ExitStack()
                px = pxstack.enter_context(tc.tile_pool(name="px", bufs=1))
                wq_t0 = wpool.tile([P, NKT, P], F8, tag="wqkv")
                nc.sync.dma_start(out=wq_t0[:], in_=wq[:, 0])
                xt_s = px.tile([P, NKT, KV], F8, tag="xt")
                nc.sync.dma_start(out=xt_s[:, 0:4], in_=xt[:, 0:4])
                nc.sync.dma_start(out=xt_s[:, 4:8], in_=xt[:, 4:8])
                wk_t0 = wpool.tile([P, NKT, P], F8, tag="wqkv")
                nc.sync.dma_start(out=wk_t0[:], in_=wk[:, 0])
                # Q/K in fp8, scores-DR grouped layout: tile ot = (u, c)
                # with u = ot//2 (head group 4u..4u+3), c = ot%2 (dh parity);
                # partition 32*g+ki holds head 4u+g, dh = 2*ki + c.
                qt_s = pproj.tile([P, NKT, QTOK], F8, tag="qt")
                kt_s = pproj.tile([P, NKT, KV], F8, tag="kt")
                # augmented V: per (kv-tile j, head pair hp) 193 cols:
                # [Edims 64 | Eden 1 | Oden 1 | zeros 63 | Odims 64]
                vaug = pproj.tile([P, NKT, NKT, VW], BF16, tag="vaug")
                nc.gpsimd.memset(vaug[:, :, :, 66:129], 0.0)

                wv_s = px.tile([P, NKT, D], F8, tag="wv")
                nc.sync.dma_start(out=wv_s[:], in_=wv[:])
                val_s = pproj.tile([P, NKT, NKT, 2], BF16, tag="val")
                nc.sync.dma_start(out=val_s[:], in_=valid16[:])
                xres_s = pproj.tile([P, NKT, QTOK], F32, tag="xres")
                b1r_s = persist.tile([P, NOT1], F32, tag="b1r")
                b1e_s = persist.tile([P, NOT1], F32, tag="b1e")
                small = {}
                sum1_sb = pproj.tile([P, 512], F32, tag="sum1")
                sumsq1_sb = pproj.tile([P, 512], F32, tag="sumsq1")

                qk_done = set()
                v_done = {0: 0, 1: 0}

                def emit_qproj(ot):
                    if ot == 0:
                        wq_t = wq_t0
                    else:
                        wq_t = wpool.tile([P, NKT, P], F8, tag="wqkv")
                        nc.sync.dma_start(out=wq_t[:], in_=wq[:, ot])
                    ps = ps_fill.tile([P, 512], F32, tag="fill")
                    for kp in range(NKT // 2):
                        nc.tensor.matmul(
                            ps[:],
                            wq_t[:, 2 * kp : 2 * kp + 2],
                            xt_s[:, 2 * kp : 2 * kp + 2, 512:1024],
                            start=(kp == 0),
                            stop=(kp == NKT // 2 - 1),
                            perf_mode=DR,
                        )
                    nc.vector.tensor_copy(qt_s[:, ot], ps[:])

                wk_ts = {0: wk_t0}

                def emit_kproj(ot, tb):
                    if tb == 0 and ot not in wk_ts:
                        wk_t = wpool.tile([P, NKT, P], F8, tag="wqkv")
                        nc.sync.dma_start(out=wk_t[:], in_=wk[:, ot])
                        wk_ts[ot] = wk_t
                    wk_t = wk_ts[ot]
                    ps = ps_fill.tile([P, 512], F32, tag="fill")
                    for kp in range(NKT // 2):
                        nc.tensor.matmul(
                            ps[:],
                            wk_t[:, 2 * kp : 2 * kp + 2],
                            xt_s[:, 2 * kp : 2 * kp + 2, tb * 512 : (tb + 1) * 512],
                            start=(kp == 0),
                            stop=(kp == NKT // 2 - 1),
                            perf_mode=DR,
                        )
                    if tb == 0:
                        nc.scalar.copy(kt_s[:, ot, 0:512], ps[:])
                    else:
                        nc.vector.tensor_copy(kt_s[:, ot, 512:1024], ps[:])
                        qk_done.add(ot)

                def emit_vproj(db, tk):
                    if db == 0:
                        # den columns for all 8 pairs x 2 parities
                        nc.vector.tensor_copy(vaug[:, tk, :, 64:66], val_s[:, tk])
                    ps = ps_fill.tile([P, 4, P], F32, tag="fill")
                    for kp in range(NKT // 2):
                        nc.tensor.matmul(
                            ps[:, :, :],
                            xt_s[:, 2 * kp : 2 * kp + 2, tk * P : (tk + 1) * P],
                            wv_s[:, 2 * kp : 2 * kp + 2, db * 512 : (db + 1) * 512],
                            start=(kp == 0),
                            stop=(kp == NKT // 2 - 1),
                            perf_mode=DR,
                        )
                    hp0 = db * 4
                    nc.vector.tensor_copy(
                        vaug[:, tk, hp0 : hp0 + 4, 0:64], ps[:, :, 0:64]
                    )
                    nc.vector.tensor_copy(
                        vaug[:, tk, hp0 : hp0 + 4, 129:193], ps[:, :, 64:128]
                    )
                    v_done[db] += 1

                stats_pending = []

                def flush_stats():
                    while stats_pending:
                        cast_t, sq_t, hp = stats_pending.pop(0)
                        pss = ps_sc.tile([P, 1024], F32, tag="sc")
                        nc.tensor.matmul(
                            pss[:, 0:512], ones128[:], cast_t[:], start=True, stop=True
                        )
                        nc.tensor.matmul(
                            pss[:, 512:1024], ones128[:], sq_t[:], start=True, stop=True
                        )
                        if hp == 0:
                            nc.vector.tensor_copy(sum1_sb[:], pss[:, 0:512])
                            nc.vector.tensor_copy(sumsq1_sb[:], pss[:, 512:1024])
                        else:
                            nc.vector.tensor_tensor(
                                sum1_sb[:], sum1_sb[:], pss[:, 0:512], OP.add
                            )
                            nc.vector.tensor_tensor(
                                sumsq1_sb[:], sumsq1_sb[:], pss[:, 512:1024], OP.add
                            )

                def emit_post(p):
                    h, cps, ctxn = p
                    hp, par = h // 2, h % 2
                    po = 64 * par
                    flush_stats()
                    # broadcast the bf16 reciprocal row across the 64 ctx
                    # partitions via a ones-matmul (bcp shares the fill pool)
                    bcp = ps_fill.tile([P, 512], F32, tag="fill")
                    if par == 0:
                        nc.tensor.matmul(
                            bcp[0:64],
                            ones_r64[:, 0:64],
                            srowE_bf[:],
                            start=True,
                            stop=True,
                        )
                    else:
                        nc.tensor.matmul(
                            bcp[64:128],
                            ones_r0[:, 0:64],
                            srowO_bf[:],
                            start=True,
                            stop=True,
                        )
                    bc_sb = tmp2.tile([P, 512], BF16, tag="bcsb")
                    nc.vector.tensor_copy(bc_sb[po : po + 64], bcp[po : po + 64])
                    nc.vector.tensor_tensor(
                        ctxn[po : po + 64],
                        cps[po : po + 64],
                        bc_sb[po : po + 64],
                        OP.mult,
                    )
                    if par == 1:
                        # pair finished: residual add + LN1 stats (Pool
                        # takes the add + cast + square; PE the stats,
                        # deferred one slot so PE never waits on Pool).
                        # Last pair runs on DVE: Pool's 0.42 efficiency
                        # would sit on the attention->LN1 critical path.
                        eng = nc.vector if hp == H // 2 - 1 else nc.gpsimd
                        eng.tensor_add(
                            xres_s[:, hp], xres_s[:, hp], ctxn[:]
                        )
                        cast_t = tmp.tile([P, 512], BF16, tag="lncast")
                        sq_t = tmp.tile([P, 512], BF16, tag="lnsq")
                        eng.tensor_copy(cast_t[:], xres_s[:, hp])
                        eng.tensor_mul(
                            sq_t[:], xres_s[:, hp], xres_s[:, hp]
                        )
                        stats_pending.append((cast_t, sq_t, hp))

                ctxn = None
                # packed expt: per-j live query range [off_j, 512) stored
                # contiguously; POFF[j] is the packed start, NCOL[j] the width
                NCOL = [512 - max(0, j - 4) * P for j in range(NKT)]
                POFF = [0] * NKT
                for j in range(1, NKT):
                    POFF[j] = POFF[j - 1] + NCOL[j - 1]

                def emit_ctx(pr):
                    nonlocal ctxn
                    h, expt = pr
                    hp, par = h // 2, h % 2
                    cps = ps_ctx.tile([P, 512], F32, tag="ctx")
                    lsl = (0, 65) if par == 0 else (65, VW)
                    m = lsl[1] - lsl[0]
                    for j in range(NKT):
                        off = max(0, j - 4) * P
                        nc.tensor.matmul(
                            cps[0:m, off:512],
                            vaug[:, j, hp, lsl[0] : lsl[1]],
                            expt[:, POFF[j] : POFF[j] + NCOL[j]],
                            start=(j == 0),
                            stop=(j == NKT - 1),
                        )
                    with nc.allow_low_precision(
                        reason="softmax denominator reciprocal to bf16"
                    ):
                        if par == 0:
                            nc.vector.reciprocal(srowE_bf[64:65], cps[64:65])
                        else:
                            nc.vector.reciprocal(srowO_bf[0:1], cps[0:1])
                    if DEBUG_TAPS and h == DBG_HEAD:
                        dbg_cps = nc.dram_tensor(
                            "dbg_cps", [P, 512], F32, kind="ExternalOutput"
                        ).ap()
                        dbg_sb = persist.tile([P, 512], F32, tag="dbgsb")
                        nc.vector.memset(dbg_sb[:], 0.0)
                        _r0, _r1 = (0, 65) if par == 0 else (64, 128)
                        nc.vector.tensor_copy(dbg_sb[_r0:_r1], cps[_r0:_r1])
                        if par == 1:
                            nc.vector.tensor_copy(dbg_sb[0:1], cps[0:1])
                        nc.sync.dma_start(out=dbg_cps[:], in_=dbg_sb[:])
                        dbg_expt = nc.dram_tensor(
                            "dbg_expt", [P, 3328], BF16, kind="ExternalOutput"
                        ).ap()
                        nc.sync.dma_start(out=dbg_expt[:], in_=expt[:])
                        dbg_vaug = nc.dram_tensor(
                            "dbg_vaug", [P, NKT, VW], BF16, kind="ExternalOutput"
                        ).ap()
                        nc.sync.dma_start(out=dbg_vaug[:], in_=vaug[:, :, hp])
                    if par == 0:
                        ctxn = tmp2.tile([P, 512], F32, tag="ctxn")
                    return (h, cps, ctxn)

                def emit_scores(h):
                    # DoubleRow over dh: contraction (ki 32, parity 2); head
                    # h lives at partition group 32*(h%4) of ot pair
                    # (2*(h//4), 2*(h//4)+1). j-tiles are computed two per
                    # 2-bank psum so each Exp covers a pair in one shot.
                    u, sub = h // 4, h % 4
                    b0 = 32 * sub
                    expt = pattn.tile([P, 3328], BF16, tag="expt")
                    for pj in range(4):
                        j0 = 2 * pj
                        w0, w1 = NCOL[j0], NCOL[j0 + 1]
                        # two independent accumulation groups must not share
                        # a PSUM bank: place the second j at a 512 offset
                        po1 = max(w0, 512)
                        ps = ps_sc.tile([P, 1024], F32, tag="sc")
                        for j, w, po in ((j0, w0, 0), (j0 + 1, w1, po1)):
                            off = 512 - w
                            nc.tensor.matmul(
                                ps[:, po : po + w],
                                kt_s[b0 : b0 + 32, 2 * u : 2 * u + 2,
                                     j * P : (j + 1) * P],
                                qt_s[b0 : b0 + 32, 2 * u : 2 * u + 2, off:512],
                                start=True,
                                stop=True,
                                perf_mode=DR,
                                tile_position=(b0, 0),
                            )
                        if po1 == w0:
                            nc.scalar.activation(
                                expt[:, POFF[j0] : POFF[j0] + w0 + w1],
                                ps[:, 0 : w0 + w1],
                                AF.Exp,
                                scale=0.125 / (WSQK * WSQK),
                            )
                        else:
                            nc.scalar.activation(
                                expt[:, POFF[j0] : POFF[j0] + w0],
                                ps[:, 0:w0],
                                AF.Exp,
                                scale=0.125 / (WSQK * WSQK),
                            )
                            nc.scalar.activation(
                                expt[:, POFF[j0 + 1] : POFF[j0 + 1] + w1],
                                ps[:, po1 : po1 + w1],
                                AF.Exp,
                                scale=0.125 / (WSQK * WSQK),
                            )
                    for j in range(4, NKT):
                        # zero the masked upper triangle of the diagonal
                        # query block post-exp (Pool, off the hot engines)
                        nc.gpsimd.affine_select(
                            out=expt[:, POFF[j] : POFF[j] + P],
                            in_=expt[:, POFF[j] : POFF[j] + P],
                            compare_op=OP.is_ge,
                            fill=0.0,
                            base=0,
                            pattern=[[1, P]],
                            channel_multiplier=-1,
                        )
                    return (h, expt)

                # filler units: Q/K projections + V-proj tiles, ordered by
                # consumption deadline, drained during the head loop
                fillers = []
                for ot in (2, 3):
                    fillers += [
                        lambda o=ot: emit_qproj(o),
                        lambda o=ot: emit_kproj(o, 0),
                        lambda o=ot: emit_kproj(o, 1),
                    ]
                fillers += [lambda t=tk: emit_vproj(0, t) for tk in range(NKT)]
                for ot in (4, 5):
                    fillers += [
                        lambda o=ot: emit_qproj(o),
                        lambda o=ot: emit_kproj(o, 0),
                        lambda o=ot: emit_kproj(o, 1),
                    ]
                fillers += [lambda t=tk: emit_vproj(1, t) for tk in range(NKT)]
                for ot in (6, 7):
                    fillers += [
                        lambda o=ot: emit_qproj(o),
                        lambda o=ot: emit_kproj(o, 0),
                        lambda o=ot: emit_kproj(o, 1),
                    ]
                fillers.reverse()  # consume via pop()

                def drain(n):
                    for _ in range(n):
                        if fillers:
                            fillers.pop()()

                def need_qk(u):
                    # scores for head group u needs both parity tiles 2u, 2u+1
                    while not ({2 * u, 2 * u + 1} <= qk_done):
                        assert fillers, f"filler queue dry before qk pair {u}"
                        fillers.pop()()

                def need_v(db):
                    while v_done[db] < NKT:
                        assert fillers, f"filler queue dry before v {db}"
                        fillers.pop()()

                # warmup: head-group-0 Q/K, then 3 heads of scores while the
                # filler queue builds V/QK state; first ctx after V db0 done
                emit_qproj(0)
                emit_kproj(0, 0)
                emit_kproj(0, 1)
                emit_qproj(1)
                emit_kproj(1, 0)
                emit_kproj(1, 1)
                from collections import deque

                prevs = deque()
                prevs.append(emit_scores(0))
                drain(3)
                prevs.append(emit_scores(1))
                nc.sync.dma_start(out=xres_s[:], in_=xres[:])
                drain(3)
                prevs.append(emit_scores(2))
                drain(4)
                need_v(0)
                nc.sync.dma_start(out=b1r_s[:], in_=b1r[:])
                nc.sync.dma_start(out=b1e_s[:], in_=b1e[:])
                for nm, _src in (
                    ("b2t", b2t),
                    ("g1t", g1t),
                    ("be1t", be1t),
                    ("g2t", g2t),
                    ("be2t", be2t),
                ):
                    t = persist.tile([P, NKT], F32, tag=nm)
                    nc.sync.dma_start(out=t[:], in_=_src[:])
                    small[nm] = t
                # prefetch the first fc1/fc2 weight chunks during attention
                # so the FFN phases never wait on the serial SP DMA queue
                for _c in range(3):
                    load_w1(_c)
                load_w2(0)
                load_w2(1)
                pending = emit_ctx(prevs.popleft())
                for h in range(3, H):
                    need_qk(h // 4)
                    prevs.append(emit_scores(h))
                    drain(1)
                    emit_post(pending)
                    nh = prevs[0][0]
                    need_v(nh // 8)
                    pending = emit_ctx(prevs.popleft())
                drain(len(fillers))
                while prevs:
                    emit_post(pending)
                    need_v(1)
                    pending = emit_ctx(prevs.popleft())
                emit_post(pending)
                flush_stats()

                if DEBUG_TAPS:
                    dbg_xres = nc.dram_tensor(
                        "dbg_xres", [P, NKT, QTOK], F32, kind="ExternalOutput"
                    ).ap()
                    nc.sync.dma_start(out=dbg_xres[:], in_=xres_s[:])
                    dbg_xt = nc.dram_tensor(
                        "dbg_xt", [P, NKT, KV], F8, kind="ExternalOutput"
                    ).ap()
                    nc.sync.dma_start(out=dbg_xt[:], in_=xt_s[:])
                    dbg_kt = nc.dram_tensor(
                        "dbg_kt", [P, NKT, KV], F8, kind="ExternalOutput"
                    ).ap()
                    nc.sync.dma_start(out=dbg_kt[:], in_=kt_s[:])
                    dbg_stats = nc.dram_tensor(
                        "dbg_stats", [P, 2, 512], F32, kind="ExternalOutput"
                    ).ap()
                    nc.sync.dma_start(out=dbg_stats[:, 0], in_=sum1_sb[:])
                    nc.sync.dma_start(out=dbg_stats[:, 1], in_=sumsq1_sb[:])
                    dbg_srow = nc.dram_tensor(
                        "dbg_srow", [P, 2, QTOK], BF16, kind="ExternalOutput"
                    ).ap()
                    nc.sync.dma_start(out=dbg_srow[:, 0], in_=srowE_bf[:])
                    nc.sync.dma_start(out=dbg_srow[:, 1], in_=srowO_bf[:])

                # ---- phase 3: LN1 (stats already accumulated) ----
                # ln1_bf (bf16) is the fc2 residual; x8/x8e are the fp8
                # hi/lo pair feeding the compensated fc1 DoubleRow passes.
                ln1_bf = lnp.tile([P, NKT, QTOK], BF16, tag="ln1")
                x8 = lnp.tile([P, NKT, QTOK], F8, tag="x8")
                x8e = lnp.tile([P, NKT, QTOK], F8, tag="x8e")
                mean1, rstd1 = ln_meanvar(sum1_sb, sumsq1_sb)
                for kt in range(NKT):
                    ln_apply(
                        xres_s[:, kt], mean1, rstd1,
                        small["g1t"], small["be1t"], ln1_bf[:, kt], kt,
                    )
                    nc.gpsimd.tensor_copy(x8[:, kt], ln1_bf[:, kt])
                    nc.vector.tensor_tensor(
                        x8e[:, kt], ln1_bf[:, kt], x8[:, kt], OP.subtract
                    )
                pxstack.close()
                attn_stack.close()

            # ---- phase 4: fc1 + selu (w1 in JIT 4-ot chunks, depth 2) ----
            pffn_stack = contextlib.ExitStack()
            pffn = pffn_stack.enter_context(tc.tile_pool(name="pffn", bufs=1))
            ps_mm = pffn_stack.enter_context(
                tc.tile_pool(name="ps_mm", bufs=3, space="PSUM")
            )
            ps_x = pffn_stack.enter_context(
                tc.tile_pool(name="ps_x", bufs=1, space="PSUM")
            )
            # deep ring for the selu temporaries: with only 2 bufs the
            # Relu of ot must wait for Pool's h8 copy of ot-2 (slot reuse),
            # putting ~1us of Pool/Act latency on the PE critical path
            pselu = pffn_stack.enter_context(tc.tile_pool(name="pselu", bufs=4))
            h8 = pffn.tile([P, NOT1, QTOK], F8, tag="h8")
            h8e = pffn.tile([P, NOT1, QTOK], F8, tag="h8e")
            # ots 0..3 run kp-major across four live psums so each matmul
            # group consumes x8/x8e kt-pairs as LN1 streams them out --
            # otherwise the first psum group alone needs the full x8 tile
            # and the PE idles through the whole LN1 quant trench
            ps03 = []
            for ot in range(4):
                w1h, w1l = w1bufs[(ot // 2) % NW1B]
                if ot == 3:
                    psi = ps_x.tile([P, 512], F32, tag="x", name=f"ps03_{ot}")
                else:
                    psi = ps_mm.tile([P, 512], F32, tag="mm", name=f"ps03_{ot}")
                ps03.append((psi, w1h, w1l))
            load_w1(3)
            load_w1(4)
            for kp in range(NKT // 2):
                for ot in range(4):
                    psi, w1h, w1l = ps03[ot]
                    for pi, xq_w in enumerate(((w1h, x8), (w1l, x8), (w1h, x8e))):
                        wt, xq = xq_w
                        nc.tensor.matmul(
                            psi[:],
                            wt[:, ot % 2, 2 * kp : 2 * kp + 2],
                            xq[:, 2 * kp : 2 * kp + 2],
                            start=(kp == 0 and pi == 0),
                            stop=(kp == NKT // 2 - 1 and pi == 2),
                            perf_mode=DR,
                        )
            for ot in range(NOT1):
                if ot < 4:
                    ps = ps03[ot][0]
                else:
                    w1h, w1l = w1bufs[(ot // 2) % NW1B]
                    if ot % 2 == 0:
                        load_w1(ot // 2 + 3)
                    if ot % 4 == 3:
                        ps = ps_x.tile([P, 512], F32, tag="x")
                    else:
                        ps = ps_mm.tile([P, 512], F32, tag="mm")
                    passes = [(w1h, x8), (w1l, x8), (w1h, x8e)]
                    for pi, (wt, xq) in enumerate(passes):
                        for kp in range(NKT // 2):
                            nc.tensor.matmul(
                                ps[:],
                                wt[:, ot % 2, 2 * kp : 2 * kp + 2],
                                xq[:, 2 * kp : 2 * kp + 2],
                                start=(pi == 0 and kp == 0),
                                stop=(pi == 2 and kp == NKT // 2 - 1),
                                perf_mode=DR,
                            )
                p_t = pselu.tile([P, QTOK], F32, tag="selup")
                nc.scalar.activation(
                    p_t[:],
                    ps[:],
                    AF.Relu,
                    scale=SELU_S / WS,
                    bias=b1r_s[:, ot : ot + 1],
                )
                e_t = pselu.tile([P, QTOK], F32, tag="selue")
                nc.scalar.activation(
                    e_t[:], ps[:], AF.Exp, scale=1.0 / WS, bias=b1e_s[:, ot : ot + 1]
                )
                # selu(z) = min(sa*e^z - sa, s*relu(z))
                nc.vector.scalar_tensor_tensor(
                    p_t[:], e_t[:], SELU_SA, p_t[:], OP.subtract, OP.min
                )
                nc.gpsimd.tensor_copy(h8[:, ot], p_t[:])
                nc.vector.tensor_tensor(h8e[:, ot], p_t[:], h8[:, ot], OP.subtract)

            # ---- phase 5: fc2 + residual + LN2 + store (full 512 width) ----
            ps_stat2 = pffn_stack.enter_context(
                tc.tile_pool(name="ps_stat2", bufs=2, space="PSUM")
            )
            res2 = pffn.tile([P, NKT, QTOK], F32, tag="res2")
            ps0_2 = ps_stat2.tile([P, 512], F32, tag="stat2")
            ps1_2 = ps_stat2.tile([P, 512], F32, tag="stat2")
            for ot in range(NKT):
                w2h, w2l = w2bufs[ot % NW2B]
                load_w2(ot + 2)  # ots 0,1 preloaded in attention
                if ot % 4 == 3:
                    ps = ps_x.tile([P, 512], F32, tag="x")
                else:
                    ps = ps_mm.tile([P, 512], F32, tag="mm")
                passes = [(w2h, h8), (w2l, h8), (w2h, h8e)]
                for pi, (wt, hq) in enumerate(passes):
                    for kp in range(NOT1 // 2):
                        nc.tensor.matmul(
                            ps[:],
                            wt[:, 2 * kp : 2 * kp + 2],
                            hq[:, 2 * kp : 2 * kp + 2],
                            start=(pi == 0 and kp == 0),
                            stop=(pi == 2 and kp == NOT1 // 2 - 1),
                            perf_mode=DR,
                        )
                t1 = tmp2.tile([P, QTOK], F32, tag="r2t")
                nc.scalar.activation(
                    t1[:],
                    ps[:],
                    AF.Identity,
                    scale=1.0 / WS,
                    bias=small["b2t"][:, ot : ot + 1],
                )
                nc.vector.tensor_tensor(
                    res2[:, ot], t1[:], ln1_bf[:, ot], OP.add
                )
                cast_t = tmp.tile([P, QTOK], BF16, tag="lncast2")
                sq_t = tmp.tile([P, QTOK], BF16, tag="lnsq2")
                nc.vector.tensor_copy(cast_t[:], res2[:, ot])
                nc.scalar.activation(sq_t[:], res2[:, ot], AF.Square)
                ln_stats_mm(ps0_2, ps1_2, cast_t, sq_t, ot)
            mean2, rstd2 = ln_meanvar(ps0_2, ps1_2)
            for kt in range(NKT):
                # spread the tail normalize across DVE and Pool so the
                # final 8-tile chain isn't serialized on one engine; the
                # scale-bias always runs on the otherwise-idle Act engine
                eng = nc.gpsimd if kt in (2, 5) else nc.vector
                ln_apply(
                    res2[:, kt], mean2, rstd2,
                    small["g2t"], small["be2t"], res2[:, kt], kt,
                    eng=eng,
                )
                nc.sync.dma_start(out=out[:, kt], in_=res2[:, kt])
            pffn_stack.close()

    _legalize_waits(nc)
    return nc


_NC_CACHE = None
TRACE = False
LAST_EXEC_NS = None


def _get_nc():
    global _NC_CACHE
    if _NC_CACHE is None:
        _NC_CACHE = _build_nc()
    return _NC_CACHE


def _tile_w(a):
    """[Din, O] -> [P, O//P(ot), Din//P(kt), P] with ot-contiguous DMA slices."""
    Din, O = a.shape
    return np.ascontiguousarray(
        a.reshape(Din // P, P, O // P, P).transpose(1, 2, 0, 3)
    )


def _pp(v, n):
    """[n*P] -> [P, n] per-partition layout."""
    return np.ascontiguousarray(v.reshape(n, P).T)


def kernel(X, wq, wk, wv, ln1_g, ln1_b, w1, b1, w2, b2, ln2_g, ln2_b):
    from concourse.bass_utils import run_bass_kernel_spmd

    X = np.asarray(X, np.float32)
    bf = ml_dtypes.bfloat16
    f8 = ml_dtypes.float8_e4m3  # IEEE flavor — matches bass float8e4

    def hilo(wt):
        hi = wt.astype(f8)
        lo = (wt - hi.astype(np.float32)).astype(f8)
        return hi, lo

    # scores-DR out-dim permutation: slot (ot, i) holds projection row
    # head*64 + dh with head = 4*(ot//2) + i//32, dh = 2*(i%32) + ot%2,
    # so head h sits at partition group 32*(h%4) of tiles (2u, 2u+1)
    # with the dh parity split across the tile pair (DoubleRow Ko dim).
    qperm = np.empty(D, np.int64)
    for _ot in range(NKT):
        for _i in range(P):
            _h = 4 * (_ot // 2) + _i // 32
            _dh = 2 * (_i % 32) + (_ot % 2)
            qperm[_ot * P + _i] = _h * 64 + _dh
    wqT = _tile_w((WSQK * np.asarray(wq, np.float32).T)[:, qperm]).astype(f8)
    wkT = _tile_w((WSQK * np.asarray(wk, np.float32).T)[:, qperm]).astype(f8)
    wvT = np.ascontiguousarray(
        WS * np.asarray(wv, np.float32).T.reshape(NKT, P, D).transpose(1, 0, 2)
    ).astype(f8)
    w1hi, w1lo = hilo(_tile_w(WS * np.asarray(w1, np.float32).T))
    w2hi, w2lo = hilo(_tile_w(WS * np.asarray(w2, np.float32).T))
    b1 = np.asarray(b1, np.float32)
    shared = dict(
        wq=wqT,
        wk=wkT,
        wv=wvT,
        w1=w1hi,
        w1e=w1lo,
        w2=w2hi,
        w2e=w2lo,
        b1r=_pp(SELU_S * b1, NOT1),
        b1e=_pp(b1 + LN_SA, NOT1),
        b2t=_pp(np.asarray(b2, np.float32), NKT),
        g1t=_pp(np.asarray(ln1_g, np.float32), NKT),
        be1t=_pp(np.asarray(ln1_b, np.float32), NKT),
        g2t=_pp(np.asarray(ln2_g, np.float32), NKT),
        be2t=_pp(np.asarray(ln2_b, np.float32), NKT),
    )

    in_maps = []
    for c in range(8):
        b, hf = c // 2, c % 2
        if hf == 1:
            xkv = X[b].T  # [D, L]
            valid = np.full(KV, WS, np.float32)
            xq = X[b, 512:]
        else:
            xkv = np.concatenate(
                [np.zeros((D, 512), np.float32), X[b, :512].T], axis=1
            )
            valid = np.concatenate(
                [np.zeros(512, np.float32), np.full(512, WS, np.float32)]
            )
            xq = X[b, :512]
        xt = (
            np.ascontiguousarray(xkv.reshape(NKT, P, KV).transpose(1, 0, 2))
        ).astype(f8)
        xres = np.ascontiguousarray(xq.T.reshape(NKT, P, QTOK).transpose(1, 0, 2))
        vt = valid.reshape(NKT, P).T  # [P, NKT]
        val16 = (
            np.repeat(vt[:, :, None], H, axis=2).reshape(P, NKT, NKT, 2).astype(bf)
        )
        m = dict(shared)
        m.update(xt=xt, xres=xres, valid16=np.ascontiguousarray(val16))
        in_maps.append(m)

    nc = _get_nc()
    global LAST_EXEC_NS
    if TRACE:
        res = run_bass_kernel_spmd(nc, in_maps, list(range(8)), trace=True)
        LAST_EXEC_NS = res.exec_time_ns
    else:
        res = run_bass_kernel_spmd(nc, in_maps, list(range(8)))

    out = np.empty((B, L, D), np.float32)
    for c in range(8):
        b, hf = c // 2, c % 2
        o = res.results[c]["out"]  # [P, NKT, QTOK]
        o = o.transpose(1, 0, 2).reshape(D, QTOK).T  # [QTOK, D]
        out[b, hf * 512 : hf * 512 + 512] = o
    return out

